# revision 1
# baseline (speedup 1.0000x reference)
"""Trainium2 Bass kernel for nn_CrossAttention (seq_len==1 cross attention,
dual-stream transformer block pair).

Math notes (exact simplifications, valid for any input values):
  - Both attentions have seq_len==1 for q and kv, so softmax over the single
    kv position is exactly 1.0 and attention output == V projection:
        mha(q_in, kv_in) = (kv_in @ wv.T + bv) @ out_w.T + out_b
    The q/k projections are dead code.  Folding the two matmuls:
        attn = kv_in @ (out_w @ wv).T + (out_w @ bv + out_b)
  - LayerNorm affine (g, b) of ln1/ln2 is folded into the following FFN
    weights host-side; residual-path affine and biases are applied on-device
    only when they are non-trivial (they are zeros/ones for the reference
    setup_inputs, so the fast path emits no extra instructions).

Per-core layout (pure data parallel over the batch dim, 8 cores):
  rows/core = 4096, macro tile R = 256 rows (2 chunks of 128 partitions).
  Per macro tile and stream s (s=0: dna attends mol; s=1: mol attends dna):
    attn   : psum[rows,512] += molT_chunk.T @ W.T_chunk          (4 K-chunks)
    h1pre  : psum + x (residual, fp32)
    LN1    : bn_stats/bn_aggr -> mean/var; z = (h1pre-m)*rsqrt(var+eps) [bf16]
    zT     : z -> DRAM -> DMA-xbar transpose back (bf16)
    FFN1   : gT[j][128od, R] += U.T chunk.T @ zT chunk; relu-evict to bf16
    FFN2   : f[rows,512] += gT chunk.T @ V.T chunk
    LN2    : y = f + z; out = (y - m)*rsqrt(var+eps) -> DMA to output half
"""

import numpy as np
import ml_dtypes
from contextlib import ExitStack

import concourse.bass as bass
import concourse.tile as tile
from concourse import bacc, mybir
from concourse.bass_utils import run_bass_kernel_spmd

E = 512
HID = 1024
NCORES = 8
EPS = 1e-5
P = 128

BF16 = mybir.dt.bfloat16
F32 = mybir.dt.float32
BF = ml_dtypes.bfloat16

_prog_cache = {}


def _build_program(rows_per_core: int, rmacro: int, flags: tuple):
    """Build + compile the per-core Bass program.

    flags = (use_c0, use_c1, use_d0, use_d1, use_e0, use_e1,
             aff_a0, aff_a1, aff_b0, aff_b1)
    """
    (use_c0, use_c1, use_d0, use_d1, use_e0, use_e1,
     aff_a0, aff_a1, aff_b0, aff_b1) = flags
    use_c = (use_c0, use_c1)
    use_d = (use_d0, use_d1)
    use_e = (use_e0, use_e1)
    aff_a = (aff_a0, aff_a1)
    aff_b = (aff_b0, aff_b1)

    R = rmacro
    NT = rows_per_core // R
    RC = R // P
    KE = E // P    # 4 K-chunks over E
    KH = HID // P  # 8 K-chunks over HID

    nc = bacc.Bacc("TRN2", target_bir_lowering=False, debug=False,
                   num_devices=NCORES)

    dna = nc.dram_tensor("dna", [NT, RC, P, E], F32, kind="ExternalInput").ap()
    mol = nc.dram_tensor("mol", [NT, RC, P, E], F32, kind="ExternalInput").ap()
    out = nc.dram_tensor("out", [NT, RC, P, 2 * E], F32,
                         kind="ExternalOutput").ap()

    wts = {}
    for s in range(2):
        wts[f"w{s}"] = nc.dram_tensor(f"w{s}", [P, KE, E], BF16,
                                      kind="ExternalInput").ap()
        wts[f"u{s}"] = nc.dram_tensor(f"u{s}", [P, KE, HID], BF16,
                                      kind="ExternalInput").ap()
        wts[f"v{s}"] = nc.dram_tensor(f"v{s}", [P, KH, E], BF16,
                                      kind="ExternalInput").ap()
        if use_c[s]:
            wts[f"c{s}"] = nc.dram_tensor(f"c{s}", [1, E], BF16,
                                          kind="ExternalInput").ap()
        if use_d[s]:
            wts[f"d{s}"] = nc.dram_tensor(f"d{s}", [1, HID], BF16,
                                          kind="ExternalInput").ap()
        if use_e[s]:
            wts[f"e{s}"] = nc.dram_tensor(f"e{s}", [1, E], BF16,
                                          kind="ExternalInput").ap()
        if aff_a[s]:
            wts[f"ga{s}"] = nc.dram_tensor(f"ga{s}", [1, E], BF16,
                                           kind="ExternalInput").ap()
            wts[f"ba{s}"] = nc.dram_tensor(f"ba{s}", [1, E], BF16,
                                           kind="ExternalInput").ap()
        if aff_b[s]:
            wts[f"gb{s}"] = nc.dram_tensor(f"gb{s}", [1, E], F32,
                                           kind="ExternalInput").ap()
            wts[f"bb{s}"] = nc.dram_tensor(f"bb{s}", [1, E], F32,
                                           kind="ExternalInput").ap()

    SUB = mybir.AluOpType.subtract
    MULT = mybir.AluOpType.mult
    Relu = mybir.ActivationFunctionType.Relu
    Sqrt = mybir.ActivationFunctionType.Sqrt

    with tile.TileContext(nc) as tc:
        with ExitStack() as ctx:
            const = ctx.enter_context(tc.tile_pool(name="const", bufs=1))
            xin = ctx.enter_context(tc.tile_pool(name="xin", bufs=6))
            xbf = ctx.enter_context(tc.tile_pool(name="xbf", bufs=4))
            xtp = ctx.enter_context(tc.tile_pool(name="xtp", bufs=6))
            hpre = ctx.enter_context(tc.tile_pool(name="hpre", bufs=8))
            zpool = ctx.enter_context(tc.tile_pool(name="zpool", bufs=6))
            ztp = ctx.enter_context(tc.tile_pool(name="ztp", bufs=6))
            gpool = ctx.enter_context(tc.tile_pool(name="gpool", bufs=4))
            ypool = ctx.enter_context(tc.tile_pool(name="ypool", bufs=8))
            opool = ctx.enter_context(tc.tile_pool(name="opool", bufs=6))
            stats = ctx.enter_context(tc.tile_pool(name="stats", bufs=24))
            ps512 = ctx.enter_context(
                tc.tile_pool(name="ps512", bufs=4, space="PSUM"))
            psg = ctx.enter_context(
                tc.tile_pool(name="psg", bufs=2, space="PSUM"))
            pszt = ctx.enter_context(
                tc.tile_pool(name="pszt", bufs=2, space="PSUM"))
            dscr = ctx.enter_context(
                tc.tile_pool(name="dscr", bufs=8, space="DRAM"))

            w_sb = {}
            for name, ap in wts.items():
                t = const.tile(list(ap.shape), ap.dtype, tag=f"w_{name}")
                nc.gpsimd.dma_start(out=t[...], in_=ap)
                w_sb[name] = t
            ident = const.tile([P, P], BF16, tag="ident")
            from concourse.masks import make_identity
            make_identity(nc, ident[...])
            # replicated affine tiles (only when needed)
            rep = {}
            for s in range(2):
                if aff_a[s]:
                    for nm in (f"ga{s}", f"ba{s}"):
                        r = const.tile([P, E], BF16, tag=f"rep_{nm}")
                        nc.sync.dma_start(out=r[...],
                                          in_=wts[nm].to_broadcast((P, E)))
                        rep[nm] = r
                if aff_b[s]:
                    for nm in (f"gb{s}", f"bb{s}"):
                        r = const.tile([P, E], F32, tag=f"rep_{nm}")
                        nc.sync.dma_start(out=r[...],
                                          in_=wts[nm].to_broadcast((P, E)))
                        rep[nm] = r

            eps_sb = const.tile([P, 1], F32, tag="eps")
            nc.vector.memset(eps_sb[...], EPS)
            ones_sb = const.tile([1, R], BF16, tag="ones")
            nc.vector.memset(ones_sb[...], 1.0)

            def layernorm_stats(src):
                """src: [P, E] fp32 sbuf -> (mean_ap, inv_ap)"""
                st6 = stats.tile([P, 6], F32, tag="st6")
                nc.vector.bn_stats(out=st6[...], in_=src)
                mv = stats.tile([P, 2], F32, tag="mv")
                nc.vector.bn_aggr(out=mv[...], in_=st6[...])
                inv = stats.tile([P, 1], F32, tag="inv")
                nc.scalar.activation(out=inv[...], in_=mv[:, 1:2], func=Sqrt,
                                     bias=eps_sb[...], scale=1.0)
                nc.vector.reciprocal(out=inv[...], in_=inv[...])
                return mv[:, 0:1], inv[...]

            def front(mt):
                """loads + attn + LN1 + z for macro tile mt."""
                x_s, xT_s = [], []
                for si, src in enumerate((dna, mol)):
                    xt = xin.tile([P, RC, E], F32, tag="xin")
                    for rc in range(RC):
                        nc.gpsimd.dma_start(out=xt[:, rc, :], in_=src[mt, rc])
                    xb = xbf.tile([P, RC, E], BF16, tag="xbf")
                    nc.scalar.copy(out=xb[...], in_=xt[...])
                    xd = dscr.tile([R, E], BF16, tag="xd")
                    for rc in range(RC):
                        nc.gpsimd.dma_start(out=xd[rc * P:(rc + 1) * P, :],
                                            in_=xb[:, rc, :])
                    xT = xtp.tile([P, KE, R], BF16, tag="xT")
                    for c in range(KE):
                        nc.sync.dma_start_transpose(
                            out=xT[:, c, :], in_=xd[:, c * P:(c + 1) * P])
                    x_s.append(xt)
                    xT_s.append(xT)

                z_s, h1_s = [], []
                for s in range(2):
                    x = x_s[s]
                    kvT = xT_s[1 - s]
                    z = zpool.tile([P, RC, E], BF16, tag="z")
                    for rc in range(RC):
                        ps = ps512.tile([P, E], F32, tag="ps512")
                        for c in range(KE):
                            nc.tensor.matmul(
                                ps[...],
                                kvT[:, c, rc * P:(rc + 1) * P],
                                w_sb[f"w{s}"][:, c, :],
                                start=(c == 0),
                                stop=(c == KE - 1 and not use_c[s]))
                        if use_c[s]:
                            nc.tensor.matmul(ps[...], ones_sb[:, 0:P],
                                             w_sb[f"c{s}"][...],
                                             start=False, stop=True)
                        hp = hpre.tile([P, E], F32, tag="hpre")
                        nc.vector.tensor_add(hp[...], ps[...], x[:, rc, :])
                        mean, inv = layernorm_stats(hp[...])
                        nc.vector.tensor_scalar(
                            out=z[:, rc, :], in0=hp[...],
                            scalar1=mean, scalar2=inv, op0=SUB, op1=MULT)
                    if aff_a[s]:
                        h1 = zpool.tile([P, RC, E], BF16, tag="h1")
                        for rc in range(RC):
                            nc.vector.tensor_mul(h1[:, rc, :], z[:, rc, :],
                                                 rep[f"ga{s}"][...])
                            nc.vector.tensor_add(h1[:, rc, :], h1[:, rc, :],
                                                 rep[f"ba{s}"][...])
                        h1_s.append(h1)
                    else:
                        h1_s.append(z)
                    z_s.append(z)
                return z_s, h1_s

            def back(mt, z_s, h1_s):
                """zT transpose (TensorE) + FFN1 + relu + FFN2 + LN2 + out."""
                gt_s = []
                for s in range(2):
                    z = z_s[s]
                    zT = ztp.tile([P, KE, R], BF16, tag="zT")
                    for c in range(KE):
                        pt = pszt.tile([P, R], BF16, tag="pszt")
                        for rc in range(RC):
                            nc.tensor.transpose(
                                pt[:, rc * P:(rc + 1) * P],
                                z[:, rc, c * P:(c + 1) * P],
                                ident[...])
                        nc.scalar.copy(out=zT[:, c, :], in_=pt[...])
                    gt = gpool.tile([P, KH, R], BF16, tag="gt")
                    for j in range(KH):
                        pg = psg.tile([P, R], F32, tag="psg")
                        for c in range(KE):
                            nc.tensor.matmul(
                                pg[...],
                                w_sb[f"u{s}"][:, c, j * P:(j + 1) * P],
                                zT[:, c, :],
                                start=(c == 0),
                                stop=(c == KE - 1 and not use_d[s]))
                        if use_d[s]:
                            nc.tensor.matmul(
                                pg[...], w_sb[f"d{s}"][:, j * P:(j + 1) * P],
                                ones_sb[:, 0:R], start=False, stop=True)
                        nc.scalar.activation(out=gt[:, j, :], in_=pg[...],
                                             func=Relu)
                    gt_s.append(gt)

                for s in range(2):
                    gt = gt_s[s]
                    h1 = h1_s[s]
                    for rc in range(RC):
                        pf = ps512.tile([P, E], F32, tag="ps512")
                        for j in range(KH):
                            nc.tensor.matmul(
                                pf[...],
                                gt[:, j, rc * P:(rc + 1) * P],
                                w_sb[f"v{s}"][:, j, :],
                                start=(j == 0),
                                stop=(j == KH - 1 and not use_e[s]))
                        if use_e[s]:
                            nc.tensor.matmul(pf[...], ones_sb[:, 0:P],
                                             w_sb[f"e{s}"][...],
                                             start=False, stop=True)
                        y = ypool.tile([P, E], F32, tag="y")
                        nc.vector.tensor_add(y[...], pf[...], h1[:, rc, :])
                        mean, inv = layernorm_stats(y[...])
                        o = opool.tile([P, E], F32, tag="o")
                        nc.vector.tensor_scalar(
                            out=o[...], in0=y[...],
                            scalar1=mean, scalar2=inv, op0=SUB, op1=MULT)
                        if aff_b[s]:
                            nc.vector.tensor_mul(o[...], o[...],
                                                 rep[f"gb{s}"][...])
                            nc.vector.tensor_add(o[...], o[...],
                                                 rep[f"bb{s}"][...])
                        nc.gpsimd.dma_start(
                            out=out[mt, rc, :, s * E:(s + 1) * E], in_=o[...])

            for mt in range(NT):
                back(mt, *front(mt))

    nc.compile()
    return nc


def _prep_host(inputs):
    """Fold weights host-side; returns (weight arrays dict, flags tuple)."""
    g = {k: np.asarray(v, dtype=np.float32) for k, v in inputs.items()}

    def trivial(a, val):
        return bool(np.all(a == val))

    def kchunks(a, nk, dt=BF):
        # [K, N] -> [P, nk, N] (chunk c = rows c*P:(c+1)*P)
        k, n = a.shape
        assert k == nk * P
        return np.ascontiguousarray(
            a.reshape(nk, P, n).transpose(1, 0, 2)).astype(dt)

    arrs = {}
    flags = []
    for s, (aw, ab, ow, ob, lna_g, lna_b, lnb_g, lnb_b, w1, b1, w2, b2) in \
            enumerate((
                (g["a1_in_w"], g["a1_in_b"], g["a1_out_w"], g["a1_out_b"],
                 g["ln1_g"], g["ln1_b"], g["ln3_g"], g["ln3_b"],
                 g["f1_w1"], g["f1_b1"], g["f1_w2"], g["f1_b2"]),
                (g["a2_in_w"], g["a2_in_b"], g["a2_out_w"], g["a2_out_b"],
                 g["ln2_g"], g["ln2_b"], g["ln4_g"], g["ln4_b"],
                 g["f2_w1"], g["f2_b1"], g["f2_w2"], g["f2_b2"]))):
        wv = aw[2 * E:3 * E]
        bv = ab[2 * E:3 * E]
        W = ow @ wv                      # [E, E]; attn = kv @ W.T + c
        c = ow @ bv + ob                 # [E]
        U = w1 * lna_g[None, :]          # LN1 gain folded into FFN1
        d = b1 + w1 @ lna_b              # LN1 bias folded into FFN1 bias
        V = w2                           # [E, HID]
        e = b2                           # [E]
        arrs[f"w{s}"] = kchunks(W.T, E // P)
        arrs[f"u{s}"] = kchunks(U.T, E // P)
        arrs[f"v{s}"] = kchunks(V.T, HID // P)
        uc = not trivial(c, 0.0)
        ud = not trivial(d, 0.0)
        ue = not trivial(e, 0.0)
        fa = not (trivial(lna_g, 1.0) and trivial(lna_b, 0.0))
        fb = not (trivial(lnb_g, 1.0) and trivial(lnb_b, 0.0))
        if uc:
            arrs[f"c{s}"] = c.reshape(1, E).astype(BF)
        if ud:
            arrs[f"d{s}"] = d.reshape(1, HID).astype(BF)
        if ue:
            arrs[f"e{s}"] = e.reshape(1, E).astype(BF)
        if fa:
            arrs[f"ga{s}"] = lna_g.reshape(1, E).astype(BF)
            arrs[f"ba{s}"] = lna_b.reshape(1, E).astype(BF)
        if fb:
            arrs[f"gb{s}"] = lnb_g.reshape(1, E).astype(np.float32)
            arrs[f"bb{s}"] = lnb_b.reshape(1, E).astype(np.float32)
        flags.append((uc, ud, ue, fa, fb))

    (uc0, ud0, ue0, fa0, fb0), (uc1, ud1, ue1, fa1, fb1) = flags
    flag_t = (uc0, uc1, ud0, ud1, ue0, ue1, fa0, fa1, fb0, fb1)
    return g, arrs, flag_t


def kernel(**inputs):
    g, arrs, flag_t = _prep_host(inputs)
    B = g["dna"].shape[0]
    rows_per_core = B // NCORES
    rmacro = 256
    key = (rows_per_core, rmacro, flag_t)
    if key not in _prog_cache:
        _prog_cache[key] = _build_program(rows_per_core, rmacro, flag_t)
    nc = _prog_cache[key]

    NT = rows_per_core // rmacro
    RC = rmacro // P
    in_maps = []
    for i in range(NCORES):
        sl = slice(i * rows_per_core, (i + 1) * rows_per_core)
        im = {
            "dna": np.ascontiguousarray(g["dna"][sl]).reshape(NT, RC, P, E),
            "mol": np.ascontiguousarray(g["mol"][sl]).reshape(NT, RC, P, E),
        }
        im.update(arrs)
        in_maps.append(im)

    res = run_bass_kernel_spmd(nc, in_maps, list(range(NCORES)))
    outs = [r["out"].reshape(rows_per_core, 2 * E) for r in res.results]
    return np.concatenate(outs, axis=0)



# revision 5
# speedup vs baseline: 1.5253x; 1.5253x over previous
"""Trainium2 Bass kernel for nn_CrossAttention (seq_len==1 cross attention,
dual-stream transformer block pair).

Math notes (exact simplifications, valid for any input values):
  - Both attentions have seq_len==1 for q and kv, so softmax over the single
    kv position is exactly 1.0 and attention output == V projection:
        mha(q_in, kv_in) = (kv_in @ wv.T + bv) @ out_w.T + out_b
    The q/k projections are dead code.  Folding the two matmuls:
        attn = kv_in @ (out_w @ wv).T + (out_w @ bv + out_b)
  - LayerNorm affine (g, b) of ln1/ln2 is folded into the following FFN
    weights host-side; residual-path affine and biases are applied on-device
    only when they are non-trivial (they are zeros/ones for the reference
    setup_inputs, so the fast path emits no extra instructions).

Implementation (v2, fp8 DoubleRow + host transposes):
  - Inputs per core (host-prepped): x_bf (row-major bf16, residuals),
    xT8 (feature-major fp8, attention moving/stationary operand).
  - All GEMM weights are scaled by 64 and cast to fp8e4 host-side; matmuls
    run in DoubleRow perf mode (contract 256 K per instruction -> 2x).
  - Residual adds ride on the PE: psum += 64*I @ x (scaled-identity
    stationary, bf16 moving operand), so LayerNorm stats (vector bn_stats)
    read PSUM directly and the normalize is fused into the PSUM->SBUF
    eviction on the scalar engine: z = (ps - m')*inv' with
    inv' = rsqrt(var(ps) + 64^2*eps) handling the descale exactly.
  - z transposed on TensorE (identity matmul) for FFN1's moving operand;
    evicted to fp8.  FFN1 relu eviction emits fp8 g^T which is directly
    FFN2's DoubleRow stationary operand.
"""

import numpy as np
import ml_dtypes
from contextlib import ExitStack

import concourse.bass as bass
import concourse.tile as tile
from concourse import bacc, mybir
from concourse.bass_utils import run_bass_kernel_spmd

E = 512
HID = 1024
NCORES = 8
EPS = 1e-5
P = 128
WSCALE = 64.0  # fp8 weight pre-scale (power of 2; descale folded into LN)

BF16 = mybir.dt.bfloat16
F32 = mybir.dt.float32
FP8 = mybir.dt.float8e4
BF = ml_dtypes.bfloat16
F8 = ml_dtypes.float8_e4m3  # matches TRN FP8_EXP4 (max 240, inf at 1111.000)

_prog_cache = {}


def _build_program(rows_per_core: int, rmacro: int, flags: tuple):
    """Build + compile the per-core Bass program.

    flags = (use_c0, use_c1, use_d0, use_d1, use_e0, use_e1,
             aff_a0, aff_a1, aff_b0, aff_b1)
    """
    (use_c0, use_c1, use_d0, use_d1, use_e0, use_e1,
     aff_a0, aff_a1, aff_b0, aff_b1) = flags
    use_c = (use_c0, use_c1)
    use_d = (use_d0, use_d1)
    use_e = (use_e0, use_e1)
    aff_a = (aff_a0, aff_a1)
    aff_b = (aff_b0, aff_b1)

    R = rmacro
    NT = rows_per_core // R
    RC = R // P
    KE = E // P    # 4 K-chunks over E
    KH = HID // P  # 8 K-chunks over HID
    EPS_EFF = EPS * WSCALE * WSCALE
    DR = mybir.MatmulPerfMode.DoubleRow

    nc = bacc.Bacc("TRN2", target_bir_lowering=False, debug=False,
                   num_devices=NCORES)

    xbf_d, xt8_d = [], []
    for s, nm in enumerate(("dna", "mol")):
        xbf_d.append(nc.dram_tensor(f"x{s}", [NT, RC, P, E], BF16,
                                    kind="ExternalInput").ap())
        xt8_d.append(nc.dram_tensor(f"xt{s}", [NT, P, KE, R], FP8,
                                    kind="ExternalInput").ap())
    out = nc.dram_tensor("out", [NT, RC, P, 2 * E], F32,
                         kind="ExternalOutput").ap()

    wts = {}
    for s in range(2):
        wts[f"w{s}"] = nc.dram_tensor(f"w{s}", [P, KE, E], FP8,
                                      kind="ExternalInput").ap()
        wts[f"u{s}"] = nc.dram_tensor(f"u{s}", [P, KE, HID], FP8,
                                      kind="ExternalInput").ap()
        wts[f"v{s}"] = nc.dram_tensor(f"v{s}", [P, KH, E], FP8,
                                      kind="ExternalInput").ap()
        if use_c[s]:
            wts[f"c{s}"] = nc.dram_tensor(f"c{s}", [1, E], BF16,
                                          kind="ExternalInput").ap()
        if use_d[s]:
            wts[f"d{s}"] = nc.dram_tensor(f"d{s}", [1, HID], BF16,
                                          kind="ExternalInput").ap()
        if use_e[s]:
            wts[f"e{s}"] = nc.dram_tensor(f"e{s}", [1, E], BF16,
                                          kind="ExternalInput").ap()
        if aff_a[s]:
            wts[f"ga{s}"] = nc.dram_tensor(f"ga{s}", [1, E], BF16,
                                           kind="ExternalInput").ap()
            wts[f"ba{s}"] = nc.dram_tensor(f"ba{s}", [1, E], BF16,
                                           kind="ExternalInput").ap()
        if aff_b[s]:
            wts[f"gb{s}"] = nc.dram_tensor(f"gb{s}", [1, E], F32,
                                           kind="ExternalInput").ap()
            wts[f"bb{s}"] = nc.dram_tensor(f"bb{s}", [1, E], F32,
                                           kind="ExternalInput").ap()

    MULT = mybir.AluOpType.mult
    Relu = mybir.ActivationFunctionType.Relu
    Sqrt = mybir.ActivationFunctionType.Sqrt
    Ident = mybir.ActivationFunctionType.Identity

    with tile.TileContext(nc) as tc:
        with ExitStack() as ctx:
            const = ctx.enter_context(tc.tile_pool(name="const", bufs=1))
            xbf = ctx.enter_context(tc.tile_pool(name="xbf", bufs=4))
            xt8 = ctx.enter_context(tc.tile_pool(name="xt8", bufs=4))
            zbf = ctx.enter_context(tc.tile_pool(name="zbf", bufs=4))
            zt8 = ctx.enter_context(tc.tile_pool(name="zt8", bufs=4))
            gt8 = ctx.enter_context(tc.tile_pool(name="gt8", bufs=3))
            h1p = ctx.enter_context(tc.tile_pool(name="h1p", bufs=4))
            opool = ctx.enter_context(tc.tile_pool(name="opool", bufs=4))
            stats = ctx.enter_context(tc.tile_pool(name="stats", bufs=48))
            ps_att = ctx.enter_context(
                tc.tile_pool(name="ps_att", bufs=2, space="PSUM"))
            ps_g = ctx.enter_context(
                tc.tile_pool(name="ps_g", bufs=2, space="PSUM"))
            ps_f = ctx.enter_context(
                tc.tile_pool(name="ps_f", bufs=2, space="PSUM"))
            ps_t = ctx.enter_context(
                tc.tile_pool(name="ps_t", bufs=2, space="PSUM"))

            w_sb = {}
            for name, ap in wts.items():
                t = const.tile(list(ap.shape), ap.dtype, tag=f"w_{name}")
                nc.sync.dma_start(out=t[...], in_=ap)
                w_sb[name] = t
            ident = const.tile([P, P], BF16, tag="ident")
            from concourse.masks import make_identity
            make_identity(nc, ident[...])
            # scaled identity for residual-accumulate matmuls
            ident_sc = const.tile([P, P], BF16, tag="ident_sc")
            nc.vector.tensor_scalar_mul(ident_sc[...], ident[...], WSCALE)
            # replicated affine tiles (only when needed)
            rep = {}
            for s in range(2):
                if aff_a[s]:
                    for nm in (f"ga{s}", f"ba{s}"):
                        r = const.tile([P, E], BF16, tag=f"rep_{nm}")
                        nc.sync.dma_start(out=r[...],
                                          in_=wts[nm].to_broadcast((P, E)))
                        rep[nm] = r
                if aff_b[s]:
                    for nm in (f"gb{s}", f"bb{s}"):
                        r = const.tile([P, E], F32, tag=f"rep_{nm}")
                        nc.sync.dma_start(out=r[...],
                                          in_=wts[nm].to_broadcast((P, E)))
                        rep[nm] = r

            eps_sb = const.tile([P, 1], F32, tag="eps")
            nc.vector.memset(eps_sb[...], EPS_EFF)
            ones_sb = const.tile([1, R], BF16, tag="ones")
            nc.vector.memset(ones_sb[...], 1.0)

            def layernorm_scales(ps):
                """ps: [P, E] fp32 psum holding WSCALE*(x) -> (inv, negminv)
                such that (ps - m')*inv' == LN(x) exactly."""
                st6 = stats.tile([P, 6], F32, tag="st6")
                nc.vector.bn_stats(out=st6[...], in_=ps)
                mv = stats.tile([P, 2], F32, tag="mv")
                nc.vector.bn_aggr(out=mv[...], in_=st6[...])
                inv = stats.tile([P, 1], F32, tag="inv")
                nc.scalar.activation(out=inv[...], in_=mv[:, 1:2], func=Sqrt,
                                     bias=eps_sb[...], scale=1.0)
                nc.vector.reciprocal(out=inv[...], in_=inv[...])
                nmi = stats.tile([P, 1], F32, tag="nmi")
                nc.vector.scalar_tensor_tensor(
                    out=nmi[...], in0=mv[:, 0:1], scalar=-1.0, in1=inv[...],
                    op0=MULT, op1=MULT)
                return inv, nmi

            def front(mt):
                """loads + attn (+residual via scaled identity) + LN1 -> z."""
                x_s, xT_s = [], []
                for s in range(2):
                    xt = xbf.tile([P, RC, E], BF16, tag=f"xin{s}")
                    for rc in range(RC):
                        nc.gpsimd.dma_start(out=xt[:, rc, :],
                                            in_=xbf_d[s][mt, rc])
                    x8 = xt8.tile([P, KE, R], FP8, tag=f"xT{s}")
                    nc.gpsimd.dma_start(out=x8[...], in_=xt8_d[s][mt])
                    x_s.append(xt)
                    xT_s.append(x8)

                z_s, h1_s = [], []
                for s in range(2):
                    kvT = xT_s[1 - s]
                    z = zbf.tile([P, RC, E], BF16, tag=f"z{s}")
                    for rc in range(RC):
                        ps = ps_att.tile([P, E], F32, tag="ps_att")
                        for c2 in range(KE // 2):
                            nc.tensor.matmul(
                                ps[...],
                                kvT[:, 2 * c2:2 * c2 + 2,
                                    rc * P:(rc + 1) * P],
                                w_sb[f"w{s}"][:, 2 * c2:2 * c2 + 2, :],
                                start=(c2 == 0), stop=False,
                                perf_mode=DR)
                        # residual: ps += WSCALE * x_q
                        nc.tensor.matmul(
                            ps[...], ident_sc[...], x_s[s][:, rc, :],
                            start=False, stop=(not use_c[s]),
                            skip_group_check=True)
                        if use_c[s]:
                            nc.tensor.matmul(ps[...], ones_sb[:, 0:P],
                                             w_sb[f"c{s}"][...],
                                             start=False, stop=True,
                                             skip_group_check=True)
                        inv, nmi = layernorm_scales(ps[...])
                        nc.scalar.activation(out=z[:, rc, :], in_=ps[...],
                                             func=Ident, scale=inv[...],
                                             bias=nmi[...])
                    if aff_a[s]:
                        h1 = h1p.tile([P, RC, E], BF16, tag=f"h1{s}")
                        for rc in range(RC):
                            nc.vector.tensor_mul(h1[:, rc, :], z[:, rc, :],
                                                 rep[f"ga{s}"][...])
                            nc.vector.tensor_add(h1[:, rc, :], h1[:, rc, :],
                                                 rep[f"ba{s}"][...])
                        h1_s.append(h1)
                    else:
                        h1_s.append(z)
                    z_s.append(z)
                return z_s, h1_s

            def back(mt, z_s, h1_s):
                """zT transpose (TensorE) + FFN1 + relu + FFN2 + LN2 + out."""
                gt_s = []
                for s in range(2):
                    z = z_s[s]
                    zT = zt8.tile([P, KE, R], FP8, tag=f"zT{s}")
                    for c in range(KE):
                        pt = ps_t.tile([P, R], BF16, tag="ps_t")
                        for rc in range(RC):
                            nc.tensor.transpose(
                                pt[:, rc * P:(rc + 1) * P],
                                z[:, rc, c * P:(c + 1) * P],
                                ident[...])
                        nc.scalar.copy(out=zT[:, c, :], in_=pt[...])
                    gt = gt8.tile([P, KH, R], FP8, tag=f"gt{s}")
                    for j in range(KH):
                        pg = ps_g.tile([P, R], F32, tag="ps_g")
                        for c2 in range(KE // 2):
                            nc.tensor.matmul(
                                pg[...],
                                w_sb[f"u{s}"][:, 2 * c2:2 * c2 + 2,
                                              j * P:(j + 1) * P],
                                zT[:, 2 * c2:2 * c2 + 2, :],
                                start=(c2 == 0),
                                stop=(c2 == KE // 2 - 1 and not use_d[s]),
                                perf_mode=DR)
                        if use_d[s]:
                            nc.tensor.matmul(
                                pg[...], w_sb[f"d{s}"][:, j * P:(j + 1) * P],
                                ones_sb[:, 0:R], start=False, stop=True,
                                skip_group_check=True)
                        nc.scalar.activation(out=gt[:, j, :], in_=pg[...],
                                             func=Relu, scale=1.0 / WSCALE)
                    gt_s.append(gt)

                for s in range(2):
                    gt = gt_s[s]
                    h1 = h1_s[s]
                    for rc in range(RC):
                        pf = ps_f.tile([P, E], F32, tag="ps_f")
                        for j2 in range(KH // 2):
                            nc.tensor.matmul(
                                pf[...],
                                gt[:, 2 * j2:2 * j2 + 2,
                                   rc * P:(rc + 1) * P],
                                w_sb[f"v{s}"][:, 2 * j2:2 * j2 + 2, :],
                                start=(j2 == 0), stop=False,
                                perf_mode=DR)
                        # residual: pf += WSCALE * h1
                        nc.tensor.matmul(
                            pf[...], ident_sc[...], h1[:, rc, :],
                            start=False, stop=(not use_e[s]),
                            skip_group_check=True)
                        if use_e[s]:
                            nc.tensor.matmul(pf[...], ones_sb[:, 0:P],
                                             w_sb[f"e{s}"][...],
                                             start=False, stop=True,
                                             skip_group_check=True)
                        inv, nmi = layernorm_scales(pf[...])
                        o = opool.tile([P, E], F32, tag="o")
                        nc.scalar.activation(out=o[...], in_=pf[...],
                                             func=Ident, scale=inv[...],
                                             bias=nmi[...])
                        if aff_b[s]:
                            nc.vector.tensor_mul(o[...], o[...],
                                                 rep[f"gb{s}"][...])
                            nc.vector.tensor_add(o[...], o[...],
                                                 rep[f"bb{s}"][...])
                        nc.sync.dma_start(
                            out=out[mt, rc, :, s * E:(s + 1) * E], in_=o[...])

            for mt in range(NT):
                back(mt, *front(mt))

    nc.compile()
    return nc


def _prep_host(inputs):
    """Fold weights host-side; returns (full arrays, weight map, flags)."""
    g = {k: np.asarray(v, dtype=np.float32) for k, v in inputs.items()}

    def trivial(a, val):
        return bool(np.all(a == val))

    def kchunks(a, nk, dt):
        # [K, N] -> [P, nk, N] (chunk c = rows c*P:(c+1)*P)
        k, n = a.shape
        assert k == nk * P
        return np.ascontiguousarray(
            a.reshape(nk, P, n).transpose(1, 0, 2)).astype(dt)

    arrs = {}
    flags = []
    for s, (aw, ab, ow, ob, lna_g, lna_b, lnb_g, lnb_b, w1, b1, w2, b2) in \
            enumerate((
                (g["a1_in_w"], g["a1_in_b"], g["a1_out_w"], g["a1_out_b"],
                 g["ln1_g"], g["ln1_b"], g["ln3_g"], g["ln3_b"],
                 g["f1_w1"], g["f1_b1"], g["f1_w2"], g["f1_b2"]),
                (g["a2_in_w"], g["a2_in_b"], g["a2_out_w"], g["a2_out_b"],
                 g["ln2_g"], g["ln2_b"], g["ln4_g"], g["ln4_b"],
                 g["f2_w1"], g["f2_b1"], g["f2_w2"], g["f2_b2"]))):
        wv = aw[2 * E:3 * E]
        bv = ab[2 * E:3 * E]
        W = ow @ wv                      # [E, E]; attn = kv @ W.T + c
        c = ow @ bv + ob                 # [E]
        U = w1 * lna_g[None, :]          # LN1 gain folded into FFN1
        d = b1 + w1 @ lna_b              # LN1 bias folded into FFN1 bias
        V = w2                           # [E, HID]
        e = b2                           # [E]
        arrs[f"w{s}"] = kchunks(W.T * WSCALE, E // P, F8)
        arrs[f"u{s}"] = kchunks(U.T * WSCALE, E // P, F8)
        arrs[f"v{s}"] = kchunks(V.T * WSCALE, HID // P, F8)
        uc = not trivial(c, 0.0)
        ud = not trivial(d, 0.0)
        ue = not trivial(e, 0.0)
        fa = not (trivial(lna_g, 1.0) and trivial(lna_b, 0.0))
        fb = not (trivial(lnb_g, 1.0) and trivial(lnb_b, 0.0))
        if uc:
            arrs[f"c{s}"] = (c * WSCALE).reshape(1, E).astype(BF)
        if ud:
            arrs[f"d{s}"] = (d * WSCALE).reshape(1, HID).astype(BF)
        if ue:
            arrs[f"e{s}"] = (e * WSCALE).reshape(1, E).astype(BF)
        if fa:
            arrs[f"ga{s}"] = lna_g.reshape(1, E).astype(BF)
            arrs[f"ba{s}"] = lna_b.reshape(1, E).astype(BF)
        if fb:
            arrs[f"gb{s}"] = lnb_g.reshape(1, E).astype(np.float32)
            arrs[f"bb{s}"] = lnb_b.reshape(1, E).astype(np.float32)
        flags.append((uc, ud, ue, fa, fb))

    (uc0, ud0, ue0, fa0, fb0), (uc1, ud1, ue1, fa1, fb1) = flags
    flag_t = (uc0, uc1, ud0, ud1, ue0, ue1, fa0, fa1, fb0, fb1)
    return g, arrs, flag_t


def _make_in_maps(g, arrs, rows_per_core, rmacro):
    NT = rows_per_core // rmacro
    RC = rmacro // P
    KE = E // P
    in_maps = [dict(arrs) for _ in range(NCORES)]
    for s, key in enumerate(("dna", "mol")):
        x = g[key]
        xb = x.astype(BF).reshape(NCORES, NT, RC, P, E)
        # xT8[mt, p, c, r] = x[mt*R + r, c*P + p]
        x8 = np.ascontiguousarray(x.T.astype(F8).reshape(
            KE, P, NCORES, NT, rmacro).transpose(2, 3, 1, 0, 4))
        for i in range(NCORES):
            in_maps[i][f"x{s}"] = np.ascontiguousarray(xb[i])
            in_maps[i][f"xt{s}"] = x8[i]
    return in_maps


def _get_program(inputs):
    g, arrs, flag_t = _prep_host(inputs)
    B = g["dna"].shape[0]
    rows_per_core = B // NCORES
    rmacro = min(512, rows_per_core)
    key = (rows_per_core, rmacro, flag_t)
    if key not in _prog_cache:
        _prog_cache[key] = _build_program(rows_per_core, rmacro, flag_t)
    nc = _prog_cache[key]
    in_maps = _make_in_maps(g, arrs, rows_per_core, rmacro)
    return nc, in_maps, rows_per_core


def kernel(**inputs):
    nc, in_maps, rows_per_core = _get_program(inputs)
    res = run_bass_kernel_spmd(nc, in_maps, list(range(NCORES)))
    outs = [r["out"].reshape(rows_per_core, 2 * E) for r in res.results]
    return np.concatenate(outs, axis=0)


# revision 12
# speedup vs baseline: 1.7802x; 1.1671x over previous
"""Trainium2 Bass kernel for nn_CrossAttention (seq_len==1 cross attention,
dual-stream transformer block pair).

Math notes (exact simplifications, valid for any input values):
  - Both attentions have seq_len==1 for q and kv, so softmax over the single
    kv position is exactly 1.0 and attention output == V projection:
        mha(q_in, kv_in) = (kv_in @ wv.T + bv) @ out_w.T + out_b
    The q/k projections are dead code.  Folding the two matmuls:
        attn = kv_in @ (out_w @ wv).T + (out_w @ bv + out_b)
  - LayerNorm affine (g, b) of ln1/ln2 is folded into the following FFN
    weights host-side; residual-path affine and biases are applied on-device
    only when they are non-trivial (they are zeros/ones for the reference
    setup_inputs, so the fast path emits no extra instructions).

Implementation (v2, fp8 DoubleRow + host transposes):
  - Inputs per core (host-prepped): x_bf (row-major bf16, residuals),
    xT8 (feature-major fp8, attention moving/stationary operand).
  - All GEMM weights are scaled by 64 and cast to fp8e4 host-side; matmuls
    run in DoubleRow perf mode (contract 256 K per instruction -> 2x).
  - Residual adds ride on the PE: psum += 64*I @ x (scaled-identity
    stationary, bf16 moving operand), so LayerNorm stats (vector bn_stats)
    read PSUM directly and the normalize is fused into the PSUM->SBUF
    eviction on the scalar engine: z = (ps - m')*inv' with
    inv' = rsqrt(var(ps) + 64^2*eps) handling the descale exactly.
  - z transposed on TensorE (identity matmul) for FFN1's moving operand;
    evicted to fp8.  FFN1 relu eviction emits fp8 g^T which is directly
    FFN2's DoubleRow stationary operand.
"""

import numpy as np
import ml_dtypes
from contextlib import ExitStack

import concourse.bass as bass
import concourse.tile as tile
from concourse import bacc, mybir
from concourse.bass_utils import run_bass_kernel_spmd

E = 512
HID = 1024
NCORES = 8
EPS = 1e-5
P = 128
WSCALE = 64.0  # fp8 weight pre-scale (power of 2; descale folded into LN)

BF16 = mybir.dt.bfloat16
F32 = mybir.dt.float32
FP8 = mybir.dt.float8e4
BF = ml_dtypes.bfloat16
F8 = ml_dtypes.float8_e4m3  # matches TRN FP8_EXP4 (max 240, inf at 1111.000)

_prog_cache = {}


def _build_program(rows_per_core: int, rmacro: int, flags: tuple):
    """Build + compile the per-core Bass program.

    flags = (use_c0, use_c1, use_d0, use_d1, use_e0, use_e1,
             aff_a0, aff_a1, aff_b0, aff_b1)
    """
    (use_c0, use_c1, use_d0, use_d1, use_e0, use_e1,
     aff_a0, aff_a1, aff_b0, aff_b1) = flags
    use_c = (use_c0, use_c1)
    use_d = (use_d0, use_d1)
    use_e = (use_e0, use_e1)
    aff_a = (aff_a0, aff_a1)
    aff_b = (aff_b0, aff_b1)

    R = rmacro
    NT = rows_per_core // R
    RC = R // P
    KE = E // P    # 4 K-chunks over E
    KH = HID // P  # 8 K-chunks over HID
    EPS_EFF = EPS * WSCALE * WSCALE
    DR = mybir.MatmulPerfMode.DoubleRow

    nc = bacc.Bacc("TRN2", target_bir_lowering=False, debug=False,
                   num_devices=NCORES)

    xbf_d, xt8_d = [], []
    for s, nm in enumerate(("dna", "mol")):
        xbf_d.append(nc.dram_tensor(f"x{s}", [NT, RC, P, E], BF16,
                                    kind="ExternalInput").ap())
        xt8_d.append(nc.dram_tensor(f"xt{s}", [NT, P, KE, R], FP8,
                                    kind="ExternalInput").ap())
    out = nc.dram_tensor("out", [NT, RC, P, 2 * E], F32,
                         kind="ExternalOutput").ap()

    wts = {}
    for s in range(2):
        wts[f"w{s}"] = nc.dram_tensor(f"w{s}", [P, KE, E], FP8,
                                      kind="ExternalInput").ap()
        wts[f"u{s}"] = nc.dram_tensor(f"u{s}", [P, KE, HID], FP8,
                                      kind="ExternalInput").ap()
        wts[f"v{s}"] = nc.dram_tensor(f"v{s}", [P, KH, E], FP8,
                                      kind="ExternalInput").ap()
        if use_c[s]:
            wts[f"c{s}"] = nc.dram_tensor(f"c{s}", [1, E], BF16,
                                          kind="ExternalInput").ap()
        if use_d[s]:
            wts[f"d{s}"] = nc.dram_tensor(f"d{s}", [1, HID], BF16,
                                          kind="ExternalInput").ap()
        if use_e[s]:
            wts[f"e{s}"] = nc.dram_tensor(f"e{s}", [1, E], BF16,
                                          kind="ExternalInput").ap()
        if aff_a[s]:
            wts[f"ga{s}"] = nc.dram_tensor(f"ga{s}", [1, E], BF16,
                                           kind="ExternalInput").ap()
            wts[f"ba{s}"] = nc.dram_tensor(f"ba{s}", [1, E], BF16,
                                           kind="ExternalInput").ap()
        if aff_b[s]:
            wts[f"gb{s}"] = nc.dram_tensor(f"gb{s}", [1, E], F32,
                                           kind="ExternalInput").ap()
            wts[f"bb{s}"] = nc.dram_tensor(f"bb{s}", [1, E], F32,
                                           kind="ExternalInput").ap()

    MULT = mybir.AluOpType.mult
    Relu = mybir.ActivationFunctionType.Relu
    Sqrt = mybir.ActivationFunctionType.Sqrt
    Ident = mybir.ActivationFunctionType.Identity

    with tile.TileContext(nc) as tc:
        with ExitStack() as ctx:
            const = ctx.enter_context(tc.tile_pool(name="const", bufs=1))
            xbf = ctx.enter_context(tc.tile_pool(name="xbf", bufs=4))
            xt8 = ctx.enter_context(tc.tile_pool(name="xt8", bufs=4))
            zbf = ctx.enter_context(tc.tile_pool(name="zbf", bufs=6))
            zt8 = ctx.enter_context(tc.tile_pool(name="zt8", bufs=4))
            gt8 = ctx.enter_context(tc.tile_pool(name="gt8", bufs=3))
            h1p = ctx.enter_context(tc.tile_pool(name="h1p", bufs=4))
            opool = ctx.enter_context(tc.tile_pool(name="opool", bufs=4))
            stats = ctx.enter_context(tc.tile_pool(name="stats", bufs=48))
            ps_att = ctx.enter_context(
                tc.tile_pool(name="ps_att", bufs=2, space="PSUM"))
            ps_g = ctx.enter_context(
                tc.tile_pool(name="ps_g", bufs=2, space="PSUM"))
            ps_f = ctx.enter_context(
                tc.tile_pool(name="ps_f", bufs=2, space="PSUM"))
            ps_t = ctx.enter_context(
                tc.tile_pool(name="ps_t", bufs=2, space="PSUM"))

            w_sb = {}
            for name, ap in wts.items():
                t = const.tile(list(ap.shape), ap.dtype, tag=f"w_{name}")
                q = nc.scalar if name.endswith("1") else nc.sync
                q.dma_start(out=t[...], in_=ap)
                w_sb[name] = t
            ident = const.tile([P, P], BF16, tag="ident")
            from concourse.masks import make_identity
            make_identity(nc, ident[...])
            # scaled identity for residual-accumulate matmuls
            ident_sc = const.tile([P, P], BF16, tag="ident_sc")
            nc.vector.tensor_scalar_mul(ident_sc[...], ident[...], WSCALE)
            # replicated affine tiles (only when needed)
            rep = {}
            for s in range(2):
                if aff_a[s]:
                    for nm in (f"ga{s}", f"ba{s}"):
                        r = const.tile([P, E], BF16, tag=f"rep_{nm}")
                        nc.sync.dma_start(out=r[...],
                                          in_=wts[nm].to_broadcast((P, E)))
                        rep[nm] = r
                if aff_b[s]:
                    for nm in (f"gb{s}", f"bb{s}"):
                        r = const.tile([P, E], F32, tag=f"rep_{nm}")
                        nc.sync.dma_start(out=r[...],
                                          in_=wts[nm].to_broadcast((P, E)))
                        rep[nm] = r

            eps_sb = const.tile([P, 1], F32, tag="eps")
            nc.vector.memset(eps_sb[...], EPS_EFF)
            ones_sb = const.tile([1, R], BF16, tag="ones")
            nc.vector.memset(ones_sb[...], 1.0)

            def layernorm_scales(ps):
                """ps: [P, E] fp32 psum holding WSCALE*(x) -> (inv, negminv)
                such that (ps - m')*inv' == LN(x) exactly."""
                st6 = stats.tile([P, 6], F32, tag="st6")
                nc.vector.bn_stats(out=st6[...], in_=ps)
                mv = stats.tile([P, 2], F32, tag="mv")
                nc.vector.bn_aggr(out=mv[...], in_=st6[...])
                inv = stats.tile([P, 1], F32, tag="inv")
                nc.scalar.activation(out=inv[...], in_=mv[:, 1:2], func=Sqrt,
                                     bias=eps_sb[...], scale=1.0)
                nc.vector.reciprocal(out=inv[...], in_=inv[...])
                nmi = stats.tile([P, 1], F32, tag="nmi")
                nc.vector.scalar_tensor_tensor(
                    out=nmi[...], in0=mv[:, 0:1], scalar=-1.0, in1=inv[...],
                    op0=MULT, op1=MULT)
                return inv, nmi

            def front(mt):
                """loads + attn (+residual via scaled identity) + LN1 -> z."""
                x_s, xT_s = [], []
                for s in range(2):
                    xt = xbf.tile([P, RC, E], BF16, tag=f"xin{s}")
                    for rc in range(RC):
                        nc.gpsimd.dma_start(out=xt[:, rc, :],
                                            in_=xbf_d[s][mt, rc])
                    x8 = xt8.tile([P, KE, R], FP8, tag=f"xT{s}")
                    nc.gpsimd.dma_start(out=x8[...], in_=xt8_d[s][mt])
                    x_s.append(xt)
                    xT_s.append(x8)

                z_s, h1_s = [], []
                for s in range(2):
                    kvT = xT_s[1 - s]
                    z = zbf.tile([P, RC, E], BF16, tag=f"z{s}")
                    for rc in range(RC):
                        ps = ps_att.tile([P, E], F32, tag="ps_att")
                        for c2 in range(KE // 2):
                            nc.tensor.matmul(
                                ps[...],
                                kvT[:, 2 * c2:2 * c2 + 2,
                                    rc * P:(rc + 1) * P],
                                w_sb[f"w{s}"][:, 2 * c2:2 * c2 + 2, :],
                                start=(c2 == 0), stop=False,
                                perf_mode=DR)
                        # residual: ps += WSCALE * x_q
                        nc.tensor.matmul(
                            ps[...], ident_sc[...], x_s[s][:, rc, :],
                            start=False, stop=(not use_c[s]),
                            skip_group_check=True)
                        if use_c[s]:
                            nc.tensor.matmul(ps[...], ones_sb[:, 0:P],
                                             w_sb[f"c{s}"][...],
                                             start=False, stop=True,
                                             skip_group_check=True)
                        inv, nmi = layernorm_scales(ps[...])
                        nc.scalar.activation(out=z[:, rc, :], in_=ps[...],
                                             func=Ident, scale=inv[...],
                                             bias=nmi[...])
                    if aff_a[s]:
                        h1 = h1p.tile([P, RC, E], BF16, tag=f"h1{s}")
                        for rc in range(RC):
                            nc.vector.tensor_mul(h1[:, rc, :], z[:, rc, :],
                                                 rep[f"ga{s}"][...])
                            nc.vector.tensor_add(h1[:, rc, :], h1[:, rc, :],
                                                 rep[f"ba{s}"][...])
                        h1_s.append(h1)
                    else:
                        h1_s.append(z)
                    z_s.append(z)
                return z_s, h1_s

            def back(mt, z_s, h1_s):
                """zT transpose (TensorE) + FFN1 + relu + FFN2 + LN2 + out."""
                gt_s = []
                for s in range(2):
                    z = z_s[s]
                    zT = zt8.tile([P, KE, R], FP8, tag=f"zT{s}")
                    for c in range(KE):
                        pt = ps_t.tile([P, R], BF16, tag="ps_t")
                        for rc in range(RC):
                            nc.tensor.transpose(
                                pt[:, rc * P:(rc + 1) * P],
                                z[:, rc, c * P:(c + 1) * P],
                                ident[...])
                        nc.vector.tensor_copy(out=zT[:, c, :], in_=pt[...])
                    gt = gt8.tile([P, KH, R], FP8, tag=f"gt{s}")
                    for j in range(KH):
                        pg = ps_g.tile([P, R], F32, tag="ps_g")
                        for c2 in range(KE // 2):
                            nc.tensor.matmul(
                                pg[...],
                                w_sb[f"u{s}"][:, 2 * c2:2 * c2 + 2,
                                              j * P:(j + 1) * P],
                                zT[:, 2 * c2:2 * c2 + 2, :],
                                start=(c2 == 0),
                                stop=(c2 == KE // 2 - 1 and not use_d[s]),
                                perf_mode=DR)
                        if use_d[s]:
                            nc.tensor.matmul(
                                pg[...], w_sb[f"d{s}"][:, j * P:(j + 1) * P],
                                ones_sb[:, 0:R], start=False, stop=True,
                                skip_group_check=True)
                        nc.scalar.activation(out=gt[:, j, :], in_=pg[...],
                                             func=Relu, scale=1.0 / WSCALE)
                    gt_s.append(gt)

                for s in range(2):
                    gt = gt_s[s]
                    h1 = h1_s[s]
                    for rc in range(RC):
                        pf = ps_f.tile([P, E], F32, tag="ps_f")
                        for j2 in range(KH // 2):
                            nc.tensor.matmul(
                                pf[...],
                                gt[:, 2 * j2:2 * j2 + 2,
                                   rc * P:(rc + 1) * P],
                                w_sb[f"v{s}"][:, 2 * j2:2 * j2 + 2, :],
                                start=(j2 == 0), stop=False,
                                perf_mode=DR)
                        # residual: pf += WSCALE * h1
                        nc.tensor.matmul(
                            pf[...], ident_sc[...], h1[:, rc, :],
                            start=False, stop=(not use_e[s]),
                            skip_group_check=True)
                        if use_e[s]:
                            nc.tensor.matmul(pf[...], ones_sb[:, 0:P],
                                             w_sb[f"e{s}"][...],
                                             start=False, stop=True,
                                             skip_group_check=True)
                        inv, nmi = layernorm_scales(pf[...])
                        o = opool.tile([P, E], F32, tag="o")
                        nc.scalar.activation(out=o[...], in_=pf[...],
                                             func=Ident, scale=inv[...],
                                             bias=nmi[...])
                        if aff_b[s]:
                            nc.vector.tensor_mul(o[...], o[...],
                                                 rep[f"gb{s}"][...])
                            nc.vector.tensor_add(o[...], o[...],
                                                 rep[f"bb{s}"][...])
                        nc.sync.dma_start(
                            out=out[mt, rc, :, s * E:(s + 1) * E], in_=o[...])

            # software pipeline: emit front(mt+1) before back(mt) so the
            # in-order PE queue alternates light front bursts with heavy
            # back phases and LN1(mt+1) overlaps FFN(mt) on vector/scalar.
            pend = {0: front(0)}
            for mt in range(NT):
                if mt + 1 < NT:
                    pend[mt + 1] = front(mt + 1)
                back(mt, *pend.pop(mt))

    nc.compile()
    return nc


def _prep_host(inputs):
    """Fold weights host-side; returns (full arrays, weight map, flags)."""
    g = {k: np.asarray(v, dtype=np.float32) for k, v in inputs.items()}

    def trivial(a, val):
        return bool(np.all(a == val))

    def kchunks(a, nk, dt):
        # [K, N] -> [P, nk, N] (chunk c = rows c*P:(c+1)*P)
        k, n = a.shape
        assert k == nk * P
        return np.ascontiguousarray(
            a.reshape(nk, P, n).transpose(1, 0, 2)).astype(dt)

    arrs = {}
    flags = []
    for s, (aw, ab, ow, ob, lna_g, lna_b, lnb_g, lnb_b, w1, b1, w2, b2) in \
            enumerate((
                (g["a1_in_w"], g["a1_in_b"], g["a1_out_w"], g["a1_out_b"],
                 g["ln1_g"], g["ln1_b"], g["ln3_g"], g["ln3_b"],
                 g["f1_w1"], g["f1_b1"], g["f1_w2"], g["f1_b2"]),
                (g["a2_in_w"], g["a2_in_b"], g["a2_out_w"], g["a2_out_b"],
                 g["ln2_g"], g["ln2_b"], g["ln4_g"], g["ln4_b"],
                 g["f2_w1"], g["f2_b1"], g["f2_w2"], g["f2_b2"]))):
        wv = aw[2 * E:3 * E]
        bv = ab[2 * E:3 * E]
        W = ow @ wv                      # [E, E]; attn = kv @ W.T + c
        c = ow @ bv + ob                 # [E]
        U = w1 * lna_g[None, :]          # LN1 gain folded into FFN1
        d = b1 + w1 @ lna_b              # LN1 bias folded into FFN1 bias
        V = w2                           # [E, HID]
        e = b2                           # [E]
        arrs[f"w{s}"] = kchunks(W.T * WSCALE, E // P, F8)
        arrs[f"u{s}"] = kchunks(U.T * WSCALE, E // P, F8)
        arrs[f"v{s}"] = kchunks(V.T * WSCALE, HID // P, F8)
        uc = not trivial(c, 0.0)
        ud = not trivial(d, 0.0)
        ue = not trivial(e, 0.0)
        fa = not (trivial(lna_g, 1.0) and trivial(lna_b, 0.0))
        fb = not (trivial(lnb_g, 1.0) and trivial(lnb_b, 0.0))
        if uc:
            arrs[f"c{s}"] = (c * WSCALE).reshape(1, E).astype(BF)
        if ud:
            arrs[f"d{s}"] = (d * WSCALE).reshape(1, HID).astype(BF)
        if ue:
            arrs[f"e{s}"] = (e * WSCALE).reshape(1, E).astype(BF)
        if fa:
            arrs[f"ga{s}"] = lna_g.reshape(1, E).astype(BF)
            arrs[f"ba{s}"] = lna_b.reshape(1, E).astype(BF)
        if fb:
            arrs[f"gb{s}"] = lnb_g.reshape(1, E).astype(np.float32)
            arrs[f"bb{s}"] = lnb_b.reshape(1, E).astype(np.float32)
        flags.append((uc, ud, ue, fa, fb))

    (uc0, ud0, ue0, fa0, fb0), (uc1, ud1, ue1, fa1, fb1) = flags
    flag_t = (uc0, uc1, ud0, ud1, ue0, ue1, fa0, fa1, fb0, fb1)
    return g, arrs, flag_t


def _make_in_maps(g, arrs, rows_per_core, rmacro):
    NT = rows_per_core // rmacro
    RC = rmacro // P
    KE = E // P
    in_maps = [dict(arrs) for _ in range(NCORES)]
    for s, key in enumerate(("dna", "mol")):
        x = g[key]
        xb = x.astype(BF).reshape(NCORES, NT, RC, P, E)
        # xT8[mt, p, c, r] = x[mt*R + r, c*P + p]
        x8 = np.ascontiguousarray(x.T.astype(F8).reshape(
            KE, P, NCORES, NT, rmacro).transpose(2, 3, 1, 0, 4))
        for i in range(NCORES):
            in_maps[i][f"x{s}"] = np.ascontiguousarray(xb[i])
            in_maps[i][f"xt{s}"] = x8[i]
    return in_maps


def _get_program(inputs):
    g, arrs, flag_t = _prep_host(inputs)
    B = g["dna"].shape[0]
    rows_per_core = B // NCORES
    rmacro = min(512, rows_per_core)
    key = (rows_per_core, rmacro, flag_t)
    if key not in _prog_cache:
        _prog_cache[key] = _build_program(rows_per_core, rmacro, flag_t)
    nc = _prog_cache[key]
    in_maps = _make_in_maps(g, arrs, rows_per_core, rmacro)
    return nc, in_maps, rows_per_core


def kernel(**inputs):
    nc, in_maps, rows_per_core = _get_program(inputs)
    res = run_bass_kernel_spmd(nc, in_maps, list(range(NCORES)))
    outs = [r["out"].reshape(rows_per_core, 2 * E) for r in res.results]
    return np.concatenate(outs, axis=0)


# revision 15
# speedup vs baseline: 1.8438x; 1.0357x over previous
"""Trainium2 Bass kernel for nn_CrossAttention (seq_len==1 cross attention,
dual-stream transformer block pair).

Math notes (exact simplifications, valid for any input values):
  - Both attentions have seq_len==1 for q and kv, so softmax over the single
    kv position is exactly 1.0 and attention output == V projection:
        mha(q_in, kv_in) = (kv_in @ wv.T + bv) @ out_w.T + out_b
    The q/k projections are dead code.  Folding the two matmuls:
        attn = kv_in @ (out_w @ wv).T + (out_w @ bv + out_b)
  - LayerNorm affine (g, b) of ln1/ln2 is folded into the following FFN
    weights host-side; residual-path affine and biases are applied on-device
    only when they are non-trivial (they are zeros/ones for the reference
    setup_inputs, so the fast path emits no extra instructions).

Implementation (v2, fp8 DoubleRow + host transposes):
  - Inputs per core (host-prepped): x_bf (row-major bf16, residuals),
    xT8 (feature-major fp8, attention moving/stationary operand).
  - All GEMM weights are scaled by 64 and cast to fp8e4 host-side; matmuls
    run in DoubleRow perf mode (contract 256 K per instruction -> 2x).
  - Residual adds ride on the PE: psum += 64*I @ x (scaled-identity
    stationary, bf16 moving operand), so LayerNorm stats (vector bn_stats)
    read PSUM directly and the normalize is fused into the PSUM->SBUF
    eviction on the scalar engine: z = (ps - m')*inv' with
    inv' = rsqrt(var(ps) + 64^2*eps) handling the descale exactly.
  - z transposed on TensorE (identity matmul) for FFN1's moving operand;
    evicted to fp8.  FFN1 relu eviction emits fp8 g^T which is directly
    FFN2's DoubleRow stationary operand.
"""

import numpy as np
import ml_dtypes
from contextlib import ExitStack

import concourse.bass as bass
import concourse.tile as tile
from concourse import bacc, mybir
from concourse.bass_utils import run_bass_kernel_spmd

E = 512
HID = 1024
NCORES = 8
EPS = 1e-5
P = 128
WSCALE = 64.0  # fp8 weight pre-scale (power of 2; descale folded into LN)

BF16 = mybir.dt.bfloat16
F32 = mybir.dt.float32
FP8 = mybir.dt.float8e4
BF = ml_dtypes.bfloat16
F8 = ml_dtypes.float8_e4m3  # matches TRN FP8_EXP4 (max 240, inf at 1111.000)

_prog_cache = {}


def _build_program(rows_per_core: int, rmacro: int, flags: tuple):
    """Build + compile the per-core Bass program.

    flags = (use_c0, use_c1, use_d0, use_d1, use_e0, use_e1,
             aff_a0, aff_a1, aff_b0, aff_b1)
    """
    (use_c0, use_c1, use_d0, use_d1, use_e0, use_e1,
     aff_a0, aff_a1, aff_b0, aff_b1) = flags
    use_c = (use_c0, use_c1)
    use_d = (use_d0, use_d1)
    use_e = (use_e0, use_e1)
    aff_a = (aff_a0, aff_a1)
    aff_b = (aff_b0, aff_b1)

    R = rmacro
    NT = rows_per_core // R
    RC = R // P
    KE = E // P    # 4 K-chunks over E
    KH = HID // P  # 8 K-chunks over HID
    EPS_EFF = EPS * WSCALE * WSCALE
    DR = mybir.MatmulPerfMode.DoubleRow

    nc = bacc.Bacc("TRN2", target_bir_lowering=False, debug=False,
                   num_devices=NCORES)

    xbf_d, xt8_d = [], []
    for s, nm in enumerate(("dna", "mol")):
        xbf_d.append(nc.dram_tensor(f"x{s}", [NT, RC, P, E], BF16,
                                    kind="ExternalInput").ap())
        xt8_d.append(nc.dram_tensor(f"xt{s}", [NT, P, KE, R], FP8,
                                    kind="ExternalInput").ap())
    out = nc.dram_tensor("out", [NT, RC, P, 2 * E], F32,
                         kind="ExternalOutput").ap()

    wts = {}
    for s in range(2):
        wts[f"w{s}"] = nc.dram_tensor(f"w{s}", [P, KE, E], FP8,
                                      kind="ExternalInput").ap()
        wts[f"u{s}"] = nc.dram_tensor(f"u{s}", [P, KE, HID], FP8,
                                      kind="ExternalInput").ap()
        wts[f"v{s}"] = nc.dram_tensor(f"v{s}", [P, KH, E], FP8,
                                      kind="ExternalInput").ap()
        if use_c[s]:
            wts[f"c{s}"] = nc.dram_tensor(f"c{s}", [1, E], BF16,
                                          kind="ExternalInput").ap()
        if use_d[s]:
            wts[f"d{s}"] = nc.dram_tensor(f"d{s}", [1, HID], BF16,
                                          kind="ExternalInput").ap()
        if use_e[s]:
            wts[f"e{s}"] = nc.dram_tensor(f"e{s}", [1, E], BF16,
                                          kind="ExternalInput").ap()
        if aff_a[s]:
            wts[f"ga{s}"] = nc.dram_tensor(f"ga{s}", [1, E], BF16,
                                           kind="ExternalInput").ap()
            wts[f"ba{s}"] = nc.dram_tensor(f"ba{s}", [1, E], BF16,
                                           kind="ExternalInput").ap()
        if aff_b[s]:
            wts[f"gb{s}"] = nc.dram_tensor(f"gb{s}", [1, E], F32,
                                           kind="ExternalInput").ap()
            wts[f"bb{s}"] = nc.dram_tensor(f"bb{s}", [1, E], F32,
                                           kind="ExternalInput").ap()

    MULT = mybir.AluOpType.mult
    Relu = mybir.ActivationFunctionType.Relu
    Sqrt = mybir.ActivationFunctionType.Sqrt
    Ident = mybir.ActivationFunctionType.Identity

    with tile.TileContext(nc) as tc:
        with ExitStack() as ctx:
            const = ctx.enter_context(tc.tile_pool(name="const", bufs=1))
            xbf = ctx.enter_context(tc.tile_pool(name="xbf", bufs=4))
            xt8 = ctx.enter_context(tc.tile_pool(name="xt8", bufs=4))
            zbf = ctx.enter_context(tc.tile_pool(name="zbf", bufs=6))
            zt8 = ctx.enter_context(tc.tile_pool(name="zt8", bufs=4))
            gt8 = ctx.enter_context(tc.tile_pool(name="gt8", bufs=3))
            h1p = ctx.enter_context(tc.tile_pool(name="h1p", bufs=4))
            opool = ctx.enter_context(tc.tile_pool(name="opool", bufs=4))
            stats = ctx.enter_context(tc.tile_pool(name="stats", bufs=48))
            ps_att = ctx.enter_context(
                tc.tile_pool(name="ps_att", bufs=2, space="PSUM"))
            ps_g = ctx.enter_context(
                tc.tile_pool(name="ps_g", bufs=2, space="PSUM"))
            ps_f = ctx.enter_context(
                tc.tile_pool(name="ps_f", bufs=2, space="PSUM"))
            ps_t = ctx.enter_context(
                tc.tile_pool(name="ps_t", bufs=2, space="PSUM"))

            w_sb = {}
            for name, ap in wts.items():
                t = const.tile(list(ap.shape), ap.dtype, tag=f"w_{name}")
                q = nc.scalar if name.endswith("1") else nc.sync
                q.dma_start(out=t[...], in_=ap)
                w_sb[name] = t
            ident = const.tile([P, P], BF16, tag="ident")
            from concourse.masks import make_identity
            make_identity(nc, ident[...])
            # scaled identity for residual-accumulate matmuls
            ident_sc = const.tile([P, P], BF16, tag="ident_sc")
            nc.vector.tensor_scalar_mul(ident_sc[...], ident[...], WSCALE)
            # replicated affine tiles (only when needed)
            rep = {}
            for s in range(2):
                if aff_a[s]:
                    for nm in (f"ga{s}", f"ba{s}"):
                        r = const.tile([P, E], BF16, tag=f"rep_{nm}")
                        nc.sync.dma_start(out=r[...],
                                          in_=wts[nm].to_broadcast((P, E)))
                        rep[nm] = r
                if aff_b[s]:
                    for nm in (f"gb{s}", f"bb{s}"):
                        r = const.tile([P, E], F32, tag=f"rep_{nm}")
                        nc.sync.dma_start(out=r[...],
                                          in_=wts[nm].to_broadcast((P, E)))
                        rep[nm] = r

            eps_sb = const.tile([P, 1], F32, tag="eps")
            nc.vector.memset(eps_sb[...], EPS_EFF)
            ones_sb = const.tile([1, R], BF16, tag="ones")
            nc.vector.memset(ones_sb[...], 1.0)

            def layernorm_scales(ps):
                """ps: [P, E] fp32 psum holding WSCALE*(x) -> (inv, negminv)
                such that (ps - m')*inv' == LN(x) exactly."""
                st6 = stats.tile([P, 6], F32, tag="st6")
                nc.vector.bn_stats(out=st6[...], in_=ps)
                mv = stats.tile([P, 2], F32, tag="mv")
                nc.vector.bn_aggr(out=mv[...], in_=st6[...])
                inv = stats.tile([P, 1], F32, tag="inv")
                nc.scalar.activation(out=inv[...], in_=mv[:, 1:2], func=Sqrt,
                                     bias=eps_sb[...], scale=1.0)
                nc.vector.reciprocal(out=inv[...], in_=inv[...])
                nmi = stats.tile([P, 1], F32, tag="nmi")
                nc.vector.scalar_tensor_tensor(
                    out=nmi[...], in0=mv[:, 0:1], scalar=-1.0, in1=inv[...],
                    op0=MULT, op1=MULT)
                return inv, nmi

            def front(mt):
                """loads + attn (+residual via scaled identity) + LN1 -> z."""
                x_s, xT_s = [], []
                for s in range(2):
                    xt = xbf.tile([P, RC, E], BF16, tag=f"xin{s}")
                    for rc in range(RC):
                        nc.gpsimd.dma_start(out=xt[:, rc, :],
                                            in_=xbf_d[s][mt, rc])
                    x8 = xt8.tile([P, KE, R], FP8, tag=f"xT{s}")
                    nc.gpsimd.dma_start(out=x8[...], in_=xt8_d[s][mt])
                    x_s.append(xt)
                    xT_s.append(x8)

                z_s, h1_s = [], []
                for s in range(2):
                    kvT = xT_s[1 - s]
                    z = zbf.tile([P, RC, E], BF16, tag=f"z{s}")
                    for rc in range(RC):
                        ps = ps_att.tile([P, E], F32, tag="ps_att")
                        for c2 in range(KE // 2):
                            nc.tensor.matmul(
                                ps[...],
                                kvT[:, 2 * c2:2 * c2 + 2,
                                    rc * P:(rc + 1) * P],
                                w_sb[f"w{s}"][:, 2 * c2:2 * c2 + 2, :],
                                start=(c2 == 0), stop=False,
                                perf_mode=DR)
                        # residual: ps += WSCALE * x_q
                        nc.tensor.matmul(
                            ps[...], ident_sc[...], x_s[s][:, rc, :],
                            start=False, stop=(not use_c[s]),
                            skip_group_check=True)
                        if use_c[s]:
                            nc.tensor.matmul(ps[...], ones_sb[:, 0:P],
                                             w_sb[f"c{s}"][...],
                                             start=False, stop=True,
                                             skip_group_check=True)
                        inv, nmi = layernorm_scales(ps[...])
                        nc.scalar.activation(out=z[:, rc, :], in_=ps[...],
                                             func=Ident, scale=inv[...],
                                             bias=nmi[...])
                    if aff_a[s]:
                        h1 = h1p.tile([P, RC, E], BF16, tag=f"h1{s}")
                        for rc in range(RC):
                            nc.vector.tensor_mul(h1[:, rc, :], z[:, rc, :],
                                                 rep[f"ga{s}"][...])
                            nc.vector.tensor_add(h1[:, rc, :], h1[:, rc, :],
                                                 rep[f"ba{s}"][...])
                        h1_s.append(h1)
                    else:
                        h1_s.append(z)
                    z_s.append(z)
                return z_s, h1_s

            def back_a(mt, z_s):
                """zT transpose (TensorE) + FFN1 + relu -> gt_s."""
                gt_s = []
                for s in range(2):
                    z = z_s[s]
                    zT = zt8.tile([P, KE, R], FP8, tag=f"zT{s}")
                    for c in range(KE):
                        pt = ps_t.tile([P, R], BF16, tag="ps_t")
                        for rc in range(RC):
                            nc.tensor.transpose(
                                pt[:, rc * P:(rc + 1) * P],
                                z[:, rc, c * P:(c + 1) * P],
                                ident[...])
                        nc.vector.tensor_copy(out=zT[:, c, :], in_=pt[...])
                    gt = gt8.tile([P, KH, R], FP8, tag=f"gt{s}")
                    for j in range(KH):
                        pg = ps_g.tile([P, R], F32, tag="ps_g")
                        for c2 in range(KE // 2):
                            nc.tensor.matmul(
                                pg[...],
                                w_sb[f"u{s}"][:, 2 * c2:2 * c2 + 2,
                                              j * P:(j + 1) * P],
                                zT[:, 2 * c2:2 * c2 + 2, :],
                                start=(c2 == 0),
                                stop=(c2 == KE // 2 - 1 and not use_d[s]),
                                perf_mode=DR)
                        if use_d[s]:
                            nc.tensor.matmul(
                                pg[...], w_sb[f"d{s}"][:, j * P:(j + 1) * P],
                                ones_sb[:, 0:R], start=False, stop=True,
                                skip_group_check=True)
                        nc.scalar.activation(out=gt[:, j, :], in_=pg[...],
                                             func=Relu, scale=1.0 / WSCALE)
                    gt_s.append(gt)
                return gt_s

            def back_b(mt, gt_s, h1_s):
                """FFN2 + residual + LN2 + output DMA."""
                for s in range(2):
                    gt = gt_s[s]
                    h1 = h1_s[s]
                    for rc in range(RC):
                        pf = ps_f.tile([P, E], F32, tag="ps_f")
                        for j2 in range(KH // 2):
                            nc.tensor.matmul(
                                pf[...],
                                gt[:, 2 * j2:2 * j2 + 2,
                                   rc * P:(rc + 1) * P],
                                w_sb[f"v{s}"][:, 2 * j2:2 * j2 + 2, :],
                                start=(j2 == 0), stop=False,
                                perf_mode=DR)
                        # residual: pf += WSCALE * h1
                        nc.tensor.matmul(
                            pf[...], ident_sc[...], h1[:, rc, :],
                            start=False, stop=(not use_e[s]),
                            skip_group_check=True)
                        if use_e[s]:
                            nc.tensor.matmul(pf[...], ones_sb[:, 0:P],
                                             w_sb[f"e{s}"][...],
                                             start=False, stop=True,
                                             skip_group_check=True)
                        inv, nmi = layernorm_scales(pf[...])
                        o = opool.tile([P, E], F32, tag="o")
                        nc.scalar.activation(out=o[...], in_=pf[...],
                                             func=Ident, scale=inv[...],
                                             bias=nmi[...])
                        if aff_b[s]:
                            nc.vector.tensor_mul(o[...], o[...],
                                                 rep[f"gb{s}"][...])
                            nc.vector.tensor_add(o[...], o[...],
                                                 rep[f"bb{s}"][...])
                        nc.sync.dma_start(
                            out=out[mt, rc, :, s * E:(s + 1) * E], in_=o[...])

            # software pipeline, emission [A(mt), F(mt+1), B(mt)] per tile:
            # keeps this tile's relu/cast evictions at the head of the
            # in-order scalar/vector queues (no head-of-line blocking behind
            # the next tile's LN1 work) while front(mt+1) MM bursts give the
            # PE slack between FFN1 and FFN2.
            z_s, h1_s = front(0)
            for mt in range(NT):
                gt_s = back_a(mt, z_s)
                h1_cur = h1_s
                if mt + 1 < NT:
                    z_s, h1_s = front(mt + 1)
                back_b(mt, gt_s, h1_cur)

    nc.compile()
    return nc


def _prep_host(inputs):
    """Fold weights host-side; returns (full arrays, weight map, flags)."""
    g = {k: np.asarray(v, dtype=np.float32) for k, v in inputs.items()}

    def trivial(a, val):
        return bool(np.all(a == val))

    def kchunks(a, nk, dt):
        # [K, N] -> [P, nk, N] (chunk c = rows c*P:(c+1)*P)
        k, n = a.shape
        assert k == nk * P
        return np.ascontiguousarray(
            a.reshape(nk, P, n).transpose(1, 0, 2)).astype(dt)

    arrs = {}
    flags = []
    for s, (aw, ab, ow, ob, lna_g, lna_b, lnb_g, lnb_b, w1, b1, w2, b2) in \
            enumerate((
                (g["a1_in_w"], g["a1_in_b"], g["a1_out_w"], g["a1_out_b"],
                 g["ln1_g"], g["ln1_b"], g["ln3_g"], g["ln3_b"],
                 g["f1_w1"], g["f1_b1"], g["f1_w2"], g["f1_b2"]),
                (g["a2_in_w"], g["a2_in_b"], g["a2_out_w"], g["a2_out_b"],
                 g["ln2_g"], g["ln2_b"], g["ln4_g"], g["ln4_b"],
                 g["f2_w1"], g["f2_b1"], g["f2_w2"], g["f2_b2"]))):
        wv = aw[2 * E:3 * E]
        bv = ab[2 * E:3 * E]
        W = ow @ wv                      # [E, E]; attn = kv @ W.T + c
        c = ow @ bv + ob                 # [E]
        U = w1 * lna_g[None, :]          # LN1 gain folded into FFN1
        d = b1 + w1 @ lna_b              # LN1 bias folded into FFN1 bias
        V = w2                           # [E, HID]
        e = b2                           # [E]
        arrs[f"w{s}"] = kchunks(W.T * WSCALE, E // P, F8)
        arrs[f"u{s}"] = kchunks(U.T * WSCALE, E // P, F8)
        arrs[f"v{s}"] = kchunks(V.T * WSCALE, HID // P, F8)
        uc = not trivial(c, 0.0)
        ud = not trivial(d, 0.0)
        ue = not trivial(e, 0.0)
        fa = not (trivial(lna_g, 1.0) and trivial(lna_b, 0.0))
        fb = not (trivial(lnb_g, 1.0) and trivial(lnb_b, 0.0))
        if uc:
            arrs[f"c{s}"] = (c * WSCALE).reshape(1, E).astype(BF)
        if ud:
            arrs[f"d{s}"] = (d * WSCALE).reshape(1, HID).astype(BF)
        if ue:
            arrs[f"e{s}"] = (e * WSCALE).reshape(1, E).astype(BF)
        if fa:
            arrs[f"ga{s}"] = lna_g.reshape(1, E).astype(BF)
            arrs[f"ba{s}"] = lna_b.reshape(1, E).astype(BF)
        if fb:
            arrs[f"gb{s}"] = lnb_g.reshape(1, E).astype(np.float32)
            arrs[f"bb{s}"] = lnb_b.reshape(1, E).astype(np.float32)
        flags.append((uc, ud, ue, fa, fb))

    (uc0, ud0, ue0, fa0, fb0), (uc1, ud1, ue1, fa1, fb1) = flags
    flag_t = (uc0, uc1, ud0, ud1, ue0, ue1, fa0, fa1, fb0, fb1)
    return g, arrs, flag_t


def _make_in_maps(g, arrs, rows_per_core, rmacro):
    NT = rows_per_core // rmacro
    RC = rmacro // P
    KE = E // P
    in_maps = [dict(arrs) for _ in range(NCORES)]
    for s, key in enumerate(("dna", "mol")):
        x = g[key]
        xb = x.astype(BF).reshape(NCORES, NT, RC, P, E)
        # xT8[mt, p, c, r] = x[mt*R + r, c*P + p]
        x8 = np.ascontiguousarray(x.T.astype(F8).reshape(
            KE, P, NCORES, NT, rmacro).transpose(2, 3, 1, 0, 4))
        for i in range(NCORES):
            in_maps[i][f"x{s}"] = np.ascontiguousarray(xb[i])
            in_maps[i][f"xt{s}"] = x8[i]
    return in_maps


def _get_program(inputs):
    g, arrs, flag_t = _prep_host(inputs)
    B = g["dna"].shape[0]
    rows_per_core = B // NCORES
    rmacro = min(512, rows_per_core)
    key = (rows_per_core, rmacro, flag_t)
    if key not in _prog_cache:
        _prog_cache[key] = _build_program(rows_per_core, rmacro, flag_t)
    nc = _prog_cache[key]
    in_maps = _make_in_maps(g, arrs, rows_per_core, rmacro)
    return nc, in_maps, rows_per_core


def kernel(**inputs):
    nc, in_maps, rows_per_core = _get_program(inputs)
    res = run_bass_kernel_spmd(nc, in_maps, list(range(NCORES)))
    outs = [r["out"].reshape(rows_per_core, 2 * E) for r in res.results]
    return np.concatenate(outs, axis=0)


# revision 17
# speedup vs baseline: 1.8835x; 1.0215x over previous
"""Trainium2 Bass kernel for nn_CrossAttention (seq_len==1 cross attention,
dual-stream transformer block pair).

Math notes (exact simplifications, valid for any input values):
  - Both attentions have seq_len==1 for q and kv, so softmax over the single
    kv position is exactly 1.0 and attention output == V projection:
        mha(q_in, kv_in) = (kv_in @ wv.T + bv) @ out_w.T + out_b
    The q/k projections are dead code.  Folding the two matmuls:
        attn = kv_in @ (out_w @ wv).T + (out_w @ bv + out_b)
  - LayerNorm affine (g, b) of ln1/ln2 is folded into the following FFN
    weights host-side; residual-path affine and biases are applied on-device
    only when they are non-trivial (they are zeros/ones for the reference
    setup_inputs, so the fast path emits no extra instructions).

Implementation (v2, fp8 DoubleRow + host transposes):
  - Inputs per core (host-prepped): x_bf (row-major bf16, residuals),
    xT8 (feature-major fp8, attention moving/stationary operand).
  - All GEMM weights are scaled by 64 and cast to fp8e4 host-side; matmuls
    run in DoubleRow perf mode (contract 256 K per instruction -> 2x).
  - Residual adds ride on the PE: psum += 64*I @ x (scaled-identity
    stationary, bf16 moving operand), so LayerNorm stats (vector bn_stats)
    read PSUM directly and the normalize is fused into the PSUM->SBUF
    eviction on the scalar engine: z = (ps - m')*inv' with
    inv' = rsqrt(var(ps) + 64^2*eps) handling the descale exactly.
  - z transposed on TensorE (identity matmul) for FFN1's moving operand;
    evicted to fp8.  FFN1 relu eviction emits fp8 g^T which is directly
    FFN2's DoubleRow stationary operand.
"""

import numpy as np
import ml_dtypes
from contextlib import ExitStack

import concourse.bass as bass
import concourse.tile as tile
from concourse import bacc, mybir
from concourse.bass_utils import run_bass_kernel_spmd

E = 512
HID = 1024
NCORES = 8
EPS = 1e-5
P = 128
WSCALE = 64.0  # fp8 weight pre-scale (power of 2; descale folded into LN)

BF16 = mybir.dt.bfloat16
F16 = mybir.dt.float16
F32 = mybir.dt.float32
FP8 = mybir.dt.float8e4
BF = ml_dtypes.bfloat16
F16NP = np.float16
F8 = ml_dtypes.float8_e4m3  # matches TRN FP8_EXP4 (max 240, inf at 1111.000)

_prog_cache = {}


def _build_program(rows_per_core: int, rmacro: int, flags: tuple):
    """Build + compile the per-core Bass program.

    flags = (use_c0, use_c1, use_d0, use_d1, use_e0, use_e1,
             aff_a0, aff_a1, aff_b0, aff_b1)
    """
    (use_c0, use_c1, use_d0, use_d1, use_e0, use_e1,
     aff_a0, aff_a1, aff_b0, aff_b1) = flags
    use_c = (use_c0, use_c1)
    use_d = (use_d0, use_d1)
    use_e = (use_e0, use_e1)
    aff_a = (aff_a0, aff_a1)
    aff_b = (aff_b0, aff_b1)

    R = rmacro
    NT = rows_per_core // R
    RC = R // P
    KE = E // P    # 4 K-chunks over E
    KH = HID // P  # 8 K-chunks over HID
    EPS_EFF = EPS * WSCALE * WSCALE
    DR = mybir.MatmulPerfMode.DoubleRow

    nc = bacc.Bacc("TRN2", target_bir_lowering=False, debug=False,
                   num_devices=NCORES)

    xbf_d, xt8_d = [], []
    for s, nm in enumerate(("dna", "mol")):
        xbf_d.append(nc.dram_tensor(f"x{s}", [NT, RC, P, E], F16,
                                    kind="ExternalInput").ap())
        xt8_d.append(nc.dram_tensor(f"xt{s}", [NT, P, KE, R], FP8,
                                    kind="ExternalInput").ap())
    out = nc.dram_tensor("out", [NT, RC, P, 2 * E], F32,
                         kind="ExternalOutput").ap()

    wts = {}
    for s in range(2):
        wts[f"w{s}"] = nc.dram_tensor(f"w{s}", [P, KE, E], FP8,
                                      kind="ExternalInput").ap()
        wts[f"u{s}"] = nc.dram_tensor(f"u{s}", [P, KE, HID], FP8,
                                      kind="ExternalInput").ap()
        wts[f"v{s}"] = nc.dram_tensor(f"v{s}", [P, KH, E], FP8,
                                      kind="ExternalInput").ap()
        if use_c[s]:
            wts[f"c{s}"] = nc.dram_tensor(f"c{s}", [1, E], BF16,
                                          kind="ExternalInput").ap()
        if use_d[s]:
            wts[f"d{s}"] = nc.dram_tensor(f"d{s}", [1, HID], BF16,
                                          kind="ExternalInput").ap()
        if use_e[s]:
            wts[f"e{s}"] = nc.dram_tensor(f"e{s}", [1, E], BF16,
                                          kind="ExternalInput").ap()
        if aff_a[s]:
            wts[f"ga{s}"] = nc.dram_tensor(f"ga{s}", [1, E], F16,
                                           kind="ExternalInput").ap()
            wts[f"ba{s}"] = nc.dram_tensor(f"ba{s}", [1, E], F16,
                                           kind="ExternalInput").ap()
        if aff_b[s]:
            wts[f"gb{s}"] = nc.dram_tensor(f"gb{s}", [1, E], F32,
                                           kind="ExternalInput").ap()
            wts[f"bb{s}"] = nc.dram_tensor(f"bb{s}", [1, E], F32,
                                           kind="ExternalInput").ap()

    MULT = mybir.AluOpType.mult
    ADD = mybir.AluOpType.add
    Copy = mybir.ActivationFunctionType.Copy
    Relu = mybir.ActivationFunctionType.Relu
    Sqrt = mybir.ActivationFunctionType.Sqrt
    Ident = mybir.ActivationFunctionType.Identity

    with tile.TileContext(nc) as tc:
        with ExitStack() as ctx:
            const = ctx.enter_context(tc.tile_pool(name="const", bufs=1))
            xbf = ctx.enter_context(tc.tile_pool(name="xbf", bufs=4))
            xt8 = ctx.enter_context(tc.tile_pool(name="xt8", bufs=4))
            zbf = ctx.enter_context(tc.tile_pool(name="zbf", bufs=3))
            hpool = ctx.enter_context(tc.tile_pool(name="hpool", bufs=4))
            zt8 = ctx.enter_context(tc.tile_pool(name="zt8", bufs=4))
            gt8 = ctx.enter_context(tc.tile_pool(name="gt8", bufs=3))
            h1p = ctx.enter_context(tc.tile_pool(name="h1p", bufs=4))
            opool = ctx.enter_context(tc.tile_pool(name="opool", bufs=4))
            stats = ctx.enter_context(tc.tile_pool(name="stats", bufs=48))
            ps_att = ctx.enter_context(
                tc.tile_pool(name="ps_att", bufs=2, space="PSUM"))
            ps_g = ctx.enter_context(
                tc.tile_pool(name="ps_g", bufs=2, space="PSUM"))
            ps_f = ctx.enter_context(
                tc.tile_pool(name="ps_f", bufs=2, space="PSUM"))
            ps_t = ctx.enter_context(
                tc.tile_pool(name="ps_t", bufs=2, space="PSUM"))

            w_sb = {}
            for name, ap in wts.items():
                t = const.tile(list(ap.shape), ap.dtype, tag=f"w_{name}")
                q = nc.scalar if name.endswith("1") else nc.sync
                q.dma_start(out=t[...], in_=ap)
                w_sb[name] = t
            ident = const.tile([P, P], F16, tag="ident")
            from concourse.masks import make_identity
            make_identity(nc, ident[...])
            # scaled identity for residual-accumulate matmuls
            ident_sc = const.tile([P, P], F16, tag="ident_sc")
            nc.vector.tensor_scalar_mul(ident_sc[...], ident[...], WSCALE)
            # replicated affine tiles (only when needed)
            rep = {}
            for s in range(2):
                if aff_a[s]:
                    for nm in (f"ga{s}", f"ba{s}"):
                        r = const.tile([P, E], F16, tag=f"rep_{nm}")
                        nc.sync.dma_start(out=r[...],
                                          in_=wts[nm].to_broadcast((P, E)))
                        rep[nm] = r
                if aff_b[s]:
                    for nm in (f"gb{s}", f"bb{s}"):
                        r = const.tile([P, E], F32, tag=f"rep_{nm}")
                        nc.sync.dma_start(out=r[...],
                                          in_=wts[nm].to_broadcast((P, E)))
                        rep[nm] = r

            eps_sb = const.tile([P, 1], F32, tag="eps")
            nc.vector.memset(eps_sb[...], EPS)
            ones_sb = const.tile([1, R], BF16, tag="ones")
            nc.vector.memset(ones_sb[...], 1.0)

            def layernorm_scales(h):
                """h: [P, E] fp16 sbuf (descaled) -> (inv, negminv)."""
                st6 = stats.tile([P, 6], F32, tag="st6")
                nc.vector.bn_stats(out=st6[...], in_=h)
                mv = stats.tile([P, 2], F32, tag="mv")
                nc.vector.bn_aggr(out=mv[...], in_=st6[...])
                inv = stats.tile([P, 1], F32, tag="inv")
                nc.scalar.activation(out=inv[...], in_=mv[:, 1:2], func=Sqrt,
                                     bias=eps_sb[...], scale=1.0)
                nc.vector.reciprocal(out=inv[...], in_=inv[...])
                nmi = stats.tile([P, 1], F32, tag="nmi")
                nc.vector.scalar_tensor_tensor(
                    out=nmi[...], in0=mv[:, 0:1], scalar=-1.0, in1=inv[...],
                    op0=MULT, op1=MULT)
                return inv, nmi

            def front(mt):
                """loads + attn (+residual via scaled identity) + LN1 -> z."""
                x_s, xT_s = [], []
                for s in range(2):
                    xt = xbf.tile([P, RC, E], F16, tag=f"xin{s}")
                    for rc in range(RC):
                        nc.gpsimd.dma_start(out=xt[:, rc, :],
                                            in_=xbf_d[s][mt, rc])
                    x8 = xt8.tile([P, KE, R], FP8, tag=f"xT{s}")
                    nc.gpsimd.dma_start(out=x8[...], in_=xt8_d[s][mt])
                    x_s.append(xt)
                    xT_s.append(x8)

                z_s, h1_s = [], []
                for s in range(2):
                    kvT = xT_s[1 - s]
                    z = zbf.tile([P, RC, E], F16, tag=f"z{s}")
                    h = hpool.tile([P, RC, E], F16, tag=f"h{s}", bufs=2)
                    for rc in range(RC):
                        ps = ps_att.tile([P, E], F32, tag="ps_att")
                        for c2 in range(KE // 2):
                            nc.tensor.matmul(
                                ps[...],
                                kvT[:, 2 * c2:2 * c2 + 2,
                                    rc * P:(rc + 1) * P],
                                w_sb[f"w{s}"][:, 2 * c2:2 * c2 + 2, :],
                                start=(c2 == 0), stop=False,
                                perf_mode=DR)
                        # residual: ps += WSCALE * x_q
                        nc.tensor.matmul(
                            ps[...], ident_sc[...], x_s[s][:, rc, :],
                            start=False, stop=(not use_c[s]),
                            skip_group_check=True)
                        if use_c[s]:
                            nc.tensor.matmul(ps[...], ones_sb[:, 0:P],
                                             w_sb[f"c{s}"][...],
                                             start=False, stop=True,
                                             skip_group_check=True)
                        # raw descale-evict to fp16 (alternate engines),
                        # freeing the PSUM bank early; stats on fp16 (2x DVE
                        # rate), normalize on gpsimd (SBUF-only engine).
                        if rc % 2 == 0:
                            nc.scalar.activation(out=h[:, rc, :], in_=ps[...],
                                                 func=Copy,
                                                 scale=1.0 / WSCALE)
                        else:
                            nc.vector.tensor_scalar_mul(h[:, rc, :], ps[...],
                                                        1.0 / WSCALE)
                        inv, nmi = layernorm_scales(h[:, rc, :])
                        nc.gpsimd.tensor_scalar(
                            out=z[:, rc, :], in0=h[:, rc, :],
                            scalar1=inv[...], scalar2=nmi[...],
                            op0=MULT, op1=ADD)
                    if aff_a[s]:
                        h1 = h1p.tile([P, RC, E], F16, tag=f"h1{s}")
                        for rc in range(RC):
                            nc.vector.tensor_mul(h1[:, rc, :], z[:, rc, :],
                                                 rep[f"ga{s}"][...])
                            nc.vector.tensor_add(h1[:, rc, :], h1[:, rc, :],
                                                 rep[f"ba{s}"][...])
                        h1_s.append(h1)
                    else:
                        h1_s.append(z)
                    z_s.append(z)
                return z_s, h1_s

            def back_a(mt, z_s):
                """zT transpose (TensorE) + FFN1 + relu -> gt_s."""
                gt_s = []
                for s in range(2):
                    z = z_s[s]
                    zT = zt8.tile([P, KE, R], FP8, tag=f"zT{s}")
                    for c in range(KE):
                        pt = ps_t.tile([P, R], F16, tag="ps_t")
                        for rc in range(RC):
                            nc.tensor.transpose(
                                pt[:, rc * P:(rc + 1) * P],
                                z[:, rc, c * P:(c + 1) * P],
                                ident[...])
                        nc.vector.tensor_copy(out=zT[:, c, :], in_=pt[...])
                    gt = gt8.tile([P, KH, R], FP8, tag=f"gt{s}")
                    for j in range(KH):
                        pg = ps_g.tile([P, R], F32, tag="ps_g")
                        for c2 in range(KE // 2):
                            nc.tensor.matmul(
                                pg[...],
                                w_sb[f"u{s}"][:, 2 * c2:2 * c2 + 2,
                                              j * P:(j + 1) * P],
                                zT[:, 2 * c2:2 * c2 + 2, :],
                                start=(c2 == 0),
                                stop=(c2 == KE // 2 - 1 and not use_d[s]),
                                perf_mode=DR)
                        if use_d[s]:
                            nc.tensor.matmul(
                                pg[...], w_sb[f"d{s}"][:, j * P:(j + 1) * P],
                                ones_sb[:, 0:R], start=False, stop=True,
                                skip_group_check=True)
                        nc.scalar.activation(out=gt[:, j, :], in_=pg[...],
                                             func=Relu, scale=1.0 / WSCALE)
                    gt_s.append(gt)
                return gt_s

            def back_b(mt, gt_s, h1_s):
                """FFN2 + residual + LN2 + output DMA."""
                for s in range(2):
                    gt = gt_s[s]
                    h1 = h1_s[s]
                    for rc in range(RC):
                        pf = ps_f.tile([P, E], F32, tag="ps_f")
                        for j2 in range(KH // 2):
                            nc.tensor.matmul(
                                pf[...],
                                gt[:, 2 * j2:2 * j2 + 2,
                                   rc * P:(rc + 1) * P],
                                w_sb[f"v{s}"][:, 2 * j2:2 * j2 + 2, :],
                                start=(j2 == 0), stop=False,
                                perf_mode=DR)
                        # residual: pf += WSCALE * h1
                        nc.tensor.matmul(
                            pf[...], ident_sc[...], h1[:, rc, :],
                            start=False, stop=(not use_e[s]),
                            skip_group_check=True)
                        if use_e[s]:
                            nc.tensor.matmul(pf[...], ones_sb[:, 0:P],
                                             w_sb[f"e{s}"][...],
                                             start=False, stop=True,
                                             skip_group_check=True)
                        y = hpool.tile([P, E], F16, tag=f"y{s}")
                        if rc % 2 == 0:
                            nc.scalar.activation(out=y[...], in_=pf[...],
                                                 func=Copy,
                                                 scale=1.0 / WSCALE)
                        else:
                            nc.vector.tensor_scalar_mul(y[...], pf[...],
                                                        1.0 / WSCALE)
                        inv, nmi = layernorm_scales(y[...])
                        o = opool.tile([P, E], F32, tag="o")
                        nc.gpsimd.tensor_scalar(
                            out=o[...], in0=y[...],
                            scalar1=inv[...], scalar2=nmi[...],
                            op0=MULT, op1=ADD)
                        if aff_b[s]:
                            nc.vector.tensor_mul(o[...], o[...],
                                                 rep[f"gb{s}"][...])
                            nc.vector.tensor_add(o[...], o[...],
                                                 rep[f"bb{s}"][...])
                        nc.sync.dma_start(
                            out=out[mt, rc, :, s * E:(s + 1) * E], in_=o[...])

            # software pipeline, emission [A(mt), F(mt+1), B(mt)] per tile:
            # keeps this tile's relu/cast evictions at the head of the
            # in-order scalar/vector queues (no head-of-line blocking behind
            # the next tile's LN1 work) while front(mt+1) MM bursts give the
            # PE slack between FFN1 and FFN2.
            z_s, h1_s = front(0)
            for mt in range(NT):
                gt_s = back_a(mt, z_s)
                h1_cur = h1_s
                if mt + 1 < NT:
                    z_s, h1_s = front(mt + 1)
                back_b(mt, gt_s, h1_cur)

    nc.compile()
    return nc


def _prep_host(inputs):
    """Fold weights host-side; returns (full arrays, weight map, flags)."""
    g = {k: np.asarray(v, dtype=np.float32) for k, v in inputs.items()}

    def trivial(a, val):
        return bool(np.all(a == val))

    def kchunks(a, nk, dt):
        # [K, N] -> [P, nk, N] (chunk c = rows c*P:(c+1)*P)
        k, n = a.shape
        assert k == nk * P
        return np.ascontiguousarray(
            a.reshape(nk, P, n).transpose(1, 0, 2)).astype(dt)

    arrs = {}
    flags = []
    for s, (aw, ab, ow, ob, lna_g, lna_b, lnb_g, lnb_b, w1, b1, w2, b2) in \
            enumerate((
                (g["a1_in_w"], g["a1_in_b"], g["a1_out_w"], g["a1_out_b"],
                 g["ln1_g"], g["ln1_b"], g["ln3_g"], g["ln3_b"],
                 g["f1_w1"], g["f1_b1"], g["f1_w2"], g["f1_b2"]),
                (g["a2_in_w"], g["a2_in_b"], g["a2_out_w"], g["a2_out_b"],
                 g["ln2_g"], g["ln2_b"], g["ln4_g"], g["ln4_b"],
                 g["f2_w1"], g["f2_b1"], g["f2_w2"], g["f2_b2"]))):
        wv = aw[2 * E:3 * E]
        bv = ab[2 * E:3 * E]
        W = ow @ wv                      # [E, E]; attn = kv @ W.T + c
        c = ow @ bv + ob                 # [E]
        U = w1 * lna_g[None, :]          # LN1 gain folded into FFN1
        d = b1 + w1 @ lna_b              # LN1 bias folded into FFN1 bias
        V = w2                           # [E, HID]
        e = b2                           # [E]
        arrs[f"w{s}"] = kchunks(W.T * WSCALE, E // P, F8)
        arrs[f"u{s}"] = kchunks(U.T * WSCALE, E // P, F8)
        arrs[f"v{s}"] = kchunks(V.T * WSCALE, HID // P, F8)
        uc = not trivial(c, 0.0)
        ud = not trivial(d, 0.0)
        ue = not trivial(e, 0.0)
        fa = not (trivial(lna_g, 1.0) and trivial(lna_b, 0.0))
        fb = not (trivial(lnb_g, 1.0) and trivial(lnb_b, 0.0))
        if uc:
            arrs[f"c{s}"] = (c * WSCALE).reshape(1, E).astype(BF)
        if ud:
            arrs[f"d{s}"] = (d * WSCALE).reshape(1, HID).astype(BF)
        if ue:
            arrs[f"e{s}"] = (e * WSCALE).reshape(1, E).astype(BF)
        if fa:
            arrs[f"ga{s}"] = lna_g.reshape(1, E).astype(F16NP)
            arrs[f"ba{s}"] = lna_b.reshape(1, E).astype(F16NP)
        if fb:
            arrs[f"gb{s}"] = lnb_g.reshape(1, E).astype(np.float32)
            arrs[f"bb{s}"] = lnb_b.reshape(1, E).astype(np.float32)
        flags.append((uc, ud, ue, fa, fb))

    (uc0, ud0, ue0, fa0, fb0), (uc1, ud1, ue1, fa1, fb1) = flags
    flag_t = (uc0, uc1, ud0, ud1, ue0, ue1, fa0, fa1, fb0, fb1)
    return g, arrs, flag_t


def _make_in_maps(g, arrs, rows_per_core, rmacro):
    NT = rows_per_core // rmacro
    RC = rmacro // P
    KE = E // P
    in_maps = [dict(arrs) for _ in range(NCORES)]
    for s, key in enumerate(("dna", "mol")):
        x = g[key]
        xb = x.astype(F16NP).reshape(NCORES, NT, RC, P, E)
        # xT8[mt, p, c, r] = x[mt*R + r, c*P + p]
        x8 = np.ascontiguousarray(x.T.astype(F8).reshape(
            KE, P, NCORES, NT, rmacro).transpose(2, 3, 1, 0, 4))
        for i in range(NCORES):
            in_maps[i][f"x{s}"] = np.ascontiguousarray(xb[i])
            in_maps[i][f"xt{s}"] = x8[i]
    return in_maps


def _get_program(inputs):
    g, arrs, flag_t = _prep_host(inputs)
    B = g["dna"].shape[0]
    rows_per_core = B // NCORES
    rmacro = min(512, rows_per_core)
    key = (rows_per_core, rmacro, flag_t)
    if key not in _prog_cache:
        _prog_cache[key] = _build_program(rows_per_core, rmacro, flag_t)
    nc = _prog_cache[key]
    in_maps = _make_in_maps(g, arrs, rows_per_core, rmacro)
    return nc, in_maps, rows_per_core


def kernel(**inputs):
    nc, in_maps, rows_per_core = _get_program(inputs)
    res = run_bass_kernel_spmd(nc, in_maps, list(range(NCORES)))
    outs = [r["out"].reshape(rows_per_core, 2 * E) for r in res.results]
    return np.concatenate(outs, axis=0)


# revision 18
# speedup vs baseline: 2.0694x; 1.0987x over previous
"""Trainium2 Bass kernel for nn_CrossAttention (seq_len==1 cross attention,
dual-stream transformer block pair).

Math notes (exact simplifications, valid for any input values):
  - Both attentions have seq_len==1 for q and kv, so softmax over the single
    kv position is exactly 1.0 and attention output == V projection:
        mha(q_in, kv_in) = (kv_in @ wv.T + bv) @ out_w.T + out_b
    The q/k projections are dead code.  Folding the two matmuls:
        attn = kv_in @ (out_w @ wv).T + (out_w @ bv + out_b)
  - LayerNorm affine (g, b) of ln1/ln2 is folded into the following FFN
    weights host-side; residual-path affine and biases are applied on-device
    only when they are non-trivial (they are zeros/ones for the reference
    setup_inputs, so the fast path emits no extra instructions).

Implementation (v2, fp8 DoubleRow + host transposes):
  - Inputs per core (host-prepped): x_bf (row-major bf16, residuals),
    xT8 (feature-major fp8, attention moving/stationary operand).
  - All GEMM weights are scaled by 64 and cast to fp8e4 host-side; matmuls
    run in DoubleRow perf mode (contract 256 K per instruction -> 2x).
  - Residual adds ride on the PE: psum += 64*I @ x (scaled-identity
    stationary, bf16 moving operand), so LayerNorm stats (vector bn_stats)
    read PSUM directly and the normalize is fused into the PSUM->SBUF
    eviction on the scalar engine: z = (ps - m')*inv' with
    inv' = rsqrt(var(ps) + 64^2*eps) handling the descale exactly.
  - z transposed on TensorE (identity matmul) for FFN1's moving operand;
    evicted to fp8.  FFN1 relu eviction emits fp8 g^T which is directly
    FFN2's DoubleRow stationary operand.
"""

import numpy as np
import ml_dtypes
from contextlib import ExitStack

import concourse.bass as bass
import concourse.tile as tile
from concourse import bacc, mybir
from concourse.bass_utils import run_bass_kernel_spmd

E = 512
HID = 1024
NCORES = 8
EPS = 1e-5
P = 128
WSCALE = 64.0  # fp8 weight pre-scale (power of 2; descale folded into LN)

BF16 = mybir.dt.bfloat16
F16 = mybir.dt.float16
F32 = mybir.dt.float32
FP8 = mybir.dt.float8e4
BF = ml_dtypes.bfloat16
F16NP = np.float16
F8 = ml_dtypes.float8_e4m3  # matches TRN FP8_EXP4 (max 240, inf at 1111.000)

_prog_cache = {}


def _build_program(rows_per_core: int, rmacro: int, flags: tuple):
    """Build + compile the per-core Bass program.

    flags = (use_c0, use_c1, use_d0, use_d1, use_e0, use_e1,
             aff_a0, aff_a1, aff_b0, aff_b1)
    """
    (use_c0, use_c1, use_d0, use_d1, use_e0, use_e1,
     aff_a0, aff_a1, aff_b0, aff_b1) = flags
    use_c = (use_c0, use_c1)
    use_d = (use_d0, use_d1)
    use_e = (use_e0, use_e1)
    aff_a = (aff_a0, aff_a1)
    aff_b = (aff_b0, aff_b1)

    R = rmacro
    NT = rows_per_core // R
    RC = R // P
    KE = E // P    # 4 K-chunks over E
    KH = HID // P  # 8 K-chunks over HID
    EPS_EFF = EPS * WSCALE * WSCALE
    DR = mybir.MatmulPerfMode.DoubleRow

    nc = bacc.Bacc("TRN2", target_bir_lowering=False, debug=False,
                   num_devices=NCORES)

    xbf_d, xt8_d = [], []
    for s, nm in enumerate(("dna", "mol")):
        xbf_d.append(nc.dram_tensor(f"x{s}", [NT, RC, P, E], F16,
                                    kind="ExternalInput").ap())
        xt8_d.append(nc.dram_tensor(f"xt{s}", [NT, P, KE, R], FP8,
                                    kind="ExternalInput").ap())
    out = nc.dram_tensor("out", [NT, RC, P, 2 * E], F32,
                         kind="ExternalOutput").ap()

    wts = {}
    for s in range(2):
        wts[f"w{s}"] = nc.dram_tensor(f"w{s}", [P, KE, E], FP8,
                                      kind="ExternalInput").ap()
        wts[f"u{s}"] = nc.dram_tensor(f"u{s}", [P, KE, HID], FP8,
                                      kind="ExternalInput").ap()
        wts[f"v{s}"] = nc.dram_tensor(f"v{s}", [P, KH, E], FP8,
                                      kind="ExternalInput").ap()
        if use_c[s]:
            wts[f"c{s}"] = nc.dram_tensor(f"c{s}", [1, E], BF16,
                                          kind="ExternalInput").ap()
        if use_d[s]:
            wts[f"d{s}"] = nc.dram_tensor(f"d{s}", [1, HID], BF16,
                                          kind="ExternalInput").ap()
        if use_e[s]:
            wts[f"e{s}"] = nc.dram_tensor(f"e{s}", [1, E], BF16,
                                          kind="ExternalInput").ap()
        if aff_a[s]:
            wts[f"ga{s}"] = nc.dram_tensor(f"ga{s}", [1, E], F16,
                                           kind="ExternalInput").ap()
            wts[f"ba{s}"] = nc.dram_tensor(f"ba{s}", [1, E], F16,
                                           kind="ExternalInput").ap()
        if aff_b[s]:
            wts[f"gb{s}"] = nc.dram_tensor(f"gb{s}", [1, E], F32,
                                           kind="ExternalInput").ap()
            wts[f"bb{s}"] = nc.dram_tensor(f"bb{s}", [1, E], F32,
                                           kind="ExternalInput").ap()

    MULT = mybir.AluOpType.mult
    ADD = mybir.AluOpType.add
    Copy = mybir.ActivationFunctionType.Copy
    Relu = mybir.ActivationFunctionType.Relu
    Sqrt = mybir.ActivationFunctionType.Sqrt
    Ident = mybir.ActivationFunctionType.Identity

    with tile.TileContext(nc) as tc:
        with ExitStack() as ctx:
            const = ctx.enter_context(tc.tile_pool(name="const", bufs=1))
            xbf = ctx.enter_context(tc.tile_pool(name="xbf", bufs=4))
            xt8 = ctx.enter_context(tc.tile_pool(name="xt8", bufs=4))
            zbf = ctx.enter_context(tc.tile_pool(name="zbf", bufs=3))
            hpool = ctx.enter_context(tc.tile_pool(name="hpool", bufs=4))
            zt8 = ctx.enter_context(tc.tile_pool(name="zt8", bufs=4))
            gt8 = ctx.enter_context(tc.tile_pool(name="gt8", bufs=3))
            h1p = ctx.enter_context(tc.tile_pool(name="h1p", bufs=4))
            opool = ctx.enter_context(tc.tile_pool(name="opool", bufs=4))
            stats = ctx.enter_context(tc.tile_pool(name="stats", bufs=48))
            ps_att = ctx.enter_context(
                tc.tile_pool(name="ps_att", bufs=2, space="PSUM"))
            ps_g = ctx.enter_context(
                tc.tile_pool(name="ps_g", bufs=2, space="PSUM"))
            ps_f = ctx.enter_context(
                tc.tile_pool(name="ps_f", bufs=2, space="PSUM"))
            ps_t = ctx.enter_context(
                tc.tile_pool(name="ps_t", bufs=2, space="PSUM"))

            w_sb = {}
            for name, ap in wts.items():
                t = const.tile(list(ap.shape), ap.dtype, tag=f"w_{name}")
                q = nc.scalar if name.endswith("1") else nc.sync
                q.dma_start(out=t[...], in_=ap)
                w_sb[name] = t
            ident = const.tile([P, P], F16, tag="ident")
            from concourse.masks import make_identity
            make_identity(nc, ident[...])
            # scaled identity for residual-accumulate matmuls
            ident_sc = const.tile([P, P], F16, tag="ident_sc")
            nc.vector.tensor_scalar_mul(ident_sc[...], ident[...], WSCALE)
            # replicated affine tiles (only when needed)
            rep = {}
            for s in range(2):
                if aff_a[s]:
                    for nm in (f"ga{s}", f"ba{s}"):
                        r = const.tile([P, E], F16, tag=f"rep_{nm}")
                        nc.sync.dma_start(out=r[...],
                                          in_=wts[nm].to_broadcast((P, E)))
                        rep[nm] = r
                if aff_b[s]:
                    for nm in (f"gb{s}", f"bb{s}"):
                        r = const.tile([P, E], F32, tag=f"rep_{nm}")
                        nc.sync.dma_start(out=r[...],
                                          in_=wts[nm].to_broadcast((P, E)))
                        rep[nm] = r

            eps_sb = const.tile([P, 1], F32, tag="eps")
            nc.vector.memset(eps_sb[...], EPS)
            ones_sb = const.tile([1, R], BF16, tag="ones")
            nc.vector.memset(ones_sb[...], 1.0)

            def batch_scales(mv4):
                """mv4: [P, RC, 2] (mean, var) -> (inv4, nmi4) each [P, RC]."""
                inv4 = stats.tile([P, RC], F32, tag="inv4")
                nc.scalar.activation(out=inv4[...], in_=mv4[:, :, 1],
                                     func=Sqrt, bias=eps_sb[...], scale=1.0)
                nc.vector.reciprocal(out=inv4[...], in_=inv4[...])
                nmi4 = stats.tile([P, RC], F32, tag="nmi4")
                nc.vector.scalar_tensor_tensor(
                    out=nmi4[...], in0=mv4[:, :, 0], scalar=-1.0,
                    in1=inv4[...], op0=MULT, op1=MULT)
                return inv4, nmi4

            def front(mt):
                """loads + attn (+residual via scaled identity) + LN1 -> z."""
                x_s, xT_s = [], []
                for s in range(2):
                    xt = xbf.tile([P, RC, E], F16, tag=f"xin{s}")
                    for rc in range(RC):
                        nc.gpsimd.dma_start(out=xt[:, rc, :],
                                            in_=xbf_d[s][mt, rc])
                    x8 = xt8.tile([P, KE, R], FP8, tag=f"xT{s}")
                    nc.gpsimd.dma_start(out=x8[...], in_=xt8_d[s][mt])
                    x_s.append(xt)
                    xT_s.append(x8)

                z_s, h1_s = [], []
                for s in range(2):
                    kvT = xT_s[1 - s]
                    z = zbf.tile([P, RC, E], F16, tag=f"z{s}")
                    h = hpool.tile([P, RC, E], F16, tag=f"h{s}", bufs=2)
                    mv4 = stats.tile([P, RC, 2], F32, tag="mv4")
                    for rc in range(RC):
                        ps = ps_att.tile([P, E], F32, tag="ps_att")
                        for c2 in range(KE // 2):
                            nc.tensor.matmul(
                                ps[...],
                                kvT[:, 2 * c2:2 * c2 + 2,
                                    rc * P:(rc + 1) * P],
                                w_sb[f"w{s}"][:, 2 * c2:2 * c2 + 2, :],
                                start=(c2 == 0), stop=False,
                                perf_mode=DR)
                        # residual: ps += WSCALE * x_q
                        nc.tensor.matmul(
                            ps[...], ident_sc[...], x_s[s][:, rc, :],
                            start=False, stop=(not use_c[s]),
                            skip_group_check=True)
                        if use_c[s]:
                            nc.tensor.matmul(ps[...], ones_sb[:, 0:P],
                                             w_sb[f"c{s}"][...],
                                             start=False, stop=True,
                                             skip_group_check=True)
                        # raw descale-evict to fp16 (alternate engines) frees
                        # the PSUM bank immediately; stats on fp16 follow.
                        if rc % 2 == 0:
                            nc.scalar.activation(out=h[:, rc, :], in_=ps[...],
                                                 func=Copy,
                                                 scale=1.0 / WSCALE)
                        else:
                            nc.vector.tensor_scalar_mul(h[:, rc, :], ps[...],
                                                        1.0 / WSCALE)
                        st6 = stats.tile([P, 6], F32, tag="st6")
                        nc.vector.bn_stats(out=st6[...], in_=h[:, rc, :])
                        nc.vector.bn_aggr(out=mv4[:, rc, :], in_=st6[...])
                    # batched [P,RC] scale chain: one sqrt/recip/stt for all
                    # RC chunks, then per-chunk normalize on gpsimd.
                    inv4, nmi4 = batch_scales(mv4)
                    for rc in range(RC):
                        nc.gpsimd.tensor_scalar(
                            out=z[:, rc, :], in0=h[:, rc, :],
                            scalar1=inv4[:, rc:rc + 1],
                            scalar2=nmi4[:, rc:rc + 1],
                            op0=MULT, op1=ADD)
                    if aff_a[s]:
                        h1 = h1p.tile([P, RC, E], F16, tag=f"h1{s}")
                        for rc in range(RC):
                            nc.vector.tensor_mul(h1[:, rc, :], z[:, rc, :],
                                                 rep[f"ga{s}"][...])
                            nc.vector.tensor_add(h1[:, rc, :], h1[:, rc, :],
                                                 rep[f"ba{s}"][...])
                        h1_s.append(h1)
                    else:
                        h1_s.append(z)
                    z_s.append(z)
                return z_s, h1_s

            def back_a(mt, z_s):
                """zT transpose (TensorE) + FFN1 + relu -> gt_s."""
                gt_s = []
                for s in range(2):
                    z = z_s[s]
                    zT = zt8.tile([P, KE, R], FP8, tag=f"zT{s}")
                    for c in range(KE):
                        pt = ps_t.tile([P, R], F16, tag="ps_t")
                        for rc in range(RC):
                            nc.tensor.transpose(
                                pt[:, rc * P:(rc + 1) * P],
                                z[:, rc, c * P:(c + 1) * P],
                                ident[...])
                        nc.vector.tensor_copy(out=zT[:, c, :], in_=pt[...])
                    gt = gt8.tile([P, KH, R], FP8, tag=f"gt{s}")
                    for j in range(KH):
                        pg = ps_g.tile([P, R], F32, tag="ps_g")
                        for c2 in range(KE // 2):
                            nc.tensor.matmul(
                                pg[...],
                                w_sb[f"u{s}"][:, 2 * c2:2 * c2 + 2,
                                              j * P:(j + 1) * P],
                                zT[:, 2 * c2:2 * c2 + 2, :],
                                start=(c2 == 0),
                                stop=(c2 == KE // 2 - 1 and not use_d[s]),
                                perf_mode=DR)
                        if use_d[s]:
                            nc.tensor.matmul(
                                pg[...], w_sb[f"d{s}"][:, j * P:(j + 1) * P],
                                ones_sb[:, 0:R], start=False, stop=True,
                                skip_group_check=True)
                        nc.scalar.activation(out=gt[:, j, :], in_=pg[...],
                                             func=Relu, scale=1.0 / WSCALE)
                    gt_s.append(gt)
                return gt_s

            def back_b(mt, gt_s, h1_s):
                """FFN2 + residual + LN2 + output DMA."""
                for s in range(2):
                    gt = gt_s[s]
                    h1 = h1_s[s]
                    y = hpool.tile([P, RC, E], F16, tag=f"y{s}", bufs=2)
                    mv4 = stats.tile([P, RC, 2], F32, tag="mv4b")
                    for rc in range(RC):
                        pf = ps_f.tile([P, E], F32, tag="ps_f")
                        for j2 in range(KH // 2):
                            nc.tensor.matmul(
                                pf[...],
                                gt[:, 2 * j2:2 * j2 + 2,
                                   rc * P:(rc + 1) * P],
                                w_sb[f"v{s}"][:, 2 * j2:2 * j2 + 2, :],
                                start=(j2 == 0), stop=False,
                                perf_mode=DR)
                        # residual: pf += WSCALE * h1
                        nc.tensor.matmul(
                            pf[...], ident_sc[...], h1[:, rc, :],
                            start=False, stop=(not use_e[s]),
                            skip_group_check=True)
                        if use_e[s]:
                            nc.tensor.matmul(pf[...], ones_sb[:, 0:P],
                                             w_sb[f"e{s}"][...],
                                             start=False, stop=True,
                                             skip_group_check=True)
                        if rc % 2 == 0:
                            nc.scalar.activation(out=y[:, rc, :], in_=pf[...],
                                                 func=Copy,
                                                 scale=1.0 / WSCALE)
                        else:
                            nc.vector.tensor_scalar_mul(y[:, rc, :], pf[...],
                                                        1.0 / WSCALE)
                        st6 = stats.tile([P, 6], F32, tag="st6")
                        nc.vector.bn_stats(out=st6[...], in_=y[:, rc, :])
                        nc.vector.bn_aggr(out=mv4[:, rc, :], in_=st6[...])
                    inv4, nmi4 = batch_scales(mv4)
                    for rc in range(RC):
                        o = opool.tile([P, E], F32, tag="o")
                        nc.gpsimd.tensor_scalar(
                            out=o[...], in0=y[:, rc, :],
                            scalar1=inv4[:, rc:rc + 1],
                            scalar2=nmi4[:, rc:rc + 1],
                            op0=MULT, op1=ADD)
                        if aff_b[s]:
                            nc.vector.tensor_mul(o[...], o[...],
                                                 rep[f"gb{s}"][...])
                            nc.vector.tensor_add(o[...], o[...],
                                                 rep[f"bb{s}"][...])
                        nc.sync.dma_start(
                            out=out[mt, rc, :, s * E:(s + 1) * E], in_=o[...])

            # software pipeline, emission [A(mt), F(mt+1), B(mt)] per tile:
            # keeps this tile's relu/cast evictions at the head of the
            # in-order scalar/vector queues (no head-of-line blocking behind
            # the next tile's LN1 work) while front(mt+1) MM bursts give the
            # PE slack between FFN1 and FFN2.
            z_s, h1_s = front(0)
            for mt in range(NT):
                gt_s = back_a(mt, z_s)
                h1_cur = h1_s
                if mt + 1 < NT:
                    z_s, h1_s = front(mt + 1)
                back_b(mt, gt_s, h1_cur)

    nc.compile()
    return nc


def _prep_host(inputs):
    """Fold weights host-side; returns (full arrays, weight map, flags)."""
    g = {k: np.asarray(v, dtype=np.float32) for k, v in inputs.items()}

    def trivial(a, val):
        return bool(np.all(a == val))

    def kchunks(a, nk, dt):
        # [K, N] -> [P, nk, N] (chunk c = rows c*P:(c+1)*P)
        k, n = a.shape
        assert k == nk * P
        return np.ascontiguousarray(
            a.reshape(nk, P, n).transpose(1, 0, 2)).astype(dt)

    arrs = {}
    flags = []
    for s, (aw, ab, ow, ob, lna_g, lna_b, lnb_g, lnb_b, w1, b1, w2, b2) in \
            enumerate((
                (g["a1_in_w"], g["a1_in_b"], g["a1_out_w"], g["a1_out_b"],
                 g["ln1_g"], g["ln1_b"], g["ln3_g"], g["ln3_b"],
                 g["f1_w1"], g["f1_b1"], g["f1_w2"], g["f1_b2"]),
                (g["a2_in_w"], g["a2_in_b"], g["a2_out_w"], g["a2_out_b"],
                 g["ln2_g"], g["ln2_b"], g["ln4_g"], g["ln4_b"],
                 g["f2_w1"], g["f2_b1"], g["f2_w2"], g["f2_b2"]))):
        wv = aw[2 * E:3 * E]
        bv = ab[2 * E:3 * E]
        W = ow @ wv                      # [E, E]; attn = kv @ W.T + c
        c = ow @ bv + ob                 # [E]
        U = w1 * lna_g[None, :]          # LN1 gain folded into FFN1
        d = b1 + w1 @ lna_b              # LN1 bias folded into FFN1 bias
        V = w2                           # [E, HID]
        e = b2                           # [E]
        arrs[f"w{s}"] = kchunks(W.T * WSCALE, E // P, F8)
        arrs[f"u{s}"] = kchunks(U.T * WSCALE, E // P, F8)
        arrs[f"v{s}"] = kchunks(V.T * WSCALE, HID // P, F8)
        uc = not trivial(c, 0.0)
        ud = not trivial(d, 0.0)
        ue = not trivial(e, 0.0)
        fa = not (trivial(lna_g, 1.0) and trivial(lna_b, 0.0))
        fb = not (trivial(lnb_g, 1.0) and trivial(lnb_b, 0.0))
        if uc:
            arrs[f"c{s}"] = (c * WSCALE).reshape(1, E).astype(BF)
        if ud:
            arrs[f"d{s}"] = (d * WSCALE).reshape(1, HID).astype(BF)
        if ue:
            arrs[f"e{s}"] = (e * WSCALE).reshape(1, E).astype(BF)
        if fa:
            arrs[f"ga{s}"] = lna_g.reshape(1, E).astype(F16NP)
            arrs[f"ba{s}"] = lna_b.reshape(1, E).astype(F16NP)
        if fb:
            arrs[f"gb{s}"] = lnb_g.reshape(1, E).astype(np.float32)
            arrs[f"bb{s}"] = lnb_b.reshape(1, E).astype(np.float32)
        flags.append((uc, ud, ue, fa, fb))

    (uc0, ud0, ue0, fa0, fb0), (uc1, ud1, ue1, fa1, fb1) = flags
    flag_t = (uc0, uc1, ud0, ud1, ue0, ue1, fa0, fa1, fb0, fb1)
    return g, arrs, flag_t


def _make_in_maps(g, arrs, rows_per_core, rmacro):
    NT = rows_per_core // rmacro
    RC = rmacro // P
    KE = E // P
    in_maps = [dict(arrs) for _ in range(NCORES)]
    for s, key in enumerate(("dna", "mol")):
        x = g[key]
        xb = x.astype(F16NP).reshape(NCORES, NT, RC, P, E)
        # xT8[mt, p, c, r] = x[mt*R + r, c*P + p]
        x8 = np.ascontiguousarray(x.T.astype(F8).reshape(
            KE, P, NCORES, NT, rmacro).transpose(2, 3, 1, 0, 4))
        for i in range(NCORES):
            in_maps[i][f"x{s}"] = np.ascontiguousarray(xb[i])
            in_maps[i][f"xt{s}"] = x8[i]
    return in_maps


def _get_program(inputs):
    g, arrs, flag_t = _prep_host(inputs)
    B = g["dna"].shape[0]
    rows_per_core = B // NCORES
    rmacro = min(512, rows_per_core)
    key = (rows_per_core, rmacro, flag_t)
    if key not in _prog_cache:
        _prog_cache[key] = _build_program(rows_per_core, rmacro, flag_t)
    nc = _prog_cache[key]
    in_maps = _make_in_maps(g, arrs, rows_per_core, rmacro)
    return nc, in_maps, rows_per_core


def kernel(**inputs):
    nc, in_maps, rows_per_core = _get_program(inputs)
    res = run_bass_kernel_spmd(nc, in_maps, list(range(NCORES)))
    outs = [r["out"].reshape(rows_per_core, 2 * E) for r in res.results]
    return np.concatenate(outs, axis=0)


# revision 19
# speedup vs baseline: 2.0992x; 1.0144x over previous
"""Trainium2 Bass kernel for nn_CrossAttention (seq_len==1 cross attention,
dual-stream transformer block pair).

Math notes (exact simplifications, valid for any input values):
  - Both attentions have seq_len==1 for q and kv, so softmax over the single
    kv position is exactly 1.0 and attention output == V projection:
        mha(q_in, kv_in) = (kv_in @ wv.T + bv) @ out_w.T + out_b
    The q/k projections are dead code.  Folding the two matmuls:
        attn = kv_in @ (out_w @ wv).T + (out_w @ bv + out_b)
  - LayerNorm affine (g, b) of ln1/ln2 is folded into the following FFN
    weights host-side; residual-path affine and biases are applied on-device
    only when they are non-trivial (they are zeros/ones for the reference
    setup_inputs, so the fast path emits no extra instructions).

Implementation (v2, fp8 DoubleRow + host transposes):
  - Inputs per core (host-prepped): x_bf (row-major bf16, residuals),
    xT8 (feature-major fp8, attention moving/stationary operand).
  - All GEMM weights are scaled by 64 and cast to fp8e4 host-side; matmuls
    run in DoubleRow perf mode (contract 256 K per instruction -> 2x).
  - Residual adds ride on the PE: psum += 64*I @ x (scaled-identity
    stationary, bf16 moving operand), so LayerNorm stats (vector bn_stats)
    read PSUM directly and the normalize is fused into the PSUM->SBUF
    eviction on the scalar engine: z = (ps - m')*inv' with
    inv' = rsqrt(var(ps) + 64^2*eps) handling the descale exactly.
  - z transposed on TensorE (identity matmul) for FFN1's moving operand;
    evicted to fp8.  FFN1 relu eviction emits fp8 g^T which is directly
    FFN2's DoubleRow stationary operand.
"""

import numpy as np
import ml_dtypes
from contextlib import ExitStack

import concourse.bass as bass
import concourse.tile as tile
from concourse import bacc, mybir
from concourse.bass_utils import run_bass_kernel_spmd

E = 512
HID = 1024
NCORES = 8
EPS = 1e-5
P = 128
WSCALE = 64.0  # fp8 weight pre-scale (power of 2; descale folded into LN)

BF16 = mybir.dt.bfloat16
F16 = mybir.dt.float16
F32 = mybir.dt.float32
FP8 = mybir.dt.float8e4
BF = ml_dtypes.bfloat16
F16NP = np.float16
F8 = ml_dtypes.float8_e4m3  # matches TRN FP8_EXP4 (max 240, inf at 1111.000)

_prog_cache = {}


def _build_program(rows_per_core: int, rmacro: int, flags: tuple):
    """Build + compile the per-core Bass program.

    flags = (use_c0, use_c1, use_d0, use_d1, use_e0, use_e1,
             aff_a0, aff_a1, aff_b0, aff_b1)
    """
    (use_c0, use_c1, use_d0, use_d1, use_e0, use_e1,
     aff_a0, aff_a1, aff_b0, aff_b1) = flags
    use_c = (use_c0, use_c1)
    use_d = (use_d0, use_d1)
    use_e = (use_e0, use_e1)
    aff_a = (aff_a0, aff_a1)
    aff_b = (aff_b0, aff_b1)

    R = rmacro
    NT = rows_per_core // R
    RC = R // P
    KE = E // P    # 4 K-chunks over E
    KH = HID // P  # 8 K-chunks over HID
    EPS_EFF = EPS * WSCALE * WSCALE
    DR = mybir.MatmulPerfMode.DoubleRow

    nc = bacc.Bacc("TRN2", target_bir_lowering=False, debug=False,
                   num_devices=NCORES)

    xbf_d, xt8_d = [], []
    for s, nm in enumerate(("dna", "mol")):
        xbf_d.append(nc.dram_tensor(f"x{s}", [NT, RC, P, E], F16,
                                    kind="ExternalInput").ap())
        xt8_d.append(nc.dram_tensor(f"xt{s}", [NT, P, KE, R], FP8,
                                    kind="ExternalInput").ap())
    out = nc.dram_tensor("out", [NT, RC, P, 2 * E], F32,
                         kind="ExternalOutput").ap()

    wts = {}
    for s in range(2):
        wts[f"w{s}"] = nc.dram_tensor(f"w{s}", [P, KE, E], FP8,
                                      kind="ExternalInput").ap()
        wts[f"u{s}"] = nc.dram_tensor(f"u{s}", [P, KE, HID], FP8,
                                      kind="ExternalInput").ap()
        wts[f"v{s}"] = nc.dram_tensor(f"v{s}", [P, KH, E], FP8,
                                      kind="ExternalInput").ap()
        if use_c[s]:
            wts[f"c{s}"] = nc.dram_tensor(f"c{s}", [1, E], BF16,
                                          kind="ExternalInput").ap()
        if use_d[s]:
            wts[f"d{s}"] = nc.dram_tensor(f"d{s}", [1, HID], BF16,
                                          kind="ExternalInput").ap()
        if use_e[s]:
            wts[f"e{s}"] = nc.dram_tensor(f"e{s}", [1, E], BF16,
                                          kind="ExternalInput").ap()
        if aff_a[s]:
            wts[f"ga{s}"] = nc.dram_tensor(f"ga{s}", [1, E], F16,
                                           kind="ExternalInput").ap()
            wts[f"ba{s}"] = nc.dram_tensor(f"ba{s}", [1, E], F16,
                                           kind="ExternalInput").ap()
        if aff_b[s]:
            wts[f"gb{s}"] = nc.dram_tensor(f"gb{s}", [1, E], F32,
                                           kind="ExternalInput").ap()
            wts[f"bb{s}"] = nc.dram_tensor(f"bb{s}", [1, E], F32,
                                           kind="ExternalInput").ap()

    MULT = mybir.AluOpType.mult
    ADD = mybir.AluOpType.add
    Copy = mybir.ActivationFunctionType.Copy
    Relu = mybir.ActivationFunctionType.Relu
    Sqrt = mybir.ActivationFunctionType.Sqrt
    Ident = mybir.ActivationFunctionType.Identity

    with tile.TileContext(nc) as tc:
        with ExitStack() as ctx:
            const = ctx.enter_context(tc.tile_pool(name="const", bufs=1))
            xbf = ctx.enter_context(tc.tile_pool(name="xbf", bufs=4))
            xt8 = ctx.enter_context(tc.tile_pool(name="xt8", bufs=4))
            zbf = ctx.enter_context(tc.tile_pool(name="zbf", bufs=3))
            hpool = ctx.enter_context(tc.tile_pool(name="hpool", bufs=4))
            zt8 = ctx.enter_context(tc.tile_pool(name="zt8", bufs=4))
            gt8 = ctx.enter_context(tc.tile_pool(name="gt8", bufs=3))
            h1p = ctx.enter_context(tc.tile_pool(name="h1p", bufs=4))
            opool = ctx.enter_context(tc.tile_pool(name="opool", bufs=4))
            stats = ctx.enter_context(tc.tile_pool(name="stats", bufs=48))
            ps_att = ctx.enter_context(
                tc.tile_pool(name="ps_att", bufs=2, space="PSUM"))
            ps_g = ctx.enter_context(
                tc.tile_pool(name="ps_g", bufs=2, space="PSUM"))
            ps_f = ctx.enter_context(
                tc.tile_pool(name="ps_f", bufs=2, space="PSUM"))
            ps_t = ctx.enter_context(
                tc.tile_pool(name="ps_t", bufs=2, space="PSUM"))

            w_sb = {}
            for name, ap in wts.items():
                t = const.tile(list(ap.shape), ap.dtype, tag=f"w_{name}")
                q = nc.scalar if name.endswith("1") else nc.sync
                q.dma_start(out=t[...], in_=ap)
                w_sb[name] = t
            ident = const.tile([P, P], F16, tag="ident")
            from concourse.masks import make_identity
            make_identity(nc, ident[...])
            # scaled identity for residual-accumulate matmuls
            ident_sc = const.tile([P, P], F16, tag="ident_sc")
            nc.vector.tensor_scalar_mul(ident_sc[...], ident[...], WSCALE)
            # replicated affine tiles (only when needed)
            rep = {}
            for s in range(2):
                if aff_a[s]:
                    for nm in (f"ga{s}", f"ba{s}"):
                        r = const.tile([P, E], F16, tag=f"rep_{nm}")
                        nc.sync.dma_start(out=r[...],
                                          in_=wts[nm].to_broadcast((P, E)))
                        rep[nm] = r
                if aff_b[s]:
                    for nm in (f"gb{s}", f"bb{s}"):
                        r = const.tile([P, E], F32, tag=f"rep_{nm}")
                        nc.sync.dma_start(out=r[...],
                                          in_=wts[nm].to_broadcast((P, E)))
                        rep[nm] = r

            eps_sb = const.tile([P, 1], F32, tag="eps")
            nc.vector.memset(eps_sb[...], EPS)
            ones_sb = const.tile([1, R], BF16, tag="ones")
            nc.vector.memset(ones_sb[...], 1.0)

            def batch_scales(mv4):
                """mv4: [P, RC, 2] (mean, var) -> (inv4, nmi4) each [P, RC]."""
                inv4 = stats.tile([P, RC], F32, tag="inv4")
                nc.scalar.activation(out=inv4[...], in_=mv4[:, :, 1],
                                     func=Sqrt, bias=eps_sb[...], scale=1.0)
                nc.vector.reciprocal(out=inv4[...], in_=inv4[...])
                nmi4 = stats.tile([P, RC], F32, tag="nmi4")
                nc.vector.scalar_tensor_tensor(
                    out=nmi4[...], in0=mv4[:, :, 0], scalar=-1.0,
                    in1=inv4[...], op0=MULT, op1=MULT)
                return inv4, nmi4

            def front_load(mt):
                """prefetch x (fp16 row-major) + xT (fp8) for tile mt."""
                x_s, xT_s = [], []
                for s in range(2):
                    xt = xbf.tile([P, RC, E], F16, tag=f"xin{s}")
                    for rc in range(RC):
                        nc.gpsimd.dma_start(out=xt[:, rc, :],
                                            in_=xbf_d[s][mt, rc])
                    x8 = xt8.tile([P, KE, R], FP8, tag=f"xT{s}")
                    nc.gpsimd.dma_start(out=x8[...], in_=xt8_d[s][mt])
                    x_s.append(xt)
                    xT_s.append(x8)
                return x_s, xT_s

            def front_compute(mt, tiles):
                """attn (+residual via scaled identity) + LN1 -> z."""
                x_s, xT_s = tiles
                z_s, h1_s = [], []
                for s in range(2):
                    kvT = xT_s[1 - s]
                    z = zbf.tile([P, RC, E], F16, tag=f"z{s}")
                    h = hpool.tile([P, RC, E], F16, tag=f"h{s}", bufs=2)
                    mv4 = stats.tile([P, RC, 2], F32, tag="mv4")
                    for rc in range(RC):
                        ps = ps_att.tile([P, E], F32, tag="ps_att")
                        for c2 in range(KE // 2):
                            nc.tensor.matmul(
                                ps[...],
                                kvT[:, 2 * c2:2 * c2 + 2,
                                    rc * P:(rc + 1) * P],
                                w_sb[f"w{s}"][:, 2 * c2:2 * c2 + 2, :],
                                start=(c2 == 0), stop=False,
                                perf_mode=DR)
                        # residual: ps += WSCALE * x_q
                        nc.tensor.matmul(
                            ps[...], ident_sc[...], x_s[s][:, rc, :],
                            start=False, stop=(not use_c[s]),
                            skip_group_check=True)
                        if use_c[s]:
                            nc.tensor.matmul(ps[...], ones_sb[:, 0:P],
                                             w_sb[f"c{s}"][...],
                                             start=False, stop=True,
                                             skip_group_check=True)
                        # raw descale-evict to fp16 (alternate engines) frees
                        # the PSUM bank immediately; stats on fp16 follow.
                        if rc % 2 == 0:
                            nc.scalar.activation(out=h[:, rc, :], in_=ps[...],
                                                 func=Copy,
                                                 scale=1.0 / WSCALE)
                        else:
                            nc.vector.tensor_scalar_mul(h[:, rc, :], ps[...],
                                                        1.0 / WSCALE)
                        st6 = stats.tile([P, 6], F32, tag="st6")
                        nc.vector.bn_stats(out=st6[...], in_=h[:, rc, :])
                        nc.vector.bn_aggr(out=mv4[:, rc, :], in_=st6[...])
                    # batched [P,RC] scale chain: one sqrt/recip/stt for all
                    # RC chunks, then per-chunk normalize on gpsimd.
                    inv4, nmi4 = batch_scales(mv4)
                    for rc in range(RC):
                        nc.gpsimd.tensor_scalar(
                            out=z[:, rc, :], in0=h[:, rc, :],
                            scalar1=inv4[:, rc:rc + 1],
                            scalar2=nmi4[:, rc:rc + 1],
                            op0=MULT, op1=ADD)
                    if aff_a[s]:
                        h1 = h1p.tile([P, RC, E], F16, tag=f"h1{s}")
                        for rc in range(RC):
                            nc.vector.tensor_mul(h1[:, rc, :], z[:, rc, :],
                                                 rep[f"ga{s}"][...])
                            nc.vector.tensor_add(h1[:, rc, :], h1[:, rc, :],
                                                 rep[f"ba{s}"][...])
                        h1_s.append(h1)
                    else:
                        h1_s.append(z)
                    z_s.append(z)
                return z_s, h1_s

            def back_a(mt, z_s):
                """zT transpose (TensorE) + FFN1 + relu -> gt_s."""
                gt_s = []
                for s in range(2):
                    z = z_s[s]
                    zT = zt8.tile([P, KE, R], FP8, tag=f"zT{s}")
                    for c in range(KE):
                        pt = ps_t.tile([P, R], F16, tag="ps_t")
                        for rc in range(RC):
                            nc.tensor.transpose(
                                pt[:, rc * P:(rc + 1) * P],
                                z[:, rc, c * P:(c + 1) * P],
                                ident[...])
                        nc.vector.tensor_copy(out=zT[:, c, :], in_=pt[...])
                    gt = gt8.tile([P, KH, R], FP8, tag=f"gt{s}")
                    for j in range(KH):
                        pg = ps_g.tile([P, R], F32, tag="ps_g")
                        for c2 in range(KE // 2):
                            nc.tensor.matmul(
                                pg[...],
                                w_sb[f"u{s}"][:, 2 * c2:2 * c2 + 2,
                                              j * P:(j + 1) * P],
                                zT[:, 2 * c2:2 * c2 + 2, :],
                                start=(c2 == 0),
                                stop=(c2 == KE // 2 - 1 and not use_d[s]),
                                perf_mode=DR)
                        if use_d[s]:
                            nc.tensor.matmul(
                                pg[...], w_sb[f"d{s}"][:, j * P:(j + 1) * P],
                                ones_sb[:, 0:R], start=False, stop=True,
                                skip_group_check=True)
                        nc.scalar.activation(out=gt[:, j, :], in_=pg[...],
                                             func=Relu, scale=1.0 / WSCALE)
                    gt_s.append(gt)
                return gt_s

            def back_b(mt, gt_s, h1_s):
                """FFN2 + residual + LN2 + output DMA."""
                for s in range(2):
                    gt = gt_s[s]
                    h1 = h1_s[s]
                    y = hpool.tile([P, RC, E], F16, tag=f"y{s}", bufs=2)
                    mv4 = stats.tile([P, RC, 2], F32, tag="mv4b")
                    for rc in range(RC):
                        pf = ps_f.tile([P, E], F32, tag="ps_f")
                        for j2 in range(KH // 2):
                            nc.tensor.matmul(
                                pf[...],
                                gt[:, 2 * j2:2 * j2 + 2,
                                   rc * P:(rc + 1) * P],
                                w_sb[f"v{s}"][:, 2 * j2:2 * j2 + 2, :],
                                start=(j2 == 0), stop=False,
                                perf_mode=DR)
                        # residual: pf += WSCALE * h1
                        nc.tensor.matmul(
                            pf[...], ident_sc[...], h1[:, rc, :],
                            start=False, stop=(not use_e[s]),
                            skip_group_check=True)
                        if use_e[s]:
                            nc.tensor.matmul(pf[...], ones_sb[:, 0:P],
                                             w_sb[f"e{s}"][...],
                                             start=False, stop=True,
                                             skip_group_check=True)
                        if rc % 2 == 0:
                            nc.scalar.activation(out=y[:, rc, :], in_=pf[...],
                                                 func=Copy,
                                                 scale=1.0 / WSCALE)
                        else:
                            nc.vector.tensor_scalar_mul(y[:, rc, :], pf[...],
                                                        1.0 / WSCALE)
                        st6 = stats.tile([P, 6], F32, tag="st6")
                        nc.vector.bn_stats(out=st6[...], in_=y[:, rc, :])
                        nc.vector.bn_aggr(out=mv4[:, rc, :], in_=st6[...])
                    inv4, nmi4 = batch_scales(mv4)
                    for rc in range(RC):
                        o = opool.tile([P, E], F32, tag="o")
                        nc.gpsimd.tensor_scalar(
                            out=o[...], in0=y[:, rc, :],
                            scalar1=inv4[:, rc:rc + 1],
                            scalar2=nmi4[:, rc:rc + 1],
                            op0=MULT, op1=ADD)
                        if aff_b[s]:
                            nc.vector.tensor_mul(o[...], o[...],
                                                 rep[f"gb{s}"][...])
                            nc.vector.tensor_add(o[...], o[...],
                                                 rep[f"bb{s}"][...])
                        nc.sync.dma_start(
                            out=out[mt, rc, :, s * E:(s + 1) * E], in_=o[...])

            # software pipeline, emission [F(mt+1), load(mt+2), A(mt),
            # B(mt)] per cycle: back_a(mt) consumes a z finished a full
            # cycle earlier (no transpose stall on the LN1 norm chain),
            # input DMAs prefetch two tiles ahead, and this tile's
            # relu/cast evictions stay near the head of the in-order
            # scalar/vector queues.
            tiles = {0: front_load(0)}
            z_s, h1_s = front_compute(0, tiles.pop(0))
            if NT > 1:
                tiles[1] = front_load(1)
            for mt in range(NT):
                z_cur, h1_cur = z_s, h1_s
                if mt + 1 < NT:
                    z_s, h1_s = front_compute(mt + 1, tiles.pop(mt + 1))
                if mt + 2 < NT:
                    tiles[mt + 2] = front_load(mt + 2)
                gt_s = back_a(mt, z_cur)
                back_b(mt, gt_s, h1_cur)

    nc.compile()
    return nc


def _prep_host(inputs):
    """Fold weights host-side; returns (full arrays, weight map, flags)."""
    g = {k: np.asarray(v, dtype=np.float32) for k, v in inputs.items()}

    def trivial(a, val):
        return bool(np.all(a == val))

    def kchunks(a, nk, dt):
        # [K, N] -> [P, nk, N] (chunk c = rows c*P:(c+1)*P)
        k, n = a.shape
        assert k == nk * P
        return np.ascontiguousarray(
            a.reshape(nk, P, n).transpose(1, 0, 2)).astype(dt)

    arrs = {}
    flags = []
    for s, (aw, ab, ow, ob, lna_g, lna_b, lnb_g, lnb_b, w1, b1, w2, b2) in \
            enumerate((
                (g["a1_in_w"], g["a1_in_b"], g["a1_out_w"], g["a1_out_b"],
                 g["ln1_g"], g["ln1_b"], g["ln3_g"], g["ln3_b"],
                 g["f1_w1"], g["f1_b1"], g["f1_w2"], g["f1_b2"]),
                (g["a2_in_w"], g["a2_in_b"], g["a2_out_w"], g["a2_out_b"],
                 g["ln2_g"], g["ln2_b"], g["ln4_g"], g["ln4_b"],
                 g["f2_w1"], g["f2_b1"], g["f2_w2"], g["f2_b2"]))):
        wv = aw[2 * E:3 * E]
        bv = ab[2 * E:3 * E]
        W = ow @ wv                      # [E, E]; attn = kv @ W.T + c
        c = ow @ bv + ob                 # [E]
        U = w1 * lna_g[None, :]          # LN1 gain folded into FFN1
        d = b1 + w1 @ lna_b              # LN1 bias folded into FFN1 bias
        V = w2                           # [E, HID]
        e = b2                           # [E]
        arrs[f"w{s}"] = kchunks(W.T * WSCALE, E // P, F8)
        arrs[f"u{s}"] = kchunks(U.T * WSCALE, E // P, F8)
        arrs[f"v{s}"] = kchunks(V.T * WSCALE, HID // P, F8)
        uc = not trivial(c, 0.0)
        ud = not trivial(d, 0.0)
        ue = not trivial(e, 0.0)
        fa = not (trivial(lna_g, 1.0) and trivial(lna_b, 0.0))
        fb = not (trivial(lnb_g, 1.0) and trivial(lnb_b, 0.0))
        if uc:
            arrs[f"c{s}"] = (c * WSCALE).reshape(1, E).astype(BF)
        if ud:
            arrs[f"d{s}"] = (d * WSCALE).reshape(1, HID).astype(BF)
        if ue:
            arrs[f"e{s}"] = (e * WSCALE).reshape(1, E).astype(BF)
        if fa:
            arrs[f"ga{s}"] = lna_g.reshape(1, E).astype(F16NP)
            arrs[f"ba{s}"] = lna_b.reshape(1, E).astype(F16NP)
        if fb:
            arrs[f"gb{s}"] = lnb_g.reshape(1, E).astype(np.float32)
            arrs[f"bb{s}"] = lnb_b.reshape(1, E).astype(np.float32)
        flags.append((uc, ud, ue, fa, fb))

    (uc0, ud0, ue0, fa0, fb0), (uc1, ud1, ue1, fa1, fb1) = flags
    flag_t = (uc0, uc1, ud0, ud1, ue0, ue1, fa0, fa1, fb0, fb1)
    return g, arrs, flag_t


def _make_in_maps(g, arrs, rows_per_core, rmacro):
    NT = rows_per_core // rmacro
    RC = rmacro // P
    KE = E // P
    in_maps = [dict(arrs) for _ in range(NCORES)]
    for s, key in enumerate(("dna", "mol")):
        x = g[key]
        xb = x.astype(F16NP).reshape(NCORES, NT, RC, P, E)
        # xT8[mt, p, c, r] = x[mt*R + r, c*P + p]
        x8 = np.ascontiguousarray(x.T.astype(F8).reshape(
            KE, P, NCORES, NT, rmacro).transpose(2, 3, 1, 0, 4))
        for i in range(NCORES):
            in_maps[i][f"x{s}"] = np.ascontiguousarray(xb[i])
            in_maps[i][f"xt{s}"] = x8[i]
    return in_maps


def _get_program(inputs):
    g, arrs, flag_t = _prep_host(inputs)
    B = g["dna"].shape[0]
    rows_per_core = B // NCORES
    rmacro = min(512, rows_per_core)
    key = (rows_per_core, rmacro, flag_t)
    if key not in _prog_cache:
        _prog_cache[key] = _build_program(rows_per_core, rmacro, flag_t)
    nc = _prog_cache[key]
    in_maps = _make_in_maps(g, arrs, rows_per_core, rmacro)
    return nc, in_maps, rows_per_core


def kernel(**inputs):
    nc, in_maps, rows_per_core = _get_program(inputs)
    res = run_bass_kernel_spmd(nc, in_maps, list(range(NCORES)))
    outs = [r["out"].reshape(rows_per_core, 2 * E) for r in res.results]
    return np.concatenate(outs, axis=0)


# revision 21
# speedup vs baseline: 2.1577x; 1.0279x over previous
"""Trainium2 Bass kernel for nn_CrossAttention (seq_len==1 cross attention,
dual-stream transformer block pair).

Math notes (exact simplifications, valid for any input values):
  - Both attentions have seq_len==1 for q and kv, so softmax over the single
    kv position is exactly 1.0 and attention output == V projection:
        mha(q_in, kv_in) = (kv_in @ wv.T + bv) @ out_w.T + out_b
    The q/k projections are dead code.  Folding the two matmuls:
        attn = kv_in @ (out_w @ wv).T + (out_w @ bv + out_b)
  - LayerNorm affine (g, b) of ln1/ln2 is folded into the following FFN
    weights host-side; residual-path affine and biases are applied on-device
    only when they are non-trivial (they are zeros/ones for the reference
    setup_inputs, so the fast path emits no extra instructions).

Implementation (v2, fp8 DoubleRow + host transposes):
  - Inputs per core (host-prepped): x_bf (row-major bf16, residuals),
    xT8 (feature-major fp8, attention moving/stationary operand).
  - All GEMM weights are scaled by 64 and cast to fp8e4 host-side; matmuls
    run in DoubleRow perf mode (contract 256 K per instruction -> 2x).
  - Residual adds ride on the PE: psum += 64*I @ x (scaled-identity
    stationary, bf16 moving operand), so LayerNorm stats (vector bn_stats)
    read PSUM directly and the normalize is fused into the PSUM->SBUF
    eviction on the scalar engine: z = (ps - m')*inv' with
    inv' = rsqrt(var(ps) + 64^2*eps) handling the descale exactly.
  - z transposed on TensorE (identity matmul) for FFN1's moving operand;
    evicted to fp8.  FFN1 relu eviction emits fp8 g^T which is directly
    FFN2's DoubleRow stationary operand.
"""

import numpy as np
import ml_dtypes
from contextlib import ExitStack

import concourse.bass as bass
import concourse.tile as tile
from concourse import bacc, mybir
from concourse.bass_utils import run_bass_kernel_spmd

E = 512
HID = 1024
NCORES = 8
EPS = 1e-5
P = 128
WSCALE = 64.0  # fp8 weight pre-scale (power of 2; descale folded into LN)

BF16 = mybir.dt.bfloat16
F16 = mybir.dt.float16
F32 = mybir.dt.float32
FP8 = mybir.dt.float8e4
BF = ml_dtypes.bfloat16
F16NP = np.float16
F8 = ml_dtypes.float8_e4m3  # matches TRN FP8_EXP4 (max 240, inf at 1111.000)

_prog_cache = {}


def _build_program(rows_per_core: int, rmacro: int, flags: tuple):
    """Build + compile the per-core Bass program.

    flags = (use_c0, use_c1, use_d0, use_d1, use_e0, use_e1,
             aff_a0, aff_a1, aff_b0, aff_b1)
    """
    (use_c0, use_c1, use_d0, use_d1, use_e0, use_e1,
     aff_a0, aff_a1, aff_b0, aff_b1) = flags
    use_c = (use_c0, use_c1)
    use_d = (use_d0, use_d1)
    use_e = (use_e0, use_e1)
    aff_a = (aff_a0, aff_a1)
    aff_b = (aff_b0, aff_b1)

    R = rmacro
    NT = rows_per_core // R
    RC = R // P
    KE = E // P    # 4 K-chunks over E
    KH = HID // P  # 8 K-chunks over HID
    EPS_EFF = EPS * WSCALE * WSCALE
    DR = mybir.MatmulPerfMode.DoubleRow

    nc = bacc.Bacc("TRN2", target_bir_lowering=False, debug=False,
                   num_devices=NCORES)

    xbf_d, xt8_d = [], []
    for s, nm in enumerate(("dna", "mol")):
        xbf_d.append(nc.dram_tensor(f"x{s}", [NT, RC, P, E], F16,
                                    kind="ExternalInput").ap())
        xt8_d.append(nc.dram_tensor(f"xt{s}", [NT, P, KE, R], FP8,
                                    kind="ExternalInput").ap())
    out = nc.dram_tensor("out", [NT, RC, P, 2 * E], F32,
                         kind="ExternalOutput").ap()

    wts = {}
    for s in range(2):
        wts[f"w{s}"] = nc.dram_tensor(f"w{s}", [P, KE, E], FP8,
                                      kind="ExternalInput").ap()
        wts[f"u{s}"] = nc.dram_tensor(f"u{s}", [P, KE, HID], FP8,
                                      kind="ExternalInput").ap()
        wts[f"v{s}"] = nc.dram_tensor(f"v{s}", [P, KH, E], FP8,
                                      kind="ExternalInput").ap()
        if use_c[s]:
            wts[f"c{s}"] = nc.dram_tensor(f"c{s}", [1, E], BF16,
                                          kind="ExternalInput").ap()
        if use_d[s]:
            wts[f"d{s}"] = nc.dram_tensor(f"d{s}", [1, HID], BF16,
                                          kind="ExternalInput").ap()
        if use_e[s]:
            wts[f"e{s}"] = nc.dram_tensor(f"e{s}", [1, E], BF16,
                                          kind="ExternalInput").ap()
        if aff_a[s]:
            wts[f"ga{s}"] = nc.dram_tensor(f"ga{s}", [1, E], F16,
                                           kind="ExternalInput").ap()
            wts[f"ba{s}"] = nc.dram_tensor(f"ba{s}", [1, E], F16,
                                           kind="ExternalInput").ap()
        if aff_b[s]:
            wts[f"gb{s}"] = nc.dram_tensor(f"gb{s}", [1, E], F32,
                                           kind="ExternalInput").ap()
            wts[f"bb{s}"] = nc.dram_tensor(f"bb{s}", [1, E], F32,
                                           kind="ExternalInput").ap()

    MULT = mybir.AluOpType.mult
    ADD = mybir.AluOpType.add
    Copy = mybir.ActivationFunctionType.Copy
    Relu = mybir.ActivationFunctionType.Relu
    Sqrt = mybir.ActivationFunctionType.Sqrt
    Ident = mybir.ActivationFunctionType.Identity

    with tile.TileContext(nc) as tc:
        with ExitStack() as ctx:
            const = ctx.enter_context(tc.tile_pool(name="const", bufs=1))
            xbf = ctx.enter_context(tc.tile_pool(name="xbf", bufs=4))
            xt8 = ctx.enter_context(tc.tile_pool(name="xt8", bufs=4))
            zbf = ctx.enter_context(tc.tile_pool(name="zbf", bufs=3))
            hpool = ctx.enter_context(tc.tile_pool(name="hpool", bufs=4))
            zt8 = ctx.enter_context(tc.tile_pool(name="zt8", bufs=4))
            gt8 = ctx.enter_context(tc.tile_pool(name="gt8", bufs=3))
            h1p = ctx.enter_context(tc.tile_pool(name="h1p", bufs=4))
            opool = ctx.enter_context(tc.tile_pool(name="opool", bufs=4))
            stats = ctx.enter_context(tc.tile_pool(name="stats", bufs=48))
            ps_att = ctx.enter_context(
                tc.tile_pool(name="ps_att", bufs=2, space="PSUM"))
            ps_g = ctx.enter_context(
                tc.tile_pool(name="ps_g", bufs=2, space="PSUM"))
            ps_f = ctx.enter_context(
                tc.tile_pool(name="ps_f", bufs=2, space="PSUM"))
            ps_t = ctx.enter_context(
                tc.tile_pool(name="ps_t", bufs=2, space="PSUM"))

            w_sb = {}
            worder = sorted(wts, key=lambda n: (n[0] != "w", n[0] != "u"))
            for name in worder:
                ap = wts[name]
                t = const.tile(list(ap.shape), ap.dtype, tag=f"w_{name}")
                q = nc.scalar if name.endswith("1") else nc.sync
                q.dma_start(out=t[...], in_=ap)
                w_sb[name] = t
            ident = const.tile([P, P], F16, tag="ident")
            from concourse.masks import make_identity
            make_identity(nc, ident[...])
            # scaled identity for residual-accumulate matmuls
            ident_sc = const.tile([P, P], F16, tag="ident_sc")
            nc.vector.tensor_scalar_mul(ident_sc[...], ident[...], WSCALE)
            # replicated affine tiles (only when needed)
            rep = {}
            for s in range(2):
                if aff_a[s]:
                    for nm in (f"ga{s}", f"ba{s}"):
                        r = const.tile([P, E], F16, tag=f"rep_{nm}")
                        nc.sync.dma_start(out=r[...],
                                          in_=wts[nm].to_broadcast((P, E)))
                        rep[nm] = r
                if aff_b[s]:
                    for nm in (f"gb{s}", f"bb{s}"):
                        r = const.tile([P, E], F32, tag=f"rep_{nm}")
                        nc.sync.dma_start(out=r[...],
                                          in_=wts[nm].to_broadcast((P, E)))
                        rep[nm] = r

            eps_sb = const.tile([P, 1], F32, tag="eps")
            nc.vector.memset(eps_sb[...], EPS)
            ones_sb = const.tile([1, R], BF16, tag="ones")
            nc.vector.memset(ones_sb[...], 1.0)

            def batch_scales(mv4):
                """mv4: [P, RC, 2] (mean, var) -> (inv4, nmi4) each [P, RC]."""
                inv4 = stats.tile([P, RC], F32, tag="inv4")
                nc.scalar.activation(out=inv4[...], in_=mv4[:, :, 1],
                                     func=Sqrt, bias=eps_sb[...], scale=1.0)
                nc.vector.reciprocal(out=inv4[...], in_=inv4[...])
                nmi4 = stats.tile([P, RC], F32, tag="nmi4")
                nc.vector.scalar_tensor_tensor(
                    out=nmi4[...], in0=mv4[:, :, 0], scalar=-1.0,
                    in1=inv4[...], op0=MULT, op1=MULT)
                return inv4, nmi4

            def front_load(mt):
                """prefetch x (fp16 row-major) + xT (fp8) for tile mt."""
                x_s, xT_s = [], []
                for s in range(2):
                    xt = xbf.tile([P, RC, E], F16, tag=f"xin{s}")
                    for rc in range(RC):
                        nc.gpsimd.dma_start(out=xt[:, rc, :],
                                            in_=xbf_d[s][mt, rc])
                    x8 = xt8.tile([P, KE, R], FP8, tag=f"xT{s}")
                    nc.gpsimd.dma_start(out=x8[...], in_=xt8_d[s][mt])
                    x_s.append(xt)
                    xT_s.append(x8)
                return x_s, xT_s

            def front_compute(mt, tiles):
                """attn (+residual via scaled identity) + LN1 -> z."""
                x_s, xT_s = tiles
                z_s, h1_s = [], []
                for s in range(2):
                    kvT = xT_s[1 - s]
                    z = zbf.tile([P, RC, E], F16, tag=f"z{s}")
                    h = hpool.tile([P, RC, E], F16, tag=f"h{s}", bufs=2)
                    mv4 = stats.tile([P, RC, 2], F32, tag="mv4")
                    for rc in range(RC):
                        ps = ps_att.tile([P, E], F32, tag="ps_att")
                        for c2 in range(KE // 2):
                            nc.tensor.matmul(
                                ps[...],
                                kvT[:, 2 * c2:2 * c2 + 2,
                                    rc * P:(rc + 1) * P],
                                w_sb[f"w{s}"][:, 2 * c2:2 * c2 + 2, :],
                                start=(c2 == 0), stop=False,
                                perf_mode=DR)
                        # residual: ps += WSCALE * x_q
                        nc.tensor.matmul(
                            ps[...], ident_sc[...], x_s[s][:, rc, :],
                            start=False, stop=(not use_c[s]),
                            skip_group_check=True)
                        if use_c[s]:
                            nc.tensor.matmul(ps[...], ones_sb[:, 0:P],
                                             w_sb[f"c{s}"][...],
                                             start=False, stop=True,
                                             skip_group_check=True)
                        # raw descale-evict to fp16 (alternate engines) frees
                        # the PSUM bank immediately; stats on fp16 follow.
                        if rc % 2 == 0:
                            nc.scalar.activation(out=h[:, rc, :], in_=ps[...],
                                                 func=Copy,
                                                 scale=1.0 / WSCALE)
                        else:
                            nc.vector.tensor_scalar_mul(h[:, rc, :], ps[...],
                                                        1.0 / WSCALE)
                        st6 = stats.tile([P, 6], F32, tag="st6")
                        nc.vector.bn_stats(out=st6[...], in_=h[:, rc, :])
                        nc.vector.bn_aggr(out=mv4[:, rc, :], in_=st6[...])
                    # batched [P,RC] scale chain: one sqrt/recip/stt for all
                    # RC chunks, then per-chunk normalize on gpsimd.
                    inv4, nmi4 = batch_scales(mv4)
                    for rc in range(RC):
                        nc.gpsimd.tensor_scalar(
                            out=z[:, rc, :], in0=h[:, rc, :],
                            scalar1=inv4[:, rc:rc + 1],
                            scalar2=nmi4[:, rc:rc + 1],
                            op0=MULT, op1=ADD)
                    if aff_a[s]:
                        h1 = h1p.tile([P, RC, E], F16, tag=f"h1{s}")
                        for rc in range(RC):
                            nc.vector.tensor_mul(h1[:, rc, :], z[:, rc, :],
                                                 rep[f"ga{s}"][...])
                            nc.vector.tensor_add(h1[:, rc, :], h1[:, rc, :],
                                                 rep[f"ba{s}"][...])
                        h1_s.append(h1)
                    else:
                        h1_s.append(z)
                    z_s.append(z)
                return z_s, h1_s

            def back_a(mt, z_s):
                """zT transpose (TensorE) + FFN1 + relu -> gt_s."""
                gt_s = []
                for s in range(2):
                    z = z_s[s]
                    zT = zt8.tile([P, KE, R], FP8, tag=f"zT{s}")
                    for c in range(KE):
                        pt = ps_t.tile([P, R], F16, tag="ps_t")
                        for rc in range(RC):
                            nc.tensor.transpose(
                                pt[:, rc * P:(rc + 1) * P],
                                z[:, rc, c * P:(c + 1) * P],
                                ident[...])
                        if c % 2 == 0:
                            nc.vector.tensor_copy(out=zT[:, c, :],
                                                  in_=pt[...])
                        else:
                            nc.scalar.copy(out=zT[:, c, :], in_=pt[...])
                    gt = gt8.tile([P, KH, R], FP8, tag=f"gt{s}")
                    for j in range(KH):
                        pg = ps_g.tile([P, R], F32, tag="ps_g")
                        for c2 in range(KE // 2):
                            nc.tensor.matmul(
                                pg[...],
                                w_sb[f"u{s}"][:, 2 * c2:2 * c2 + 2,
                                              j * P:(j + 1) * P],
                                zT[:, 2 * c2:2 * c2 + 2, :],
                                start=(c2 == 0),
                                stop=(c2 == KE // 2 - 1 and not use_d[s]),
                                perf_mode=DR)
                        if use_d[s]:
                            nc.tensor.matmul(
                                pg[...], w_sb[f"d{s}"][:, j * P:(j + 1) * P],
                                ones_sb[:, 0:R], start=False, stop=True,
                                skip_group_check=True)
                        nc.scalar.activation(out=gt[:, j, :], in_=pg[...],
                                             func=Relu, scale=1.0 / WSCALE)
                    gt_s.append(gt)
                return gt_s

            def back_b(mt, gt_s, h1_s):
                """FFN2 + residual + LN2 + output DMA."""
                for s in range(2):
                    gt = gt_s[s]
                    h1 = h1_s[s]
                    y = hpool.tile([P, RC, E], F16, tag=f"y{s}", bufs=2)
                    mv4 = stats.tile([P, RC, 2], F32, tag="mv4b")
                    for rc in range(RC):
                        pf = ps_f.tile([P, E], F32, tag="ps_f")
                        for j2 in range(KH // 2):
                            nc.tensor.matmul(
                                pf[...],
                                gt[:, 2 * j2:2 * j2 + 2,
                                   rc * P:(rc + 1) * P],
                                w_sb[f"v{s}"][:, 2 * j2:2 * j2 + 2, :],
                                start=(j2 == 0), stop=False,
                                perf_mode=DR)
                        # residual: pf += WSCALE * h1
                        nc.tensor.matmul(
                            pf[...], ident_sc[...], h1[:, rc, :],
                            start=False, stop=(not use_e[s]),
                            skip_group_check=True)
                        if use_e[s]:
                            nc.tensor.matmul(pf[...], ones_sb[:, 0:P],
                                             w_sb[f"e{s}"][...],
                                             start=False, stop=True,
                                             skip_group_check=True)
                        if rc % 2 == 0:
                            nc.scalar.activation(out=y[:, rc, :], in_=pf[...],
                                                 func=Copy,
                                                 scale=1.0 / WSCALE)
                        else:
                            nc.vector.tensor_scalar_mul(y[:, rc, :], pf[...],
                                                        1.0 / WSCALE)
                        st6 = stats.tile([P, 6], F32, tag="st6")
                        nc.vector.bn_stats(out=st6[...], in_=y[:, rc, :])
                        nc.vector.bn_aggr(out=mv4[:, rc, :], in_=st6[...])
                    inv4, nmi4 = batch_scales(mv4)
                    for rc in range(RC):
                        o = opool.tile([P, E], F32, tag="o")
                        nc.gpsimd.tensor_scalar(
                            out=o[...], in0=y[:, rc, :],
                            scalar1=inv4[:, rc:rc + 1],
                            scalar2=nmi4[:, rc:rc + 1],
                            op0=MULT, op1=ADD)
                        if aff_b[s]:
                            nc.vector.tensor_mul(o[...], o[...],
                                                 rep[f"gb{s}"][...])
                            nc.vector.tensor_add(o[...], o[...],
                                                 rep[f"bb{s}"][...])
                        nc.sync.dma_start(
                            out=out[mt, rc, :, s * E:(s + 1) * E], in_=o[...])

            # software pipeline, emission [F(mt+1), load(mt+2), A(mt),
            # B(mt)] per cycle: back_a(mt) consumes a z finished a full
            # cycle earlier (no transpose stall on the LN1 norm chain),
            # input DMAs prefetch two tiles ahead, and this tile's
            # relu/cast evictions stay near the head of the in-order
            # scalar/vector queues.
            tiles = {0: front_load(0)}
            z_s, h1_s = front_compute(0, tiles.pop(0))
            if NT > 1:
                tiles[1] = front_load(1)
            for mt in range(NT):
                z_cur, h1_cur = z_s, h1_s
                if mt + 1 < NT:
                    z_s, h1_s = front_compute(mt + 1, tiles.pop(mt + 1))
                if mt + 2 < NT:
                    tiles[mt + 2] = front_load(mt + 2)
                gt_s = back_a(mt, z_cur)
                back_b(mt, gt_s, h1_cur)

    nc.compile()
    return nc


def _prep_host(inputs):
    """Fold weights host-side; returns (full arrays, weight map, flags)."""
    g = {k: np.asarray(v, dtype=np.float32) for k, v in inputs.items()}

    def trivial(a, val):
        return bool(np.all(a == val))

    def kchunks(a, nk, dt):
        # [K, N] -> [P, nk, N] (chunk c = rows c*P:(c+1)*P)
        k, n = a.shape
        assert k == nk * P
        return np.ascontiguousarray(
            a.reshape(nk, P, n).transpose(1, 0, 2)).astype(dt)

    arrs = {}
    flags = []
    for s, (aw, ab, ow, ob, lna_g, lna_b, lnb_g, lnb_b, w1, b1, w2, b2) in \
            enumerate((
                (g["a1_in_w"], g["a1_in_b"], g["a1_out_w"], g["a1_out_b"],
                 g["ln1_g"], g["ln1_b"], g["ln3_g"], g["ln3_b"],
                 g["f1_w1"], g["f1_b1"], g["f1_w2"], g["f1_b2"]),
                (g["a2_in_w"], g["a2_in_b"], g["a2_out_w"], g["a2_out_b"],
                 g["ln2_g"], g["ln2_b"], g["ln4_g"], g["ln4_b"],
                 g["f2_w1"], g["f2_b1"], g["f2_w2"], g["f2_b2"]))):
        wv = aw[2 * E:3 * E]
        bv = ab[2 * E:3 * E]
        W = ow @ wv                      # [E, E]; attn = kv @ W.T + c
        c = ow @ bv + ob                 # [E]
        U = w1 * lna_g[None, :]          # LN1 gain folded into FFN1
        d = b1 + w1 @ lna_b              # LN1 bias folded into FFN1 bias
        V = w2                           # [E, HID]
        e = b2                           # [E]
        arrs[f"w{s}"] = kchunks(W.T * WSCALE, E // P, F8)
        arrs[f"u{s}"] = kchunks(U.T * WSCALE, E // P, F8)
        arrs[f"v{s}"] = kchunks(V.T * WSCALE, HID // P, F8)
        uc = not trivial(c, 0.0)
        ud = not trivial(d, 0.0)
        ue = not trivial(e, 0.0)
        fa = not (trivial(lna_g, 1.0) and trivial(lna_b, 0.0))
        fb = not (trivial(lnb_g, 1.0) and trivial(lnb_b, 0.0))
        if uc:
            arrs[f"c{s}"] = (c * WSCALE).reshape(1, E).astype(BF)
        if ud:
            arrs[f"d{s}"] = (d * WSCALE).reshape(1, HID).astype(BF)
        if ue:
            arrs[f"e{s}"] = (e * WSCALE).reshape(1, E).astype(BF)
        if fa:
            arrs[f"ga{s}"] = lna_g.reshape(1, E).astype(F16NP)
            arrs[f"ba{s}"] = lna_b.reshape(1, E).astype(F16NP)
        if fb:
            arrs[f"gb{s}"] = lnb_g.reshape(1, E).astype(np.float32)
            arrs[f"bb{s}"] = lnb_b.reshape(1, E).astype(np.float32)
        flags.append((uc, ud, ue, fa, fb))

    (uc0, ud0, ue0, fa0, fb0), (uc1, ud1, ue1, fa1, fb1) = flags
    flag_t = (uc0, uc1, ud0, ud1, ue0, ue1, fa0, fa1, fb0, fb1)
    return g, arrs, flag_t


def _make_in_maps(g, arrs, rows_per_core, rmacro):
    NT = rows_per_core // rmacro
    RC = rmacro // P
    KE = E // P
    in_maps = [dict(arrs) for _ in range(NCORES)]
    for s, key in enumerate(("dna", "mol")):
        x = g[key]
        xb = x.astype(F16NP).reshape(NCORES, NT, RC, P, E)
        # xT8[mt, p, c, r] = x[mt*R + r, c*P + p]
        x8 = np.ascontiguousarray(x.T.astype(F8).reshape(
            KE, P, NCORES, NT, rmacro).transpose(2, 3, 1, 0, 4))
        for i in range(NCORES):
            in_maps[i][f"x{s}"] = np.ascontiguousarray(xb[i])
            in_maps[i][f"xt{s}"] = x8[i]
    return in_maps


def _get_program(inputs):
    g, arrs, flag_t = _prep_host(inputs)
    B = g["dna"].shape[0]
    rows_per_core = B // NCORES
    rmacro = min(512, rows_per_core)
    key = (rows_per_core, rmacro, flag_t)
    if key not in _prog_cache:
        _prog_cache[key] = _build_program(rows_per_core, rmacro, flag_t)
    nc = _prog_cache[key]
    in_maps = _make_in_maps(g, arrs, rows_per_core, rmacro)
    return nc, in_maps, rows_per_core


def kernel(**inputs):
    nc, in_maps, rows_per_core = _get_program(inputs)
    res = run_bass_kernel_spmd(nc, in_maps, list(range(NCORES)))
    outs = [r["out"].reshape(rows_per_core, 2 * E) for r in res.results]
    return np.concatenate(outs, axis=0)


# revision 22
# speedup vs baseline: 2.1634x; 1.0027x over previous
"""Trainium2 Bass kernel for nn_CrossAttention (seq_len==1 cross attention,
dual-stream transformer block pair).

Math notes (exact simplifications, valid for any input values):
  - Both attentions have seq_len==1 for q and kv, so softmax over the single
    kv position is exactly 1.0 and attention output == V projection:
        mha(q_in, kv_in) = (kv_in @ wv.T + bv) @ out_w.T + out_b
    The q/k projections are dead code.  Folding the two matmuls:
        attn = kv_in @ (out_w @ wv).T + (out_w @ bv + out_b)
  - LayerNorm affine (g, b) of ln1/ln2 is folded into the following FFN
    weights host-side; residual-path affine and biases are applied on-device
    only when they are non-trivial (they are zeros/ones for the reference
    setup_inputs, so the fast path emits no extra instructions).

Implementation (v2, fp8 DoubleRow + host transposes):
  - Inputs per core (host-prepped): x_bf (row-major bf16, residuals),
    xT8 (feature-major fp8, attention moving/stationary operand).
  - All GEMM weights are scaled by 64 and cast to fp8e4 host-side; matmuls
    run in DoubleRow perf mode (contract 256 K per instruction -> 2x).
  - Residual adds ride on the PE: psum += 64*I @ x (scaled-identity
    stationary, bf16 moving operand), so LayerNorm stats (vector bn_stats)
    read PSUM directly and the normalize is fused into the PSUM->SBUF
    eviction on the scalar engine: z = (ps - m')*inv' with
    inv' = rsqrt(var(ps) + 64^2*eps) handling the descale exactly.
  - z transposed on TensorE (identity matmul) for FFN1's moving operand;
    evicted to fp8.  FFN1 relu eviction emits fp8 g^T which is directly
    FFN2's DoubleRow stationary operand.
"""

import numpy as np
import ml_dtypes
from contextlib import ExitStack

import concourse.bass as bass
import concourse.tile as tile
from concourse import bacc, mybir
from concourse.bass_utils import run_bass_kernel_spmd

E = 512
HID = 1024
NCORES = 8
EPS = 1e-5
P = 128
WSCALE = 64.0  # fp8 weight pre-scale (power of 2; descale folded into LN)

BF16 = mybir.dt.bfloat16
F16 = mybir.dt.float16
F32 = mybir.dt.float32
FP8 = mybir.dt.float8e4
BF = ml_dtypes.bfloat16
F16NP = np.float16
F8 = ml_dtypes.float8_e4m3  # matches TRN FP8_EXP4 (max 240, inf at 1111.000)

_prog_cache = {}


def _build_program(rows_per_core: int, rmacro: int, flags: tuple):
    """Build + compile the per-core Bass program.

    flags = (use_c0, use_c1, use_d0, use_d1, use_e0, use_e1,
             aff_a0, aff_a1, aff_b0, aff_b1)
    """
    (use_c0, use_c1, use_d0, use_d1, use_e0, use_e1,
     aff_a0, aff_a1, aff_b0, aff_b1) = flags
    use_c = (use_c0, use_c1)
    use_d = (use_d0, use_d1)
    use_e = (use_e0, use_e1)
    aff_a = (aff_a0, aff_a1)
    aff_b = (aff_b0, aff_b1)

    R = rmacro
    NT = rows_per_core // R
    RC = R // P
    KE = E // P    # 4 K-chunks over E
    KH = HID // P  # 8 K-chunks over HID
    EPS_EFF = EPS * WSCALE * WSCALE
    DR = mybir.MatmulPerfMode.DoubleRow

    nc = bacc.Bacc("TRN2", target_bir_lowering=False, debug=False,
                   num_devices=NCORES)

    xbf_d, xt8_d = [], []
    for s, nm in enumerate(("dna", "mol")):
        xbf_d.append(nc.dram_tensor(f"x{s}", [NT, RC, P, E], F16,
                                    kind="ExternalInput").ap())
        xt8_d.append(nc.dram_tensor(f"xt{s}", [NT, P, KE, R], FP8,
                                    kind="ExternalInput").ap())
    out = nc.dram_tensor("out", [NT, RC, P, 2 * E], F32,
                         kind="ExternalOutput").ap()

    wts = {}
    for s in range(2):
        wts[f"w{s}"] = nc.dram_tensor(f"w{s}", [P, KE, E], FP8,
                                      kind="ExternalInput").ap()
        wts[f"u{s}"] = nc.dram_tensor(f"u{s}", [P, KE, HID], FP8,
                                      kind="ExternalInput").ap()
        wts[f"v{s}"] = nc.dram_tensor(f"v{s}", [P, KH, E], FP8,
                                      kind="ExternalInput").ap()
        if use_c[s]:
            wts[f"c{s}"] = nc.dram_tensor(f"c{s}", [1, E], BF16,
                                          kind="ExternalInput").ap()
        if use_d[s]:
            wts[f"d{s}"] = nc.dram_tensor(f"d{s}", [1, HID], BF16,
                                          kind="ExternalInput").ap()
        if use_e[s]:
            wts[f"e{s}"] = nc.dram_tensor(f"e{s}", [1, E], BF16,
                                          kind="ExternalInput").ap()
        if aff_a[s]:
            wts[f"ga{s}"] = nc.dram_tensor(f"ga{s}", [1, E], F16,
                                           kind="ExternalInput").ap()
            wts[f"ba{s}"] = nc.dram_tensor(f"ba{s}", [1, E], F16,
                                           kind="ExternalInput").ap()
        if aff_b[s]:
            wts[f"gb{s}"] = nc.dram_tensor(f"gb{s}", [1, E], F32,
                                           kind="ExternalInput").ap()
            wts[f"bb{s}"] = nc.dram_tensor(f"bb{s}", [1, E], F32,
                                           kind="ExternalInput").ap()

    MULT = mybir.AluOpType.mult
    ADD = mybir.AluOpType.add
    Copy = mybir.ActivationFunctionType.Copy
    Relu = mybir.ActivationFunctionType.Relu
    Sqrt = mybir.ActivationFunctionType.Sqrt
    Ident = mybir.ActivationFunctionType.Identity

    with tile.TileContext(nc) as tc:
        with ExitStack() as ctx:
            const = ctx.enter_context(tc.tile_pool(name="const", bufs=1))
            xbf = ctx.enter_context(tc.tile_pool(name="xbf", bufs=4))
            xt8 = ctx.enter_context(tc.tile_pool(name="xt8", bufs=4))
            zbf = ctx.enter_context(tc.tile_pool(name="zbf", bufs=4))
            hpool = ctx.enter_context(tc.tile_pool(name="hpool", bufs=4))
            zt8 = ctx.enter_context(tc.tile_pool(name="zt8", bufs=4))
            gt8 = ctx.enter_context(tc.tile_pool(name="gt8", bufs=3))
            h1p = ctx.enter_context(tc.tile_pool(name="h1p", bufs=4))
            opool = ctx.enter_context(tc.tile_pool(name="opool", bufs=3))
            stats = ctx.enter_context(tc.tile_pool(name="stats", bufs=48))
            ps_att = ctx.enter_context(
                tc.tile_pool(name="ps_att", bufs=2, space="PSUM"))
            ps_g = ctx.enter_context(
                tc.tile_pool(name="ps_g", bufs=2, space="PSUM"))
            ps_f = ctx.enter_context(
                tc.tile_pool(name="ps_f", bufs=2, space="PSUM"))
            ps_t = ctx.enter_context(
                tc.tile_pool(name="ps_t", bufs=2, space="PSUM"))

            w_sb = {}
            worder = sorted(wts, key=lambda n: (n[0] != "w", n[0] != "u"))
            for name in worder:
                ap = wts[name]
                t = const.tile(list(ap.shape), ap.dtype, tag=f"w_{name}")
                q = nc.scalar if name.endswith("1") else nc.sync
                q.dma_start(out=t[...], in_=ap)
                w_sb[name] = t
            ident = const.tile([P, P], F16, tag="ident")
            from concourse.masks import make_identity
            make_identity(nc, ident[...])
            # scaled identity for residual-accumulate matmuls
            ident_sc = const.tile([P, P], F16, tag="ident_sc")
            nc.vector.tensor_scalar_mul(ident_sc[...], ident[...], WSCALE)
            # replicated affine tiles (only when needed)
            rep = {}
            for s in range(2):
                if aff_a[s]:
                    for nm in (f"ga{s}", f"ba{s}"):
                        r = const.tile([P, E], F16, tag=f"rep_{nm}")
                        nc.sync.dma_start(out=r[...],
                                          in_=wts[nm].to_broadcast((P, E)))
                        rep[nm] = r
                if aff_b[s]:
                    for nm in (f"gb{s}", f"bb{s}"):
                        r = const.tile([P, E], F32, tag=f"rep_{nm}")
                        nc.sync.dma_start(out=r[...],
                                          in_=wts[nm].to_broadcast((P, E)))
                        rep[nm] = r

            eps_sb = const.tile([P, 1], F32, tag="eps")
            nc.vector.memset(eps_sb[...], EPS)
            ones_sb = const.tile([1, R], BF16, tag="ones")
            nc.vector.memset(ones_sb[...], 1.0)

            def batch_scales(mv4):
                """mv4: [P, RC, 2] (mean, var) -> (inv4, nmi4) each [P, RC]."""
                inv4 = stats.tile([P, RC], F32, tag="inv4")
                nc.scalar.activation(out=inv4[...], in_=mv4[:, :, 1],
                                     func=Sqrt, bias=eps_sb[...], scale=1.0)
                nc.vector.reciprocal(out=inv4[...], in_=inv4[...])
                nmi4 = stats.tile([P, RC], F32, tag="nmi4")
                nc.vector.scalar_tensor_tensor(
                    out=nmi4[...], in0=mv4[:, :, 0], scalar=-1.0,
                    in1=inv4[...], op0=MULT, op1=MULT)
                return inv4, nmi4

            def front_load(mt):
                """prefetch x (fp16 row-major) + xT (fp8) for tile mt."""
                x_s, xT_s = [], []
                for s in range(2):
                    xt = xbf.tile([P, RC, E], F16, tag=f"xin{s}")
                    for rc in range(RC):
                        nc.gpsimd.dma_start(out=xt[:, rc, :],
                                            in_=xbf_d[s][mt, rc])
                    x8 = xt8.tile([P, KE, R], FP8, tag=f"xT{s}")
                    nc.gpsimd.dma_start(out=x8[...], in_=xt8_d[s][mt])
                    x_s.append(xt)
                    xT_s.append(x8)
                return x_s, xT_s

            def front_compute(mt, tiles):
                """attn (+residual via scaled identity) + LN1 -> z."""
                x_s, xT_s = tiles
                z_s, h1_s = [], []
                for s in range(2):
                    kvT = xT_s[1 - s]
                    z = zbf.tile([P, RC, E], F16, tag=f"z{s}")
                    h = hpool.tile([P, RC, E], F16, tag=f"h{s}", bufs=2)
                    mv4 = stats.tile([P, RC, 2], F32, tag="mv4")
                    for rc in range(RC):
                        ps = ps_att.tile([P, E], F32, tag="ps_att")
                        for c2 in range(KE // 2):
                            nc.tensor.matmul(
                                ps[...],
                                kvT[:, 2 * c2:2 * c2 + 2,
                                    rc * P:(rc + 1) * P],
                                w_sb[f"w{s}"][:, 2 * c2:2 * c2 + 2, :],
                                start=(c2 == 0), stop=False,
                                perf_mode=DR)
                        # residual: ps += WSCALE * x_q
                        nc.tensor.matmul(
                            ps[...], ident_sc[...], x_s[s][:, rc, :],
                            start=False, stop=(not use_c[s]),
                            skip_group_check=True)
                        if use_c[s]:
                            nc.tensor.matmul(ps[...], ones_sb[:, 0:P],
                                             w_sb[f"c{s}"][...],
                                             start=False, stop=True,
                                             skip_group_check=True)
                        # raw descale-evict to fp16 (alternate engines) frees
                        # the PSUM bank immediately; stats on fp16 follow.
                        if rc % 2 == 0:
                            nc.scalar.activation(out=h[:, rc, :], in_=ps[...],
                                                 func=Copy,
                                                 scale=1.0 / WSCALE)
                        else:
                            nc.vector.tensor_scalar_mul(h[:, rc, :], ps[...],
                                                        1.0 / WSCALE)
                        st6 = stats.tile([P, 6], F32, tag="st6")
                        nc.vector.bn_stats(out=st6[...], in_=h[:, rc, :])
                        nc.vector.bn_aggr(out=mv4[:, rc, :], in_=st6[...])
                    # batched [P,RC] scale chain: one sqrt/recip/stt for all
                    # RC chunks, then per-chunk normalize on gpsimd.
                    inv4, nmi4 = batch_scales(mv4)
                    for rc in range(RC):
                        nc.gpsimd.tensor_scalar(
                            out=z[:, rc, :], in0=h[:, rc, :],
                            scalar1=inv4[:, rc:rc + 1],
                            scalar2=nmi4[:, rc:rc + 1],
                            op0=MULT, op1=ADD)
                    if aff_a[s]:
                        h1 = h1p.tile([P, RC, E], F16, tag=f"h1{s}")
                        for rc in range(RC):
                            nc.vector.tensor_mul(h1[:, rc, :], z[:, rc, :],
                                                 rep[f"ga{s}"][...])
                            nc.vector.tensor_add(h1[:, rc, :], h1[:, rc, :],
                                                 rep[f"ba{s}"][...])
                        h1_s.append(h1)
                    else:
                        h1_s.append(z)
                    z_s.append(z)
                return z_s, h1_s

            def back_a(mt, z_s):
                """zT transpose (TensorE) + FFN1 + relu -> gt_s."""
                gt_s = []
                for s in range(2):
                    z = z_s[s]
                    zT = zt8.tile([P, KE, R], FP8, tag=f"zT{s}")
                    for c in range(KE):
                        pt = ps_t.tile([P, R], F16, tag="ps_t")
                        for rc in range(RC):
                            nc.tensor.transpose(
                                pt[:, rc * P:(rc + 1) * P],
                                z[:, rc, c * P:(c + 1) * P],
                                ident[...])
                        if c % 2 == 0:
                            nc.vector.tensor_copy(out=zT[:, c, :],
                                                  in_=pt[...])
                        else:
                            nc.scalar.copy(out=zT[:, c, :], in_=pt[...])
                    gt = gt8.tile([P, KH, R], FP8, tag=f"gt{s}")
                    for j in range(KH):
                        pg = ps_g.tile([P, R], F32, tag="ps_g")
                        for c2 in range(KE // 2):
                            nc.tensor.matmul(
                                pg[...],
                                w_sb[f"u{s}"][:, 2 * c2:2 * c2 + 2,
                                              j * P:(j + 1) * P],
                                zT[:, 2 * c2:2 * c2 + 2, :],
                                start=(c2 == 0),
                                stop=(c2 == KE // 2 - 1 and not use_d[s]),
                                perf_mode=DR)
                        if use_d[s]:
                            nc.tensor.matmul(
                                pg[...], w_sb[f"d{s}"][:, j * P:(j + 1) * P],
                                ones_sb[:, 0:R], start=False, stop=True,
                                skip_group_check=True)
                        nc.scalar.activation(out=gt[:, j, :], in_=pg[...],
                                             func=Relu, scale=1.0 / WSCALE)
                    gt_s.append(gt)
                return gt_s

            def back_b(mt, gt_s, h1_s):
                """FFN2 + residual + LN2 + output DMA."""
                for s in range(2):
                    gt = gt_s[s]
                    h1 = h1_s[s]
                    y = hpool.tile([P, RC, E], F16, tag=f"y{s}", bufs=2)
                    mv4 = stats.tile([P, RC, 2], F32, tag="mv4b")
                    for rc in range(RC):
                        pf = ps_f.tile([P, E], F32, tag="ps_f")
                        for j2 in range(KH // 2):
                            nc.tensor.matmul(
                                pf[...],
                                gt[:, 2 * j2:2 * j2 + 2,
                                   rc * P:(rc + 1) * P],
                                w_sb[f"v{s}"][:, 2 * j2:2 * j2 + 2, :],
                                start=(j2 == 0), stop=False,
                                perf_mode=DR)
                        # residual: pf += WSCALE * h1
                        nc.tensor.matmul(
                            pf[...], ident_sc[...], h1[:, rc, :],
                            start=False, stop=(not use_e[s]),
                            skip_group_check=True)
                        if use_e[s]:
                            nc.tensor.matmul(pf[...], ones_sb[:, 0:P],
                                             w_sb[f"e{s}"][...],
                                             start=False, stop=True,
                                             skip_group_check=True)
                        if rc % 2 == 0:
                            nc.scalar.activation(out=y[:, rc, :], in_=pf[...],
                                                 func=Copy,
                                                 scale=1.0 / WSCALE)
                        else:
                            nc.vector.tensor_scalar_mul(y[:, rc, :], pf[...],
                                                        1.0 / WSCALE)
                        st6 = stats.tile([P, 6], F32, tag="st6")
                        nc.vector.bn_stats(out=st6[...], in_=y[:, rc, :])
                        nc.vector.bn_aggr(out=mv4[:, rc, :], in_=st6[...])
                    inv4, nmi4 = batch_scales(mv4)
                    for rc in range(RC):
                        o = opool.tile([P, E], F32, tag="o")
                        nc.gpsimd.tensor_scalar(
                            out=o[...], in0=y[:, rc, :],
                            scalar1=inv4[:, rc:rc + 1],
                            scalar2=nmi4[:, rc:rc + 1],
                            op0=MULT, op1=ADD)
                        if aff_b[s]:
                            nc.vector.tensor_mul(o[...], o[...],
                                                 rep[f"gb{s}"][...])
                            nc.vector.tensor_add(o[...], o[...],
                                                 rep[f"bb{s}"][...])
                        nc.sync.dma_start(
                            out=out[mt, rc, :, s * E:(s + 1) * E], in_=o[...])

            # software pipeline, emission [F(mt+1), load(mt+2), A(mt),
            # B(mt)] per cycle: back_a(mt) consumes a z finished a full
            # cycle earlier (no transpose stall on the LN1 norm chain),
            # input DMAs prefetch two tiles ahead, and this tile's
            # relu/cast evictions stay near the head of the in-order
            # scalar/vector queues.
            tiles, fronts = {}, {}
            for k in range(min(2, NT)):
                tiles[k] = front_load(k)
            for k in range(min(2, NT)):
                fronts[k] = front_compute(k, tiles.pop(k))
                if k + 2 < NT:
                    tiles[k + 2] = front_load(k + 2)
            for mt in range(NT):
                if mt + 2 < NT:
                    fronts[mt + 2] = front_compute(mt + 2, tiles.pop(mt + 2))
                if mt + 3 < NT:
                    tiles[mt + 3] = front_load(mt + 3)
                z_cur, h1_cur = fronts.pop(mt)
                gt_s = back_a(mt, z_cur)
                back_b(mt, gt_s, h1_cur)

    nc.compile()
    return nc


def _prep_host(inputs):
    """Fold weights host-side; returns (full arrays, weight map, flags)."""
    g = {k: np.asarray(v, dtype=np.float32) for k, v in inputs.items()}

    def trivial(a, val):
        return bool(np.all(a == val))

    def kchunks(a, nk, dt):
        # [K, N] -> [P, nk, N] (chunk c = rows c*P:(c+1)*P)
        k, n = a.shape
        assert k == nk * P
        return np.ascontiguousarray(
            a.reshape(nk, P, n).transpose(1, 0, 2)).astype(dt)

    arrs = {}
    flags = []
    for s, (aw, ab, ow, ob, lna_g, lna_b, lnb_g, lnb_b, w1, b1, w2, b2) in \
            enumerate((
                (g["a1_in_w"], g["a1_in_b"], g["a1_out_w"], g["a1_out_b"],
                 g["ln1_g"], g["ln1_b"], g["ln3_g"], g["ln3_b"],
                 g["f1_w1"], g["f1_b1"], g["f1_w2"], g["f1_b2"]),
                (g["a2_in_w"], g["a2_in_b"], g["a2_out_w"], g["a2_out_b"],
                 g["ln2_g"], g["ln2_b"], g["ln4_g"], g["ln4_b"],
                 g["f2_w1"], g["f2_b1"], g["f2_w2"], g["f2_b2"]))):
        wv = aw[2 * E:3 * E]
        bv = ab[2 * E:3 * E]
        W = ow @ wv                      # [E, E]; attn = kv @ W.T + c
        c = ow @ bv + ob                 # [E]
        U = w1 * lna_g[None, :]          # LN1 gain folded into FFN1
        d = b1 + w1 @ lna_b              # LN1 bias folded into FFN1 bias
        V = w2                           # [E, HID]
        e = b2                           # [E]
        arrs[f"w{s}"] = kchunks(W.T * WSCALE, E // P, F8)
        arrs[f"u{s}"] = kchunks(U.T * WSCALE, E // P, F8)
        arrs[f"v{s}"] = kchunks(V.T * WSCALE, HID // P, F8)
        uc = not trivial(c, 0.0)
        ud = not trivial(d, 0.0)
        ue = not trivial(e, 0.0)
        fa = not (trivial(lna_g, 1.0) and trivial(lna_b, 0.0))
        fb = not (trivial(lnb_g, 1.0) and trivial(lnb_b, 0.0))
        if uc:
            arrs[f"c{s}"] = (c * WSCALE).reshape(1, E).astype(BF)
        if ud:
            arrs[f"d{s}"] = (d * WSCALE).reshape(1, HID).astype(BF)
        if ue:
            arrs[f"e{s}"] = (e * WSCALE).reshape(1, E).astype(BF)
        if fa:
            arrs[f"ga{s}"] = lna_g.reshape(1, E).astype(F16NP)
            arrs[f"ba{s}"] = lna_b.reshape(1, E).astype(F16NP)
        if fb:
            arrs[f"gb{s}"] = lnb_g.reshape(1, E).astype(np.float32)
            arrs[f"bb{s}"] = lnb_b.reshape(1, E).astype(np.float32)
        flags.append((uc, ud, ue, fa, fb))

    (uc0, ud0, ue0, fa0, fb0), (uc1, ud1, ue1, fa1, fb1) = flags
    flag_t = (uc0, uc1, ud0, ud1, ue0, ue1, fa0, fa1, fb0, fb1)
    return g, arrs, flag_t


def _make_in_maps(g, arrs, rows_per_core, rmacro):
    NT = rows_per_core // rmacro
    RC = rmacro // P
    KE = E // P
    in_maps = [dict(arrs) for _ in range(NCORES)]
    for s, key in enumerate(("dna", "mol")):
        x = g[key]
        xb = x.astype(F16NP).reshape(NCORES, NT, RC, P, E)
        # xT8[mt, p, c, r] = x[mt*R + r, c*P + p]
        x8 = np.ascontiguousarray(x.T.astype(F8).reshape(
            KE, P, NCORES, NT, rmacro).transpose(2, 3, 1, 0, 4))
        for i in range(NCORES):
            in_maps[i][f"x{s}"] = np.ascontiguousarray(xb[i])
            in_maps[i][f"xt{s}"] = x8[i]
    return in_maps


def _get_program(inputs):
    g, arrs, flag_t = _prep_host(inputs)
    B = g["dna"].shape[0]
    rows_per_core = B // NCORES
    rmacro = min(512, rows_per_core)
    key = (rows_per_core, rmacro, flag_t)
    if key not in _prog_cache:
        _prog_cache[key] = _build_program(rows_per_core, rmacro, flag_t)
    nc = _prog_cache[key]
    in_maps = _make_in_maps(g, arrs, rows_per_core, rmacro)
    return nc, in_maps, rows_per_core


def kernel(**inputs):
    nc, in_maps, rows_per_core = _get_program(inputs)
    res = run_bass_kernel_spmd(nc, in_maps, list(range(NCORES)))
    outs = [r["out"].reshape(rows_per_core, 2 * E) for r in res.results]
    return np.concatenate(outs, axis=0)


# revision 23
# speedup vs baseline: 2.2478x; 1.0390x over previous
"""Trainium2 Bass kernel for nn_CrossAttention (seq_len==1 cross attention,
dual-stream transformer block pair).

Math notes (exact simplifications, valid for any input values):
  - Both attentions have seq_len==1 for q and kv, so softmax over the single
    kv position is exactly 1.0 and attention output == V projection:
        mha(q_in, kv_in) = (kv_in @ wv.T + bv) @ out_w.T + out_b
    The q/k projections are dead code.  Folding the two matmuls:
        attn = kv_in @ (out_w @ wv).T + (out_w @ bv + out_b)
  - LayerNorm affine (g, b) of ln1/ln2 is folded into the following FFN
    weights host-side; residual-path affine and biases are applied on-device
    only when they are non-trivial (they are zeros/ones for the reference
    setup_inputs, so the fast path emits no extra instructions).

Implementation (v2, fp8 DoubleRow + host transposes):
  - Inputs per core (host-prepped): x_bf (row-major bf16, residuals),
    xT8 (feature-major fp8, attention moving/stationary operand).
  - All GEMM weights are scaled by 64 and cast to fp8e4 host-side; matmuls
    run in DoubleRow perf mode (contract 256 K per instruction -> 2x).
  - Residual adds ride on the PE: psum += 64*I @ x (scaled-identity
    stationary, bf16 moving operand), so LayerNorm stats (vector bn_stats)
    read PSUM directly and the normalize is fused into the PSUM->SBUF
    eviction on the scalar engine: z = (ps - m')*inv' with
    inv' = rsqrt(var(ps) + 64^2*eps) handling the descale exactly.
  - z transposed on TensorE (identity matmul) for FFN1's moving operand;
    evicted to fp8.  FFN1 relu eviction emits fp8 g^T which is directly
    FFN2's DoubleRow stationary operand.
"""

import numpy as np
import ml_dtypes
from contextlib import ExitStack

import concourse.bass as bass
import concourse.tile as tile
from concourse import bacc, mybir
from concourse.bass_utils import run_bass_kernel_spmd

E = 512
HID = 1024
NCORES = 8
EPS = 1e-5
P = 128
WSCALE = 64.0  # fp8 weight pre-scale (power of 2; descale folded into LN)

BF16 = mybir.dt.bfloat16
F16 = mybir.dt.float16
F32 = mybir.dt.float32
FP8 = mybir.dt.float8e4
BF = ml_dtypes.bfloat16
F16NP = np.float16
F8 = ml_dtypes.float8_e4m3  # matches TRN FP8_EXP4 (max 240, inf at 1111.000)

_prog_cache = {}


def _build_program(rows_per_core: int, rmacro: int, flags: tuple):
    """Build + compile the per-core Bass program.

    flags = (use_c0, use_c1, use_d0, use_d1, use_e0, use_e1,
             aff_a0, aff_a1, aff_b0, aff_b1)
    """
    (use_c0, use_c1, use_d0, use_d1, use_e0, use_e1,
     aff_a0, aff_a1, aff_b0, aff_b1) = flags
    use_c = (use_c0, use_c1)
    use_d = (use_d0, use_d1)
    use_e = (use_e0, use_e1)
    aff_a = (aff_a0, aff_a1)
    aff_b = (aff_b0, aff_b1)

    R = rmacro
    NT = rows_per_core // R
    RC = R // P
    KE = E // P    # 4 K-chunks over E
    KH = HID // P  # 8 K-chunks over HID
    EPS_EFF = EPS * WSCALE * WSCALE
    DR = mybir.MatmulPerfMode.DoubleRow

    nc = bacc.Bacc("TRN2", target_bir_lowering=False, debug=False,
                   num_devices=NCORES)

    xbf_d, xt8_d = [], []
    for s, nm in enumerate(("dna", "mol")):
        xbf_d.append(nc.dram_tensor(f"x{s}", [NT, RC, P, E], F16,
                                    kind="ExternalInput").ap())
        xt8_d.append(nc.dram_tensor(f"xt{s}", [NT, P, KE, R], FP8,
                                    kind="ExternalInput").ap())
    out = nc.dram_tensor("out", [NT, RC, P, 2 * E], F32,
                         kind="ExternalOutput").ap()

    wts = {}
    for s in range(2):
        wts[f"w{s}"] = nc.dram_tensor(f"w{s}", [P, KE, E], FP8,
                                      kind="ExternalInput").ap()
        wts[f"u{s}"] = nc.dram_tensor(f"u{s}", [P, KE, HID], FP8,
                                      kind="ExternalInput").ap()
        wts[f"v{s}"] = nc.dram_tensor(f"v{s}", [P, KH, E], FP8,
                                      kind="ExternalInput").ap()
        if use_c[s]:
            wts[f"c{s}"] = nc.dram_tensor(f"c{s}", [1, E], BF16,
                                          kind="ExternalInput").ap()
        if use_d[s]:
            wts[f"d{s}"] = nc.dram_tensor(f"d{s}", [1, HID], BF16,
                                          kind="ExternalInput").ap()
        if use_e[s]:
            wts[f"e{s}"] = nc.dram_tensor(f"e{s}", [1, E], BF16,
                                          kind="ExternalInput").ap()
        if aff_a[s]:
            wts[f"ga{s}"] = nc.dram_tensor(f"ga{s}", [1, E], F16,
                                           kind="ExternalInput").ap()
            wts[f"ba{s}"] = nc.dram_tensor(f"ba{s}", [1, E], F16,
                                           kind="ExternalInput").ap()
        if aff_b[s]:
            wts[f"gb{s}"] = nc.dram_tensor(f"gb{s}", [1, E], F32,
                                           kind="ExternalInput").ap()
            wts[f"bb{s}"] = nc.dram_tensor(f"bb{s}", [1, E], F32,
                                           kind="ExternalInput").ap()

    MULT = mybir.AluOpType.mult
    ADD = mybir.AluOpType.add
    Copy = mybir.ActivationFunctionType.Copy
    Relu = mybir.ActivationFunctionType.Relu
    Sqrt = mybir.ActivationFunctionType.Sqrt
    Ident = mybir.ActivationFunctionType.Identity

    with tile.TileContext(nc) as tc:
        with ExitStack() as ctx:
            const = ctx.enter_context(tc.tile_pool(name="const", bufs=1))
            xbf = ctx.enter_context(tc.tile_pool(name="xbf", bufs=4))
            xt8 = ctx.enter_context(tc.tile_pool(name="xt8", bufs=4))
            zbf = ctx.enter_context(tc.tile_pool(name="zbf", bufs=4))
            hpool = ctx.enter_context(tc.tile_pool(name="hpool", bufs=4))
            zt8 = ctx.enter_context(tc.tile_pool(name="zt8", bufs=4))
            gt8 = ctx.enter_context(tc.tile_pool(name="gt8", bufs=3))
            h1p = ctx.enter_context(tc.tile_pool(name="h1p", bufs=4))
            opool = ctx.enter_context(tc.tile_pool(name="opool", bufs=3))
            stats = ctx.enter_context(tc.tile_pool(name="stats", bufs=48))
            ps_att = ctx.enter_context(
                tc.tile_pool(name="ps_att", bufs=2, space="PSUM"))
            ps_g = ctx.enter_context(
                tc.tile_pool(name="ps_g", bufs=2, space="PSUM"))
            ps_f = ctx.enter_context(
                tc.tile_pool(name="ps_f", bufs=2, space="PSUM"))
            ps_t = ctx.enter_context(
                tc.tile_pool(name="ps_t", bufs=2, space="PSUM"))

            w_sb = {}
            worder = sorted(wts, key=lambda n: (n[0] != "w", n[0] != "u"))
            for name in worder:
                ap = wts[name]
                t = const.tile(list(ap.shape), ap.dtype, tag=f"w_{name}")
                q = nc.scalar if name.endswith("1") else nc.sync
                q.dma_start(out=t[...], in_=ap)
                w_sb[name] = t
            ident = const.tile([P, P], F16, tag="ident")
            from concourse.masks import make_identity
            make_identity(nc, ident[...])
            # scaled identity for residual-accumulate matmuls
            ident_sc = const.tile([P, P], F16, tag="ident_sc")
            nc.vector.tensor_scalar_mul(ident_sc[...], ident[...], WSCALE)
            # replicated affine tiles (only when needed)
            rep = {}
            for s in range(2):
                if aff_a[s]:
                    for nm in (f"ga{s}", f"ba{s}"):
                        r = const.tile([P, E], F16, tag=f"rep_{nm}")
                        nc.sync.dma_start(out=r[...],
                                          in_=wts[nm].to_broadcast((P, E)))
                        rep[nm] = r
                if aff_b[s]:
                    for nm in (f"gb{s}", f"bb{s}"):
                        r = const.tile([P, E], F32, tag=f"rep_{nm}")
                        nc.sync.dma_start(out=r[...],
                                          in_=wts[nm].to_broadcast((P, E)))
                        rep[nm] = r

            eps_sb = const.tile([P, 1], F32, tag="eps")
            nc.vector.memset(eps_sb[...], EPS)
            ones_sb = const.tile([1, R], BF16, tag="ones")
            nc.vector.memset(ones_sb[...], 1.0)

            def batch_scales(mv4):
                """mv4: [P, RC, 2] (mean, var) -> (inv4, nmi4) each [P, RC]."""
                inv4 = stats.tile([P, RC], F32, tag="inv4")
                nc.scalar.activation(out=inv4[...], in_=mv4[:, :, 1],
                                     func=Sqrt, bias=eps_sb[...], scale=1.0)
                nc.vector.reciprocal(out=inv4[...], in_=inv4[...])
                nmi4 = stats.tile([P, RC], F32, tag="nmi4")
                nc.vector.scalar_tensor_tensor(
                    out=nmi4[...], in0=mv4[:, :, 0], scalar=-1.0,
                    in1=inv4[...], op0=MULT, op1=MULT)
                return inv4, nmi4

            def front_load(mt):
                """prefetch x (fp16 row-major) + xT (fp8) for tile mt."""
                x_s, xT_s = [], []
                for s in range(2):
                    xt = xbf.tile([P, RC, E], F16, tag=f"xin{s}")
                    for rc in range(RC):
                        nc.sync.dma_start(out=xt[:, rc, :],
                                          in_=xbf_d[s][mt, rc])
                    x8 = xt8.tile([P, KE, R], FP8, tag=f"xT{s}")
                    nc.sync.dma_start(out=x8[...], in_=xt8_d[s][mt])
                    x_s.append(xt)
                    xT_s.append(x8)
                return x_s, xT_s

            def front_compute(mt, tiles):
                """attn (+residual via scaled identity) + LN1 -> z."""
                x_s, xT_s = tiles
                z_s, h1_s = [], []
                for s in range(2):
                    kvT = xT_s[1 - s]
                    z = zbf.tile([P, RC, E], F16, tag=f"z{s}")
                    h = hpool.tile([P, RC, E], F16, tag=f"h{s}", bufs=2)
                    mv4 = stats.tile([P, RC, 2], F32, tag="mv4")
                    for rc in range(RC):
                        ps = ps_att.tile([P, E], F32, tag="ps_att")
                        for c2 in range(KE // 2):
                            nc.tensor.matmul(
                                ps[...],
                                kvT[:, 2 * c2:2 * c2 + 2,
                                    rc * P:(rc + 1) * P],
                                w_sb[f"w{s}"][:, 2 * c2:2 * c2 + 2, :],
                                start=(c2 == 0), stop=False,
                                perf_mode=DR)
                        # residual: ps += WSCALE * x_q
                        nc.tensor.matmul(
                            ps[...], ident_sc[...], x_s[s][:, rc, :],
                            start=False, stop=(not use_c[s]),
                            skip_group_check=True)
                        if use_c[s]:
                            nc.tensor.matmul(ps[...], ones_sb[:, 0:P],
                                             w_sb[f"c{s}"][...],
                                             start=False, stop=True,
                                             skip_group_check=True)
                        # raw descale-evict to fp16 (alternate engines) frees
                        # the PSUM bank immediately; stats on fp16 follow.
                        if rc % 2 == 0:
                            nc.scalar.activation(out=h[:, rc, :], in_=ps[...],
                                                 func=Copy,
                                                 scale=1.0 / WSCALE)
                        else:
                            nc.vector.tensor_scalar_mul(h[:, rc, :], ps[...],
                                                        1.0 / WSCALE)
                        st6 = stats.tile([P, 6], F32, tag="st6")
                        nc.vector.bn_stats(out=st6[...], in_=h[:, rc, :])
                        nc.vector.bn_aggr(out=mv4[:, rc, :], in_=st6[...])
                    # batched [P,RC] scale chain: one sqrt/recip/stt for all
                    # RC chunks, then per-chunk normalize on gpsimd.
                    inv4, nmi4 = batch_scales(mv4)
                    for rc in range(RC):
                        nc.gpsimd.tensor_scalar(
                            out=z[:, rc, :], in0=h[:, rc, :],
                            scalar1=inv4[:, rc:rc + 1],
                            scalar2=nmi4[:, rc:rc + 1],
                            op0=MULT, op1=ADD)
                    if aff_a[s]:
                        h1 = h1p.tile([P, RC, E], F16, tag=f"h1{s}")
                        for rc in range(RC):
                            nc.vector.tensor_mul(h1[:, rc, :], z[:, rc, :],
                                                 rep[f"ga{s}"][...])
                            nc.vector.tensor_add(h1[:, rc, :], h1[:, rc, :],
                                                 rep[f"ba{s}"][...])
                        h1_s.append(h1)
                    else:
                        h1_s.append(z)
                    z_s.append(z)
                return z_s, h1_s

            def back_a_s(mt, z_s, s):
                """zT transpose (TensorE) + FFN1 + relu for one stream."""
                if True:
                    z = z_s[s]
                    zT = zt8.tile([P, KE, R], FP8, tag=f"zT{s}")
                    for c in range(KE):
                        pt = ps_t.tile([P, R], F16, tag="ps_t")
                        for rc in range(RC):
                            nc.tensor.transpose(
                                pt[:, rc * P:(rc + 1) * P],
                                z[:, rc, c * P:(c + 1) * P],
                                ident[...])
                        if c % 2 == 0:
                            nc.vector.tensor_copy(out=zT[:, c, :],
                                                  in_=pt[...])
                        else:
                            nc.scalar.copy(out=zT[:, c, :], in_=pt[...])
                    gt = gt8.tile([P, KH, R], FP8, tag=f"gt{s}")
                    for j in range(KH):
                        pg = ps_g.tile([P, R], F32, tag="ps_g")
                        for c2 in range(KE // 2):
                            nc.tensor.matmul(
                                pg[...],
                                w_sb[f"u{s}"][:, 2 * c2:2 * c2 + 2,
                                              j * P:(j + 1) * P],
                                zT[:, 2 * c2:2 * c2 + 2, :],
                                start=(c2 == 0),
                                stop=(c2 == KE // 2 - 1 and not use_d[s]),
                                perf_mode=DR)
                        if use_d[s]:
                            nc.tensor.matmul(
                                pg[...], w_sb[f"d{s}"][:, j * P:(j + 1) * P],
                                ones_sb[:, 0:R], start=False, stop=True,
                                skip_group_check=True)
                        nc.scalar.activation(out=gt[:, j, :], in_=pg[...],
                                             func=Relu, scale=1.0 / WSCALE)
                    return gt

            def back_b_s(mt, gt, h1_s, s):
                """FFN2 + residual + LN2 + output DMA for one stream."""
                if True:
                    h1 = h1_s[s]
                    y = hpool.tile([P, RC, E], F16, tag=f"y{s}", bufs=2)
                    mv4 = stats.tile([P, RC, 2], F32, tag="mv4b")
                    for rc in range(RC):
                        pf = ps_f.tile([P, E], F32, tag="ps_f")
                        for j2 in range(KH // 2):
                            nc.tensor.matmul(
                                pf[...],
                                gt[:, 2 * j2:2 * j2 + 2,
                                   rc * P:(rc + 1) * P],
                                w_sb[f"v{s}"][:, 2 * j2:2 * j2 + 2, :],
                                start=(j2 == 0), stop=False,
                                perf_mode=DR)
                        # residual: pf += WSCALE * h1
                        nc.tensor.matmul(
                            pf[...], ident_sc[...], h1[:, rc, :],
                            start=False, stop=(not use_e[s]),
                            skip_group_check=True)
                        if use_e[s]:
                            nc.tensor.matmul(pf[...], ones_sb[:, 0:P],
                                             w_sb[f"e{s}"][...],
                                             start=False, stop=True,
                                             skip_group_check=True)
                        if rc % 2 == 0:
                            nc.scalar.activation(out=y[:, rc, :], in_=pf[...],
                                                 func=Copy,
                                                 scale=1.0 / WSCALE)
                        else:
                            nc.vector.tensor_scalar_mul(y[:, rc, :], pf[...],
                                                        1.0 / WSCALE)
                        st6 = stats.tile([P, 6], F32, tag="st6")
                        nc.vector.bn_stats(out=st6[...], in_=y[:, rc, :])
                        nc.vector.bn_aggr(out=mv4[:, rc, :], in_=st6[...])
                    inv4, nmi4 = batch_scales(mv4)
                    for rc in range(RC):
                        o = opool.tile([P, E], F32, tag="o")
                        nc.gpsimd.tensor_scalar(
                            out=o[...], in0=y[:, rc, :],
                            scalar1=inv4[:, rc:rc + 1],
                            scalar2=nmi4[:, rc:rc + 1],
                            op0=MULT, op1=ADD)
                        if aff_b[s]:
                            nc.vector.tensor_mul(o[...], o[...],
                                                 rep[f"gb{s}"][...])
                            nc.vector.tensor_add(o[...], o[...],
                                                 rep[f"bb{s}"][...])
                        nc.sync.dma_start(
                            out=out[mt, rc, :, s * E:(s + 1) * E], in_=o[...])

            # software pipeline, emission [F(mt+1), load(mt+2), A(mt),
            # B(mt)] per cycle: back_a(mt) consumes a z finished a full
            # cycle earlier (no transpose stall on the LN1 norm chain),
            # input DMAs prefetch two tiles ahead, and this tile's
            # relu/cast evictions stay near the head of the in-order
            # scalar/vector queues.
            tiles, fronts = {}, {}
            for k in range(min(2, NT)):
                tiles[k] = front_load(k)
            for k in range(min(2, NT)):
                fronts[k] = front_compute(k, tiles.pop(k))
                if k + 2 < NT:
                    tiles[k + 2] = front_load(k + 2)
            for mt in range(NT):
                if mt + 2 < NT:
                    fronts[mt + 2] = front_compute(mt + 2, tiles.pop(mt + 2))
                if mt + 3 < NT:
                    tiles[mt + 3] = front_load(mt + 3)
                z_cur, h1_cur = fronts.pop(mt)
                if mt + 1 < NT:
                    gt0 = back_a_s(mt, z_cur, 0)
                    gt1 = back_a_s(mt, z_cur, 1)
                    back_b_s(mt, gt0, h1_cur, 0)
                    back_b_s(mt, gt1, h1_cur, 1)
                else:
                    # drain tile: overlap stream 0's LN2 tail with stream 1
                    gt0 = back_a_s(mt, z_cur, 0)
                    back_b_s(mt, gt0, h1_cur, 0)
                    gt1 = back_a_s(mt, z_cur, 1)
                    back_b_s(mt, gt1, h1_cur, 1)

    nc.compile()
    return nc


def _prep_host(inputs):
    """Fold weights host-side; returns (full arrays, weight map, flags)."""
    g = {k: np.asarray(v, dtype=np.float32) for k, v in inputs.items()}

    def trivial(a, val):
        return bool(np.all(a == val))

    def kchunks(a, nk, dt):
        # [K, N] -> [P, nk, N] (chunk c = rows c*P:(c+1)*P)
        k, n = a.shape
        assert k == nk * P
        return np.ascontiguousarray(
            a.reshape(nk, P, n).transpose(1, 0, 2)).astype(dt)

    arrs = {}
    flags = []
    for s, (aw, ab, ow, ob, lna_g, lna_b, lnb_g, lnb_b, w1, b1, w2, b2) in \
            enumerate((
                (g["a1_in_w"], g["a1_in_b"], g["a1_out_w"], g["a1_out_b"],
                 g["ln1_g"], g["ln1_b"], g["ln3_g"], g["ln3_b"],
                 g["f1_w1"], g["f1_b1"], g["f1_w2"], g["f1_b2"]),
                (g["a2_in_w"], g["a2_in_b"], g["a2_out_w"], g["a2_out_b"],
                 g["ln2_g"], g["ln2_b"], g["ln4_g"], g["ln4_b"],
                 g["f2_w1"], g["f2_b1"], g["f2_w2"], g["f2_b2"]))):
        wv = aw[2 * E:3 * E]
        bv = ab[2 * E:3 * E]
        W = ow @ wv                      # [E, E]; attn = kv @ W.T + c
        c = ow @ bv + ob                 # [E]
        U = w1 * lna_g[None, :]          # LN1 gain folded into FFN1
        d = b1 + w1 @ lna_b              # LN1 bias folded into FFN1 bias
        V = w2                           # [E, HID]
        e = b2                           # [E]
        arrs[f"w{s}"] = kchunks(W.T * WSCALE, E // P, F8)
        arrs[f"u{s}"] = kchunks(U.T * WSCALE, E // P, F8)
        arrs[f"v{s}"] = kchunks(V.T * WSCALE, HID // P, F8)
        uc = not trivial(c, 0.0)
        ud = not trivial(d, 0.0)
        ue = not trivial(e, 0.0)
        fa = not (trivial(lna_g, 1.0) and trivial(lna_b, 0.0))
        fb = not (trivial(lnb_g, 1.0) and trivial(lnb_b, 0.0))
        if uc:
            arrs[f"c{s}"] = (c * WSCALE).reshape(1, E).astype(BF)
        if ud:
            arrs[f"d{s}"] = (d * WSCALE).reshape(1, HID).astype(BF)
        if ue:
            arrs[f"e{s}"] = (e * WSCALE).reshape(1, E).astype(BF)
        if fa:
            arrs[f"ga{s}"] = lna_g.reshape(1, E).astype(F16NP)
            arrs[f"ba{s}"] = lna_b.reshape(1, E).astype(F16NP)
        if fb:
            arrs[f"gb{s}"] = lnb_g.reshape(1, E).astype(np.float32)
            arrs[f"bb{s}"] = lnb_b.reshape(1, E).astype(np.float32)
        flags.append((uc, ud, ue, fa, fb))

    (uc0, ud0, ue0, fa0, fb0), (uc1, ud1, ue1, fa1, fb1) = flags
    flag_t = (uc0, uc1, ud0, ud1, ue0, ue1, fa0, fa1, fb0, fb1)
    return g, arrs, flag_t


def _make_in_maps(g, arrs, rows_per_core, rmacro):
    NT = rows_per_core // rmacro
    RC = rmacro // P
    KE = E // P
    in_maps = [dict(arrs) for _ in range(NCORES)]
    for s, key in enumerate(("dna", "mol")):
        x = g[key]
        xb = x.astype(F16NP).reshape(NCORES, NT, RC, P, E)
        # xT8[mt, p, c, r] = x[mt*R + r, c*P + p]
        x8 = np.ascontiguousarray(x.T.astype(F8).reshape(
            KE, P, NCORES, NT, rmacro).transpose(2, 3, 1, 0, 4))
        for i in range(NCORES):
            in_maps[i][f"x{s}"] = np.ascontiguousarray(xb[i])
            in_maps[i][f"xt{s}"] = x8[i]
    return in_maps


def _get_program(inputs):
    g, arrs, flag_t = _prep_host(inputs)
    B = g["dna"].shape[0]
    rows_per_core = B // NCORES
    rmacro = min(512, rows_per_core)
    key = (rows_per_core, rmacro, flag_t)
    if key not in _prog_cache:
        _prog_cache[key] = _build_program(rows_per_core, rmacro, flag_t)
    nc = _prog_cache[key]
    in_maps = _make_in_maps(g, arrs, rows_per_core, rmacro)
    return nc, in_maps, rows_per_core


def kernel(**inputs):
    nc, in_maps, rows_per_core = _get_program(inputs)
    res = run_bass_kernel_spmd(nc, in_maps, list(range(NCORES)))
    outs = [r["out"].reshape(rows_per_core, 2 * E) for r in res.results]
    return np.concatenate(outs, axis=0)
